# revision 19
# baseline (speedup 1.0000x reference)
"""Trainium2 Bass kernel for a transformer MiniBlock (B=4, T=2048, C=1024, 16 heads,
causal attention, 4x FFN), sharded over 8 NeuronCores.

Sharding: core = (batch b=core//2, role r=core%2). Each core runs the full block for
1024 of its batch's 2048 tokens (two 512-token chunks, balanced for causal work:
role 0 owns chunks {0,3}, role 1 owns {1,2}), computing K/V over the full sequence
(no cross-core communication). The program is SPMD-uniform: k-window loop bounds are
per-slot maxima over roles; per-core causal masks (input data) zero the difference.

All tensors are bf16 on-chip (fp32 PSUM accumulation), which doubles effective
SBUF/DMA capacity, enables fast weight loads, and keeps rel-err ~3e-3. Activations
stay channel-major end to end; LN stats / softmax sums / broadcasts use small
ones-matmuls; the attention softmax is computed k-major with a ones-column appended
to V so denominators fall out of the AV matmul. Weights are pre-tiled on the host so
every weight DMA is fully contiguous. V stays resident in SBUF (no DRAM spill).
Even/odd head score matmuls are row-packed (tile_position) to run concurrently, and
exp is a single 1024-wide activation spanning two PSUM banks.
"""
import sys

sys.path.insert(0, "/opt/trn_rl_repo")

import numpy as np
from contextlib import ExitStack

import concourse.bacc as bacc
import concourse.mybir as mybir
import concourse.tile as tile
from concourse.tile import add_dep_helper

F32 = mybir.dt.float32
BF = mybir.dt.bfloat16
AF = mybir.ActivationFunctionType
ALU = mybir.AluOpType

P = 128
T = 2048          # full sequence
C = 1024          # embedding
NQ = 1024         # query tokens per core
H4 = 4096         # ffn hidden
NPAIR = 8         # head pairs (2 heads of 64 dims = 128 channels)
KC = C // P       # 8 channel tiles
NSLOT = 2         # 512-token query chunks per core
CW = 512          # chunk width
NKT_PROG = [8, 16]            # k-tiles per slot (program constant, max over roles)
GSETS = [[0, 3], [1, 2]]      # global 512-chunk index per slot, per role
LN_EPS = 1e-5
SCALE = float(64) ** -0.5     # head_size^-0.5 = 0.125


def _build():
    nc = bacc.Bacc(None, target_bir_lowering=False, debug=False)
    names = {}

    class _PE:
        """All matmuls go through here. A nosync dep chain pins PE issue
        order to program order, which makes ldweights=False (stationary
        reuse across consecutive matmuls) safe from scheduler interleaving."""
        prev = None

        @classmethod
        def mm(cls, out, stat, mov, start, stop, skip=False, reuse=False):
            inst = nc.tensor.matmul(out, stat, mov, start=start, stop=stop,
                                    skip_group_check=skip)
            # NOTE: walrus ignores ldweights=False (LDWEIGHTS is always
            # emitted per matmul) and a forced PE program-order dep chain
            # measured 40% slower on HW, so this helper is a plain matmul.
            return inst

    MM = _PE.mm
    with tile.TileContext(nc) as tc, ExitStack() as top:
        dram = top.enter_context(tc.tile_pool(name="io", bufs=1, space="DRAM"))

        def din(name, shape, dt=BF):
            t = dram.tile(shape, dt, kind="ExternalInput", name=f"i_{name}")
            names[name] = t.name
            return t

        xT_d = din("xT", [C, T])
        xTq_d = din("xTq", [C, NQ])
        wq_d = din("wq", [NPAIR, P, KC * P])
        wk_d = din("wk", [NPAIR, P, KC * P])
        wv_d = din("wv", [2, P, KC * 512])
        wo_d = din("wo", [NPAIR, P, KC * P])
        w1_d = din("w1", [32, P, KC * P])
        w2_d = din("w2", [NPAIR, P, 32 * P])
        masks_d = din("masks", [NSLOT, 8, P, CW])
        bot_d = din("bot", [P, KC], F32)
        b1t_d = din("b1t", [P, 32], F32)
        b2t_d = din("b2t", [P, KC], F32)
        g1_d = din("g1c", [P, KC], F32)
        be1_d = din("be1c", [P, KC], F32)
        g2_d = din("g2c", [P, KC], F32)
        be2_d = din("be2c", [P, KC], F32)
        onesC_d = din("onesC", [P, 1])
        onesP_d = din("onesP", [P, P])

        out_d = dram.tile([C, NQ], F32, kind="ExternalOutput", name="o_out")
        names["out"] = out_d.name

        # ---- persistent small sbuf ----
        pers = top.enter_context(tc.tile_pool(name="pers", bufs=1))
        onesC = pers.tile([P, 1], BF, tag="onesC")
        nc.sync.dma_start(out=onesC[:], in_=onesC_d[:])
        onesP = pers.tile([P, P], BF, tag="onesP")
        nc.sync.dma_start(out=onesP[:], in_=onesP_d[:])
        bot = pers.tile([P, KC], F32, tag="bot")
        nc.sync.dma_start(out=bot[:], in_=bot_d[:])
        b1t = pers.tile([P, 32], F32, tag="b1t")
        nc.sync.dma_start(out=b1t[:], in_=b1t_d[:])
        b2t = pers.tile([P, KC], F32, tag="b2t")
        nc.sync.dma_start(out=b2t[:], in_=b2t_d[:])
        lncol = {}
        for nm, d in [("g1", g1_d), ("be1", be1_d), ("g2", g2_d), ("be2", be2_d)]:
            t = pers.tile([P, KC], F32, tag=f"ln_{nm}", name=f"ln_{nm}")
            nc.sync.dma_start(out=t[:], in_=d[:])
            lncol[nm] = t
        zero_col = pers.tile([P, 1], F32, tag="zero_col")
        nc.vector.memset(zero_col[:], 0.0)
        eps_col = pers.tile([P, 1], F32, tag="eps_col")
        nc.vector.memset(eps_col[:], LN_EPS)

        # =====================================================================
        # transposed-layout layernorm, bf16 (in place unless out_tiles given)
        # =====================================================================
        def layernorm_T(xtiles, n, gname, bname, out_tag, out_tiles=None):
            ctx = ExitStack()
            work = ctx.enter_context(tc.tile_pool(name=f"lnw_{out_tag}", bufs=2))
            stat = ctx.enter_context(tc.tile_pool(name=f"lns_{out_tag}", bufs=1))
            pL = ctx.enter_context(tc.tile_pool(name=f"lnp_{out_tag}", bufs=2,
                                                space="PSUM"))
            pB = ctx.enter_context(tc.tile_pool(name=f"lnb_{out_tag}", bufs=2,
                                                space="PSUM"))
            nn = n // 512
            mu_row = stat.tile([1, n], BF, tag="mu_row")
            msq_row = stat.tile([1, n], BF, tag="msq_row")
            for i in range(nn):
                s = slice(i * 512, (i + 1) * 512)
                ps_sum = pL.tile([1, 512], F32, tag="lsum", name="ps_sum")
                ps_sq = pL.tile([1, 512], F32, tag="lsq", name="ps_sq")
                for kc in range(KC):
                    sq = work.tile([P, 512], BF, tag="sq", name="sq")
                    nc.gpsimd.tensor_tensor(sq[:], xtiles[kc][:, s],
                                            xtiles[kc][:, s], ALU.mult)
                    MM(ps_sum[:], onesC[:], xtiles[kc][:, s],
                       start=(kc == 0), stop=(kc == KC - 1), skip=True,
                       reuse=not (i == 0 and kc == 0))
                    MM(ps_sq[:], onesC[:], sq[:],
                       start=(kc == 0), stop=(kc == KC - 1), skip=True,
                       reuse=True)
                nc.scalar.activation(mu_row[:, s], ps_sum[:], AF.Copy, scale=1.0 / C)
                nc.scalar.activation(msq_row[:, s], ps_sq[:], AF.Copy, scale=1.0 / C)
            mu_b = stat.tile([P, n], BF, tag="mu_b")
            rstd_b = stat.tile([P, n], BF, tag="rstd_b")
            for i in range(nn):
                s = slice(i * 512, (i + 1) * 512)
                psb = pB.tile([P, 1024], F32, tag="bc", name="psb")
                MM(psb[:, 0:512], onesP[0:1, :], mu_row[:, s],
                   start=True, stop=True, skip=True, reuse=(i != 0))
                MM(psb[:, 512:1024], onesP[0:1, :], msq_row[:, s],
                   start=True, stop=True, skip=True, reuse=True)
                nc.vector.tensor_copy(mu_b[:, s], psb[:, 0:512])
                mu2 = work.tile([P, 512], BF, tag="mu2", name="mu2")
                nc.vector.tensor_tensor(mu2[:], mu_b[:, s], mu_b[:, s], ALU.mult)
                var = work.tile([P, 512], F32, tag="var", name="var")
                nc.vector.tensor_tensor(var[:], psb[:, 512:1024], mu2[:],
                                        ALU.subtract)
                nc.scalar.activation(var[:], var[:], AF.Ln, bias=eps_col[:])
                nc.scalar.activation(rstd_b[:, s], var[:], AF.Exp, scale=-0.5)
            outs = []
            for kc in range(KC):
                o = out_tiles[kc] if out_tiles is not None else xtiles[kc]
                nc.vector.tensor_tensor(o[:], xtiles[kc][:], mu_b[:], ALU.subtract)
                nc.vector.tensor_tensor(o[:], o[:], rstd_b[:], ALU.mult)
                nc.vector.tensor_scalar(o[:], o[:], lncol[gname][:, kc:kc + 1],
                                        lncol[bname][:, kc:kc + 1], ALU.mult, ALU.add)
                outs.append(o)
            ctx.close()
            return outs

        # =====================================================================
        # Phase 1: q path — LN1 of the core's own query columns, project
        # =====================================================================
        p_xTq = top.enter_context(tc.tile_pool(name="p_xTq", bufs=1))
        p_oT = top.enter_context(tc.tile_pool(name="p_oT", bufs=1))
        oT = [p_oT.tile([P, NSLOT, CW], BF, tag=f"oT{m}", name=f"oT{m}")
              for m in range(NPAIR)]
        att_ctx = ExitStack()
        p_qT = att_ctx.enter_context(tc.tile_pool(name="p_qT", bufs=1))
        xTq = []
        for kc in range(KC):
            tq = p_xTq.tile([P, NQ], BF, tag=f"xTq{kc}", name=f"xTq{kc}")
            nc.sync.dma_start(out=tq[:], in_=xTq_d[kc * P:(kc + 1) * P, :])
            xTq.append(tq)
        qT = []

        # =====================================================================
        # Phase 2: k and v paths — LN1 over the full sequence (Q projection is
        # issued right after LN1T so its matmuls overlap the LN1T apply ops)
        # =====================================================================
        p_kT = att_ctx.enter_context(tc.tile_pool(name="p_kT", bufs=1))
        p_v65 = att_ctx.enter_context(tc.tile_pool(name="p_v65", bufs=1))
        v65 = p_v65.tile([P, T // P, NPAIR, 2, 65], BF, tag="v65", name="v65")
        nc.vector.memset(v65[:, :, :, :, 64:65], 1.0)
        kT = []
        with ExitStack() as phk:
            p_xT = phk.enter_context(tc.tile_pool(name="p_xT", bufs=1))
            xT = []
            for kc in range(KC):
                t = p_xT.tile([P, T], BF, tag=f"xT{kc}", name=f"xT{kc}")
                nc.sync.dma_start(out=t[:], in_=xT_d[kc * P:(kc + 1) * P, :])
                xT.append(t)
            p_ln1q = phk.enter_context(tc.tile_pool(name="p_ln1q", bufs=1))
            ln1q_tiles = [p_ln1q.tile([P, NQ], BF, tag=f"ln1q{kc}",
                                      name=f"ln1q{kc}")
                          for kc in range(KC)]
            ln1q = layernorm_T(xTq, NQ, "g1", "be1", "ln1q",
                               out_tiles=ln1q_tiles)
            ln1T = layernorm_T(xT, T, "g1", "be1", "ln1T")

            with ExitStack() as phqw:
                wqp = phqw.enter_context(tc.tile_pool(name="wqp", bufs=2))
                pA = phqw.enter_context(tc.tile_pool(name="pAq", bufs=4,
                                                     space="PSUM"))
                for m in range(NPAIR):
                    wqm = wqp.tile([P, KC, P], BF, tag="wqm", name="wqm")
                    nc.sync.dma_start(
                        out=wqm[:], in_=wq_d[m].rearrange("p (k m) -> p k m", k=KC))
                    qt = p_qT.tile([P, NQ], BF, tag=f"qT{m}", name=f"qT{m}")
                    pss = [pA.tile([P, 512], F32, tag="proj", name="ps")
                           for _ in range(2)]
                    for kc in range(KC):
                        for nq in range(2):
                            MM(pss[nq][:], wqm[:, kc, :],
                               ln1q[kc][:, nq * 512:(nq + 1) * 512],
                               start=(kc == 0), stop=(kc == KC - 1), skip=True,
                               reuse=(nq > 0))
                    for nq in range(2):
                        nc.scalar.activation(qt[:, nq * 512:(nq + 1) * 512],
                                             pss[nq][:], AF.Copy)
                    qT.append(qt)

            with ExitStack() as phkw:
                wkp = phkw.enter_context(tc.tile_pool(name="wkp", bufs=2))
                pA = phkw.enter_context(tc.tile_pool(name="pAk", bufs=8,
                                                     space="PSUM"))
                for m in range(NPAIR):
                    wkm = wkp.tile([P, KC, P], BF, tag="wkm", name="wkm")
                    nc.sync.dma_start(
                        out=wkm[:], in_=wk_d[m].rearrange("p (k m) -> p k m", k=KC))
                    kt_t = p_kT.tile([P, T], BF, tag=f"kT{m}", name=f"kT{m}")
                    pss = [pA.tile([P, 512], F32, tag="proj", name="ps")
                           for _ in range(4)]
                    for kc in range(KC):
                        for n in range(4):
                            MM(pss[n][:], wkm[:, kc, :],
                               ln1T[kc][:, n * 512:(n + 1) * 512],
                               start=(kc == 0), stop=(kc == KC - 1), skip=True,
                               reuse=(n > 0))
                    for n in range(4):
                        nc.scalar.activation(kt_t[:, n * 512:(n + 1) * 512],
                                             pss[n][:], AF.Copy)
                    kT.append(kt_t)

            # v: token-major into resident v65 (ones col prefilled); one
            # stationary (ln1T chunk) feeds both output-dim halves
            with ExitStack() as phv:
                wvp = phv.enter_context(tc.tile_pool(name="wvp", bufs=1))
                pA = phv.enter_context(tc.tile_pool(name="pAv", bufs=4,
                                                    space="PSUM"))
                wvt = []
                for n in range(2):
                    wvn = wvp.tile([P, KC, 512], BF, tag=f"wvn{n}", name=f"wvn{n}")
                    nc.sync.dma_start(
                        out=wvn[:], in_=wv_d[n].rearrange("p (k d) -> p k d", k=KC))
                    wvt.append(wvn)
                for tt in range(T // P):
                    pss = [pA.tile([P, 512], F32, tag="proj", name="ps")
                           for _ in range(2)]
                    for kc in range(KC):
                        for n in range(2):
                            MM(pss[n][:], ln1T[kc][:, tt * P:(tt + 1) * P],
                               wvt[n][:, kc, :],
                               start=(kc == 0), stop=(kc == KC - 1), skip=True,
                               reuse=(n > 0))
                    for n in range(2):
                        nc.vector.tensor_copy(
                            v65[:, tt, 4 * n:4 * (n + 1), :, 0:64],
                            pss[n][:].rearrange("p (pr par d) -> p pr par d",
                                                pr=4, par=2))

        # =====================================================================
        # Phase 3: attention (ln1T freed; masks/avn fit above qT/kT/v65)
        # =====================================================================
        with ExitStack() as ph3:
            p_mask = ph3.enter_context(tc.tile_pool(name="p_mask", bufs=1))
            masks = {}
            for s in range(NSLOT):
                for j in range(8):
                    mt = p_mask.tile([P, CW], BF, tag=f"mask{s}_{j}",
                                     name=f"mask{s}_{j}")
                    nc.sync.dma_start(out=mt[:], in_=masks_d[s, j])
                    masks[(s, j)] = mt
            womp = ph3.enter_context(tc.tile_pool(name="womp", bufs=1))
            womt = []
            for m in range(NPAIR):
                w = womp.tile([P, KC, P], BF, tag=f"wom{m}", name=f"wom{m}")
                nc.sync.dma_start(
                    out=w[:], in_=wo_d[m].rearrange("p (k m) -> p k m", k=KC))
                womt.append(w)
            avn_pool = ph3.enter_context(tc.tile_pool(name="avn", bufs=2))
            sm_pool = ph3.enter_context(tc.tile_pool(name="sm", bufs=2))
            pt_pool = ph3.enter_context(tc.tile_pool(name="pt", bufs=3))

            psc_pool = ph3.enter_context(
                tc.tile_pool(name="psc", bufs=2, space="PSUM"))
            pav_pool = ph3.enter_context(
                tc.tile_pool(name="pav", bufs=2, space="PSUM"))
            for s in range(NSLOT):
                nkt = NKT_PROG[s]
                mask_base = 0 if s == 0 else 8
                qs = slice(s * CW, (s + 1) * CW)
                avn = {}
                avt = {}

                def _normalize(pair):
                    # evict, broadcast sums, 1/d = exp(-ln d), scale; issued
                    # one pair behind so its matmuls never wait on evictions
                    av = avt.pop(pair)
                    an = sm_pool.tile([65, 1024], BF, tag="an", name="an")
                    nc.vector.tensor_copy(an[:], av[0:65, :])
                    bc = psc_pool.tile([64, 1024], F32, tag="sc", name="bc")
                    MM(bc[:, 0:512], onesP[64:65, 0:64],
                       an[64:65, 0:512], start=True, stop=True, skip=True)
                    MM(bc[:, 512:1024], onesP[64:65, 0:64],
                       an[64:65, 512:1024], start=True, stop=True, skip=True,
                       reuse=True)
                    rec = sm_pool.tile([64, 1024], BF, tag="rec", name="rec")
                    lnd = sm_pool.tile([64, 1024], F32, tag="lnd", name="lnd")
                    nc.scalar.activation(lnd[:], bc[:], AF.Ln)
                    nc.scalar.activation(rec[:], lnd[:], AF.Exp, scale=-1.0)
                    anp = avn_pool.tile([P, CW], BF, tag=f"avn{pair}",
                                        name=f"avn{pair}")
                    nc.vector.tensor_tensor(anp[0:64, :], an[0:64, 0:512],
                                            rec[:, 0:512], ALU.mult)
                    tmo = sm_pool.tile([64, CW], BF, tag="tmo", name="tmo")
                    nc.vector.tensor_tensor(tmo[:], an[0:64, 512:1024],
                                            rec[:, 512:1024], ALU.mult)
                    nc.sync.dma_start(out=anp[64:128, :], in_=tmo[:])
                    avn[pair] = anp

                for pair in range(NPAIR):
                    av = pav_pool.tile([65, 1024], F32, tag="av", name="av")
                    avt[pair] = av
                    for kt in range(nkt):
                        kws = slice(kt * P, (kt + 1) * P)
                        psc = psc_pool.tile([P, 1024], F32, tag="sc",
                                            name="psc")
                        MM(psc[:, 0:512], kT[pair][0:64, kws],
                           qT[pair][0:64, qs], start=True, stop=True,
                           skip=True)
                        MM(psc[:, 512:1024], kT[pair][64:128, kws],
                           qT[pair][64:128, qs], start=True, stop=True,
                           skip=True)
                        pt = pt_pool.tile([P, 1024], BF, tag="pt", name="pt")
                        nc.scalar.activation(pt[:], psc[:], AF.Exp,
                                             bias=zero_col[:], scale=SCALE)
                        jm = kt - mask_base
                        if 0 <= jm < 8:
                            mt = masks[(s, jm)]
                            nc.gpsimd.tensor_tensor(pt[:, 0:512], pt[:, 0:512],
                                                    mt[:], ALU.mult)
                            nc.gpsimd.tensor_tensor(pt[:, 512:1024],
                                                    pt[:, 512:1024],
                                                    mt[:], ALU.mult)
                        st = (kt == 0)
                        sp = (kt == nkt - 1)
                        MM(av[0:65, 0:512], v65[:, kt, pair, 0, :],
                           pt[:, 0:512], start=st, stop=sp, skip=True)
                        MM(av[0:65, 512:1024], v65[:, kt, pair, 1, :],
                           pt[:, 512:1024], start=st, stop=sp, skip=True)
                    if pair > 0:
                        _normalize(pair - 1)
                _normalize(NPAIR - 1)
                # Wo for this slot (bias bo folded into eviction); psum
                # shares the score slot so both slots pipeline in 8 banks
                for m in range(NPAIR):
                    ps = psc_pool.tile([P, CW], F32, tag="sc", name="wops")
                    for k in range(NPAIR):
                        MM(ps[:], womt[m][:, k, :], avn[k][:],
                           start=(k == 0), stop=(k == NPAIR - 1), skip=True)
                    nc.vector.tensor_scalar(oT[m][:, s, :], ps[:],
                                            bot[:, m:m + 1], None, ALU.add)

        att_ctx.close()

        # =====================================================================
        # Phase 4: x2 = oT + xTq ; LN2 (not in place)
        # =====================================================================
        p_x2t = top.enter_context(tc.tile_pool(name="p_x2t", bufs=1))
        p_ln2T = top.enter_context(tc.tile_pool(name="p_ln2T", bufs=1))
        x2T = []
        ln2T_tiles = []
        for kc in range(KC):
            x2 = p_x2t.tile([P, NQ], BF, tag=f"x2t{kc}", name=f"x2t{kc}")
            nc.vector.tensor_tensor(
                x2[:], xTq[kc][:],
                oT[kc][:].rearrange("p s w -> p (s w)"), ALU.add)
            x2T.append(x2)
            lt = p_ln2T.tile([P, NQ], BF, tag=f"ln2T{kc}", name=f"ln2T{kc}")
            ln2T_tiles.append(lt)
        ln2T = layernorm_T(x2T, NQ, "g2", "be2", "ln2T", out_tiles=ln2T_tiles)

        # =====================================================================
        # Phase 5: FFN in two hidden-dim halves
        # =====================================================================
        with ExitStack() as ph5:
            ff1_pool = ph5.enter_context(tc.tile_pool(name="ff1", bufs=1))
            facc_pool = ph5.enter_context(tc.tile_pool(name="facc", bufs=1))
            w1_pool = ph5.enter_context(tc.tile_pool(name="w1s", bufs=2))
            w2_pool = ph5.enter_context(tc.tile_pool(name="w2s", bufs=2))
            fst_pool = ph5.enter_context(tc.tile_pool(name="fst", bufs=3))
            pF = ph5.enter_context(tc.tile_pool(name="pF", bufs=4, space="PSUM"))
            ffacc = [facc_pool.tile([P, NQ], BF, tag=f"facc{m}", name=f"ffacc{m}")
                     for m in range(KC)]
            for half in range(2):
                ff1 = []
                for m in range(16):
                    mm_i = half * 16 + m
                    w1m = w1_pool.tile([P, KC, P], BF, tag="w1m", name="w1m")
                    nc.sync.dma_start(
                        out=w1m[:],
                        in_=w1_d[mm_i].rearrange("p (k m) -> p k m", k=KC))
                    f = ff1_pool.tile([P, NQ], BF, tag=f"f{m}", name=f"f{m}")
                    pss = [pF.tile([P, 512], F32, tag="proj", name="ps")
                           for _ in range(2)]
                    for kc in range(KC):
                        for tch in range(2):
                            MM(pss[tch][:], w1m[:, kc, :],
                               ln2T[kc][:, tch * 512:(tch + 1) * 512],
                               start=(kc == 0), stop=(kc == KC - 1), skip=True,
                               reuse=(tch > 0))
                    for tch in range(2):
                        s = slice(tch * 512, (tch + 1) * 512)
                        # relu(x + b1) eviction
                        nc.vector.tensor_scalar(f[:, s], pss[tch][:],
                                                b1t[:, mm_i:mm_i + 1],
                                                0.0, ALU.add, ALU.max)
                    ff1.append(f)
                for mc in range(KC):
                    w2m = w2_pool.tile([P, 16, P], BF, tag="w2m", name="w2m")
                    nc.sync.dma_start(
                        out=w2m[:],
                        in_=w2_d[mc][:, half * 2048:(half + 1) * 2048]
                        .rearrange("p (k m) -> p k m", k=16))
                    pss = [pF.tile([P, 512], F32, tag="proj", name="ps")
                           for _ in range(2)]
                    for kt in range(16):
                        for tch in range(2):
                            MM(pss[tch][:], w2m[:, kt, :],
                               ff1[kt][:, tch * 512:(tch + 1) * 512],
                               start=(kt == 0), stop=(kt == 15), skip=True,
                               reuse=(tch > 0))
                    for tch in range(2):
                        s = slice(tch * 512, (tch + 1) * 512)
                        if half == 0:
                            nc.scalar.activation(ffacc[mc][:, s], pss[tch][:],
                                                 AF.Copy)
                        else:
                            o = fst_pool.tile([P, 512], F32, tag="fo", name="fo")
                            nc.vector.tensor_scalar(o[:], pss[tch][:],
                                                    b2t[:, mc:mc + 1],
                                                    None, ALU.add)
                            nc.vector.tensor_tensor(o[:], o[:], ffacc[mc][:, s],
                                                    ALU.add)
                            nc.vector.tensor_tensor(o[:], o[:], x2T[mc][:, s],
                                                    ALU.add)
                            nc.sync.dma_start(out=out_d[mc * P:(mc + 1) * P, s],
                                              in_=o[:])

    nc.compile()
    return nc, names


_CACHE = {}


def _get_built():
    if "nc" not in _CACHE:
        _CACHE["nc"], _CACHE["names"] = _build()
    return _CACHE["nc"], _CACHE["names"]


def _host_inputs(x, Wq, Wk, Wv, Wo, bo, ln1_g, ln1_b, ln2_g, ln2_b, W1, b1, W2, b2):
    """Build the 8 per-core input maps (host work = sharding/layout only)."""
    from ml_dtypes import bfloat16
    f = np.float32

    def wtile(W, nmb, nkc):
        # [mb, p, kc*P_or_512] with [mb,p,kc*w+j] = W[kc*P+p, mb*wout+j]
        kin, cout = W.shape
        wout = cout // nmb
        return np.ascontiguousarray(
            W.reshape(nkc, P, nmb, wout).transpose(2, 1, 0, 3)
            .reshape(nmb, P, nkc * wout).astype(bfloat16))

    shared = {
        "wq": wtile(np.asarray(Wq, f), NPAIR, KC),
        "wk": wtile(np.asarray(Wk, f), NPAIR, KC),
        "wv": wtile(np.asarray(Wv, f), 2, KC),
        "wo": wtile(np.asarray(Wo, f), NPAIR, KC),
        "w1": wtile(np.asarray(W1, f), 32, KC),
        "w2": wtile(np.asarray(W2, f), NPAIR, 32),
        "bot": np.ascontiguousarray(np.asarray(bo, f).reshape(KC, P).T),
        "b1t": np.ascontiguousarray(np.asarray(b1, f).reshape(32, P).T),
        "b2t": np.ascontiguousarray(np.asarray(b2, f).reshape(KC, P).T),
        "g1c": np.ascontiguousarray(np.asarray(ln1_g, f).reshape(KC, P).T),
        "be1c": np.ascontiguousarray(np.asarray(ln1_b, f).reshape(KC, P).T),
        "g2c": np.ascontiguousarray(np.asarray(ln2_g, f).reshape(KC, P).T),
        "be2c": np.ascontiguousarray(np.asarray(ln2_b, f).reshape(KC, P).T),
        "onesC": np.ones((P, 1), bfloat16),
        "onesP": np.ones((P, P), bfloat16),
    }
    kl = np.arange(P)[:, None]
    ql = np.arange(CW)[None, :]
    in_maps = []
    for c in range(8):
        b, r = c // 2, c % 2
        gs = GSETS[r]
        xTb = np.ascontiguousarray(np.asarray(x[b], f).T.astype(bfloat16))
        qcols = np.concatenate([np.arange(CW * g, CW * (g + 1)) for g in gs])
        xTq = np.ascontiguousarray(xTb[:, qcols])
        m = np.empty((NSLOT, 8, P, CW), bfloat16)
        for s in range(NSLOT):
            q0 = CW * gs[s]
            base = 0 if s == 0 else 8
            for j in range(8):
                kt = base + j
                m[s, j] = ((P * kt + kl) <= (q0 + ql)).astype(bfloat16)
        im = dict(shared)
        im["xT"] = xTb
        im["xTq"] = xTq
        im["masks"] = m
        in_maps.append(im)
    return in_maps


def _unshard(outs):
    out = np.empty((4, T, C), np.float32)
    for c in range(8):
        b, r = c // 2, c % 2
        oT = outs[c]  # (C, NQ)
        for s, g in enumerate(GSETS[r]):
            out[b, CW * g:CW * (g + 1), :] = oT[:, CW * s:CW * (s + 1)].T
    return out


def kernel(**inputs):
    from concourse.bass_utils import run_bass_kernel_spmd
    from concourse.bass_interp import get_hw_module

    args = {k: np.asarray(v, np.float32) for k, v in inputs.items()}
    in_maps_named = _host_inputs(**args)

    nc, names = _get_built()
    in_maps = [{names[k]: v for k, v in im.items()} for im in in_maps_named]

    hw = get_hw_module(nc.m)
    old = nc.m
    nc.m = hw
    try:
        res = run_bass_kernel_spmd(nc, in_maps, core_ids=list(range(8)))
    finally:
        nc.m = old
    outs = [r[names["out"]] for r in res.results]
    return _unshard(outs)


if __name__ == "__main__":
    import reference
    inp = {k: np.asarray(v) for k, v in reference.setup_inputs().items()}
    got = kernel(**inp)
    exp = np.asarray(reference.reference(**inp))
    err = np.abs(got - exp).max() / np.abs(exp).max()
    print("Relative error:", err)


# revision 20
# speedup vs baseline: 1.1829x; 1.1829x over previous
"""Trainium2 Bass kernel for a transformer MiniBlock (B=4, T=2048, C=1024, 16 heads,
causal attention, 4x FFN), sharded over 8 NeuronCores.

Sharding: core = (batch b=core//2, role r=core%2). Each core runs the full block for
1024 of its batch's 2048 tokens (two 512-token chunks, balanced for causal work:
role 0 owns chunks {0,3}, role 1 owns {1,2}), computing K/V over the full sequence
(no cross-core communication). The program is SPMD-uniform: k-window loop bounds are
per-slot maxima over roles; per-core causal masks (input data) zero the difference.

All tensors are bf16 on-chip (fp32 PSUM accumulation), which doubles effective
SBUF/DMA capacity, enables fast weight loads, and keeps rel-err ~3e-3. Activations
stay channel-major end to end; LN stats / softmax sums / broadcasts use small
ones-matmuls; the attention softmax is computed k-major with a ones-column appended
to V so denominators fall out of the AV matmul. Weights are pre-tiled on the host so
every weight DMA is fully contiguous. V stays resident in SBUF (no DRAM spill).
Even/odd head score matmuls are row-packed (tile_position) to run concurrently, and
exp is a single 1024-wide activation spanning two PSUM banks.
"""
import sys

sys.path.insert(0, "/opt/trn_rl_repo")

import numpy as np
from contextlib import ExitStack

import concourse.bacc as bacc
import concourse.mybir as mybir
import concourse.tile as tile
from concourse.tile import add_dep_helper

F32 = mybir.dt.float32
BF = mybir.dt.bfloat16
AF = mybir.ActivationFunctionType
ALU = mybir.AluOpType

P = 128
T = 2048          # full sequence
C = 1024          # embedding
NQ = 1024         # query tokens per core
H4 = 4096         # ffn hidden
NPAIR = 8         # head pairs (2 heads of 64 dims = 128 channels)
KC = C // P       # 8 channel tiles
NSLOT = 2         # 512-token query chunks per core
CW = 512          # chunk width
NKT_PROG = [8, 16]            # k-tiles per slot (program constant, max over roles)
GSETS = [[0, 3], [1, 2]]      # global 512-chunk index per slot, per role
LN_EPS = 1e-5
SCALE = float(64) ** -0.5     # head_size^-0.5 = 0.125


def _build():
    nc = bacc.Bacc(None, target_bir_lowering=False, debug=False)
    names = {}

    class _PE:
        """All matmuls go through here. A nosync dep chain pins PE issue
        order to program order, which makes ldweights=False (stationary
        reuse across consecutive matmuls) safe from scheduler interleaving."""
        prev = None

        @classmethod
        def mm(cls, out, stat, mov, start, stop, skip=False, reuse=False):
            inst = nc.tensor.matmul(out, stat, mov, start=start, stop=stop,
                                    skip_group_check=skip)
            # NOTE: walrus ignores ldweights=False (LDWEIGHTS is always
            # emitted per matmul) and a forced PE program-order dep chain
            # measured 40% slower on HW, so this helper is a plain matmul.
            return inst

    MM = _PE.mm
    with tile.TileContext(nc) as tc, ExitStack() as top:
        dram = top.enter_context(tc.tile_pool(name="io", bufs=1, space="DRAM"))

        def din(name, shape, dt=BF):
            t = dram.tile(shape, dt, kind="ExternalInput", name=f"i_{name}")
            names[name] = t.name
            return t

        xT_d = din("xT", [C, T])
        xTq_d = din("xTq", [C, NQ])
        wq_d = din("wq", [NPAIR, P, KC * P])
        wk_d = din("wk", [NPAIR, P, KC * P])
        wv_d = din("wv", [2, P, KC * 512])
        wo_d = din("wo", [NPAIR, P, KC * P])
        w1_d = din("w1", [32, P, KC * P])
        w2_d = din("w2", [NPAIR, P, 32 * P])
        masks_d = din("masks", [NSLOT, 8, P, CW])
        bot_d = din("bot", [P, KC], F32)
        b1t_d = din("b1t", [P, 32], F32)
        b2t_d = din("b2t", [P, KC], F32)
        g1_d = din("g1c", [P, KC], F32)
        be1_d = din("be1c", [P, KC], F32)
        g2_d = din("g2c", [P, KC], F32)
        be2_d = din("be2c", [P, KC], F32)
        onesC_d = din("onesC", [P, 1])
        onesP_d = din("onesP", [P, P])

        out_d = dram.tile([C, NQ], F32, kind="ExternalOutput", name="o_out")
        names["out"] = out_d.name

        # ---- persistent small sbuf ----
        pers = top.enter_context(tc.tile_pool(name="pers", bufs=1))
        onesC = pers.tile([P, 1], BF, tag="onesC")
        nc.sync.dma_start(out=onesC[:], in_=onesC_d[:])
        onesP = pers.tile([P, P], BF, tag="onesP")
        nc.sync.dma_start(out=onesP[:], in_=onesP_d[:])
        bot = pers.tile([P, KC], F32, tag="bot")
        nc.sync.dma_start(out=bot[:], in_=bot_d[:])
        b1t = pers.tile([P, 32], F32, tag="b1t")
        nc.sync.dma_start(out=b1t[:], in_=b1t_d[:])
        b2t = pers.tile([P, KC], F32, tag="b2t")
        nc.sync.dma_start(out=b2t[:], in_=b2t_d[:])
        lncol = {}
        for nm, d in [("g1", g1_d), ("be1", be1_d), ("g2", g2_d), ("be2", be2_d)]:
            t = pers.tile([P, KC], F32, tag=f"ln_{nm}", name=f"ln_{nm}")
            nc.sync.dma_start(out=t[:], in_=d[:])
            lncol[nm] = t
        zero_col = pers.tile([P, 1], F32, tag="zero_col")
        nc.vector.memset(zero_col[:], 0.0)
        eps_col = pers.tile([P, 1], F32, tag="eps_col")
        nc.vector.memset(eps_col[:], LN_EPS)

        # =====================================================================
        # transposed-layout layernorm, bf16 (in place unless out_tiles given)
        # =====================================================================
        def layernorm_T(xtiles, n, gname, bname, out_tag, out_tiles=None):
            ctx = ExitStack()
            work = ctx.enter_context(tc.tile_pool(name=f"lnw_{out_tag}", bufs=2))
            stat = ctx.enter_context(tc.tile_pool(name=f"lns_{out_tag}", bufs=1))
            pL = ctx.enter_context(tc.tile_pool(name=f"lnp_{out_tag}", bufs=2,
                                                space="PSUM"))
            pB = ctx.enter_context(tc.tile_pool(name=f"lnb_{out_tag}", bufs=2,
                                                space="PSUM"))
            nn = n // 512
            mu_row = stat.tile([1, n], BF, tag="mu_row")
            msq_row = stat.tile([1, n], BF, tag="msq_row")
            for i in range(nn):
                s = slice(i * 512, (i + 1) * 512)
                ps_sum = pL.tile([1, 512], F32, tag="lsum", name="ps_sum")
                ps_sq = pL.tile([1, 512], F32, tag="lsq", name="ps_sq")
                for kc in range(KC):
                    sq = work.tile([P, 512], BF, tag="sq", name="sq")
                    nc.vector.tensor_tensor(sq[:], xtiles[kc][:, s],
                                            xtiles[kc][:, s], ALU.mult)
                    MM(ps_sum[:], onesC[:], xtiles[kc][:, s],
                       start=(kc == 0), stop=(kc == KC - 1), skip=True,
                       reuse=not (i == 0 and kc == 0))
                    MM(ps_sq[:], onesC[:], sq[:],
                       start=(kc == 0), stop=(kc == KC - 1), skip=True,
                       reuse=True)
                nc.scalar.activation(mu_row[:, s], ps_sum[:], AF.Copy, scale=1.0 / C)
                nc.scalar.activation(msq_row[:, s], ps_sq[:], AF.Copy, scale=1.0 / C)
            mu_b = stat.tile([P, n], BF, tag="mu_b")
            rstd_b = stat.tile([P, n], BF, tag="rstd_b")
            for i in range(nn):
                s = slice(i * 512, (i + 1) * 512)
                psb = pB.tile([P, 1024], F32, tag="bc", name="psb")
                MM(psb[:, 0:512], onesP[0:1, :], mu_row[:, s],
                   start=True, stop=True, skip=True, reuse=(i != 0))
                MM(psb[:, 512:1024], onesP[0:1, :], msq_row[:, s],
                   start=True, stop=True, skip=True, reuse=True)
                nc.vector.tensor_copy(mu_b[:, s], psb[:, 0:512])
                mu2 = work.tile([P, 512], BF, tag="mu2", name="mu2")
                nc.vector.tensor_tensor(mu2[:], mu_b[:, s], mu_b[:, s], ALU.mult)
                var = work.tile([P, 512], F32, tag="var", name="var")
                nc.vector.tensor_tensor(var[:], psb[:, 512:1024], mu2[:],
                                        ALU.subtract)
                nc.scalar.activation(var[:], var[:], AF.Ln, bias=eps_col[:])
                nc.scalar.activation(rstd_b[:, s], var[:], AF.Exp, scale=-0.5)
            outs = []
            for kc in range(KC):
                o = out_tiles[kc] if out_tiles is not None else xtiles[kc]
                nc.vector.tensor_tensor(o[:], xtiles[kc][:], mu_b[:], ALU.subtract)
                nc.vector.tensor_tensor(o[:], o[:], rstd_b[:], ALU.mult)
                nc.vector.tensor_scalar(o[:], o[:], lncol[gname][:, kc:kc + 1],
                                        lncol[bname][:, kc:kc + 1], ALU.mult, ALU.add)
                outs.append(o)
            ctx.close()
            return outs

        # =====================================================================
        # Phase 1: q path — LN1 of the core's own query columns, project
        # =====================================================================
        p_xTq = top.enter_context(tc.tile_pool(name="p_xTq", bufs=1))
        p_oT = top.enter_context(tc.tile_pool(name="p_oT", bufs=1))
        oT = [p_oT.tile([P, NSLOT, CW], BF, tag=f"oT{m}", name=f"oT{m}")
              for m in range(NPAIR)]
        att_ctx = ExitStack()
        p_qT = att_ctx.enter_context(tc.tile_pool(name="p_qT", bufs=1))
        xTq = []
        for kc in range(KC):
            tq = p_xTq.tile([P, NQ], BF, tag=f"xTq{kc}", name=f"xTq{kc}")
            nc.sync.dma_start(out=tq[:], in_=xTq_d[kc * P:(kc + 1) * P, :])
            xTq.append(tq)
        qT = []

        # =====================================================================
        # Phase 2: k and v paths — LN1 over the full sequence (Q projection is
        # issued right after LN1T so its matmuls overlap the LN1T apply ops)
        # =====================================================================
        p_kT = att_ctx.enter_context(tc.tile_pool(name="p_kT", bufs=1))
        p_v65 = att_ctx.enter_context(tc.tile_pool(name="p_v65", bufs=1))
        v65 = p_v65.tile([P, T // P, NPAIR, 2, 65], BF, tag="v65", name="v65")
        nc.vector.memset(v65[:, :, :, :, 64:65], 1.0)
        kT = []
        with ExitStack() as phk:
            p_xT = phk.enter_context(tc.tile_pool(name="p_xT", bufs=1))
            xT = []
            for kc in range(KC):
                t = p_xT.tile([P, T], BF, tag=f"xT{kc}", name=f"xT{kc}")
                nc.sync.dma_start(out=t[:], in_=xT_d[kc * P:(kc + 1) * P, :])
                xT.append(t)
            p_ln1q = phk.enter_context(tc.tile_pool(name="p_ln1q", bufs=1))
            ln1q_tiles = [p_ln1q.tile([P, NQ], BF, tag=f"ln1q{kc}",
                                      name=f"ln1q{kc}")
                          for kc in range(KC)]
            ln1q = layernorm_T(xTq, NQ, "g1", "be1", "ln1q",
                               out_tiles=ln1q_tiles)
            ln1T = layernorm_T(xT, T, "g1", "be1", "ln1T")

            with ExitStack() as phqw:
                wqp = phqw.enter_context(tc.tile_pool(name="wqp", bufs=2))
                pA = phqw.enter_context(tc.tile_pool(name="pAq", bufs=4,
                                                     space="PSUM"))
                for m in range(NPAIR):
                    wqm = wqp.tile([P, KC, P], BF, tag="wqm", name="wqm")
                    nc.sync.dma_start(
                        out=wqm[:], in_=wq_d[m].rearrange("p (k m) -> p k m", k=KC))
                    qt = p_qT.tile([P, NQ], BF, tag=f"qT{m}", name=f"qT{m}")
                    pss = [pA.tile([P, 512], F32, tag="proj", name="ps")
                           for _ in range(2)]
                    for kc in range(KC):
                        for nq in range(2):
                            MM(pss[nq][:], wqm[:, kc, :],
                               ln1q[kc][:, nq * 512:(nq + 1) * 512],
                               start=(kc == 0), stop=(kc == KC - 1), skip=True,
                               reuse=(nq > 0))
                    for nq in range(2):
                        nc.scalar.activation(qt[:, nq * 512:(nq + 1) * 512],
                                             pss[nq][:], AF.Copy)
                    qT.append(qt)

            with ExitStack() as phkw:
                wkp = phkw.enter_context(tc.tile_pool(name="wkp", bufs=2))
                pA = phkw.enter_context(tc.tile_pool(name="pAk", bufs=8,
                                                     space="PSUM"))
                for m in range(NPAIR):
                    wkm = wkp.tile([P, KC, P], BF, tag="wkm", name="wkm")
                    nc.sync.dma_start(
                        out=wkm[:], in_=wk_d[m].rearrange("p (k m) -> p k m", k=KC))
                    kt_t = p_kT.tile([P, T], BF, tag=f"kT{m}", name=f"kT{m}")
                    pss = [pA.tile([P, 512], F32, tag="proj", name="ps")
                           for _ in range(4)]
                    for kc in range(KC):
                        for n in range(4):
                            MM(pss[n][:], wkm[:, kc, :],
                               ln1T[kc][:, n * 512:(n + 1) * 512],
                               start=(kc == 0), stop=(kc == KC - 1), skip=True,
                               reuse=(n > 0))
                    for n in range(4):
                        nc.scalar.activation(kt_t[:, n * 512:(n + 1) * 512],
                                             pss[n][:], AF.Copy)
                    kT.append(kt_t)

            # v: token-major into resident v65 (ones col prefilled); one
            # stationary (ln1T chunk) feeds both output-dim halves
            with ExitStack() as phv:
                wvp = phv.enter_context(tc.tile_pool(name="wvp", bufs=1))
                pA = phv.enter_context(tc.tile_pool(name="pAv", bufs=4,
                                                    space="PSUM"))
                wvt = []
                for n in range(2):
                    wvn = wvp.tile([P, KC, 512], BF, tag=f"wvn{n}", name=f"wvn{n}")
                    nc.sync.dma_start(
                        out=wvn[:], in_=wv_d[n].rearrange("p (k d) -> p k d", k=KC))
                    wvt.append(wvn)
                for tt in range(T // P):
                    pss = [pA.tile([P, 512], F32, tag="proj", name="ps")
                           for _ in range(2)]
                    for kc in range(KC):
                        for n in range(2):
                            MM(pss[n][:], ln1T[kc][:, tt * P:(tt + 1) * P],
                               wvt[n][:, kc, :],
                               start=(kc == 0), stop=(kc == KC - 1), skip=True,
                               reuse=(n > 0))
                    for n in range(2):
                        nc.vector.tensor_copy(
                            v65[:, tt, 4 * n:4 * (n + 1), :, 0:64],
                            pss[n][:].rearrange("p (pr par d) -> p pr par d",
                                                pr=4, par=2))

        # =====================================================================
        # Phase 3: attention (ln1T freed; masks/avn fit above qT/kT/v65)
        # =====================================================================
        with ExitStack() as ph3:
            p_mask = ph3.enter_context(tc.tile_pool(name="p_mask", bufs=1))
            masks = {}
            for s in range(NSLOT):
                for j in range(8):
                    mt = p_mask.tile([P, CW], BF, tag=f"mask{s}_{j}",
                                     name=f"mask{s}_{j}")
                    nc.sync.dma_start(out=mt[:], in_=masks_d[s, j])
                    masks[(s, j)] = mt
            womp = ph3.enter_context(tc.tile_pool(name="womp", bufs=1))
            womt = []
            for m in range(NPAIR):
                w = womp.tile([P, KC, P], BF, tag=f"wom{m}", name=f"wom{m}")
                nc.sync.dma_start(
                    out=w[:], in_=wo_d[m].rearrange("p (k m) -> p k m", k=KC))
                womt.append(w)
            avn_pool = ph3.enter_context(tc.tile_pool(name="avn", bufs=2))
            sm_pool = ph3.enter_context(tc.tile_pool(name="sm", bufs=2))
            pt_pool = ph3.enter_context(tc.tile_pool(name="pt", bufs=3))

            psc_pool = ph3.enter_context(
                tc.tile_pool(name="psc", bufs=2, space="PSUM"))
            pav_pool = ph3.enter_context(
                tc.tile_pool(name="pav", bufs=2, space="PSUM"))
            for s in range(NSLOT):
                nkt = NKT_PROG[s]
                mask_base = 0 if s == 0 else 8
                qs = slice(s * CW, (s + 1) * CW)
                avn = {}
                avt = {}

                def _normalize(pair):
                    # evict, broadcast sums, 1/d = exp(-ln d), scale; issued
                    # one pair behind so its matmuls never wait on evictions
                    av = avt.pop(pair)
                    an = sm_pool.tile([65, 1024], BF, tag="an", name="an")
                    nc.vector.tensor_copy(an[:], av[0:65, :])
                    bc = psc_pool.tile([64, 1024], F32, tag="sc", name="bc")
                    MM(bc[:, 0:512], onesP[64:65, 0:64],
                       an[64:65, 0:512], start=True, stop=True, skip=True)
                    MM(bc[:, 512:1024], onesP[64:65, 0:64],
                       an[64:65, 512:1024], start=True, stop=True, skip=True,
                       reuse=True)
                    rec = sm_pool.tile([64, 1024], BF, tag="rec", name="rec")
                    lnd = sm_pool.tile([64, 1024], F32, tag="lnd", name="lnd")
                    nc.scalar.activation(lnd[:], bc[:], AF.Ln)
                    nc.scalar.activation(rec[:], lnd[:], AF.Exp, scale=-1.0)
                    anp = avn_pool.tile([P, CW], BF, tag=f"avn{pair}",
                                        name=f"avn{pair}")
                    nc.vector.tensor_tensor(anp[0:64, :], an[0:64, 0:512],
                                            rec[:, 0:512], ALU.mult)
                    tmo = sm_pool.tile([64, CW], BF, tag="tmo", name="tmo")
                    nc.vector.tensor_tensor(tmo[:], an[0:64, 512:1024],
                                            rec[:, 512:1024], ALU.mult)
                    nc.sync.dma_start(out=anp[64:128, :], in_=tmo[:])
                    avn[pair] = anp

                for pair in range(NPAIR):
                    av = pav_pool.tile([65, 1024], F32, tag="av", name="av")
                    avt[pair] = av
                    for kt in range(nkt):
                        kws = slice(kt * P, (kt + 1) * P)
                        psc = psc_pool.tile([P, 1024], F32, tag="sc",
                                            name="psc")
                        MM(psc[:, 0:512], kT[pair][0:64, kws],
                           qT[pair][0:64, qs], start=True, stop=True,
                           skip=True)
                        MM(psc[:, 512:1024], kT[pair][64:128, kws],
                           qT[pair][64:128, qs], start=True, stop=True,
                           skip=True)
                        pt = pt_pool.tile([P, 1024], BF, tag="pt", name="pt")
                        nc.scalar.activation(pt[:], psc[:], AF.Exp,
                                             bias=zero_col[:], scale=SCALE)
                        jm = kt - mask_base
                        if 0 <= jm < 8:
                            mt = masks[(s, jm)]
                            nc.vector.tensor_tensor(pt[:, 0:512], pt[:, 0:512],
                                                    mt[:], ALU.mult)
                            nc.vector.tensor_tensor(pt[:, 512:1024],
                                                    pt[:, 512:1024],
                                                    mt[:], ALU.mult)
                        st = (kt == 0)
                        sp = (kt == nkt - 1)
                        MM(av[0:65, 0:512], v65[:, kt, pair, 0, :],
                           pt[:, 0:512], start=st, stop=sp, skip=True)
                        MM(av[0:65, 512:1024], v65[:, kt, pair, 1, :],
                           pt[:, 512:1024], start=st, stop=sp, skip=True)
                    if pair > 0:
                        _normalize(pair - 1)
                _normalize(NPAIR - 1)
                # Wo for this slot (bias bo folded into eviction); psum
                # shares the score slot so both slots pipeline in 8 banks
                for m in range(NPAIR):
                    ps = psc_pool.tile([P, CW], F32, tag="sc", name="wops")
                    for k in range(NPAIR):
                        MM(ps[:], womt[m][:, k, :], avn[k][:],
                           start=(k == 0), stop=(k == NPAIR - 1), skip=True)
                    nc.vector.tensor_scalar(oT[m][:, s, :], ps[:],
                                            bot[:, m:m + 1], None, ALU.add)

        att_ctx.close()

        # =====================================================================
        # Phase 4: x2 = oT + xTq ; LN2 (not in place)
        # =====================================================================
        p_x2t = top.enter_context(tc.tile_pool(name="p_x2t", bufs=1))
        p_ln2T = top.enter_context(tc.tile_pool(name="p_ln2T", bufs=1))
        x2T = []
        ln2T_tiles = []
        for kc in range(KC):
            x2 = p_x2t.tile([P, NQ], BF, tag=f"x2t{kc}", name=f"x2t{kc}")
            nc.vector.tensor_tensor(
                x2[:], xTq[kc][:],
                oT[kc][:].rearrange("p s w -> p (s w)"), ALU.add)
            x2T.append(x2)
            lt = p_ln2T.tile([P, NQ], BF, tag=f"ln2T{kc}", name=f"ln2T{kc}")
            ln2T_tiles.append(lt)
        ln2T = layernorm_T(x2T, NQ, "g2", "be2", "ln2T", out_tiles=ln2T_tiles)

        # =====================================================================
        # Phase 5: FFN in two hidden-dim halves
        # =====================================================================
        with ExitStack() as ph5:
            ff1_pool = ph5.enter_context(tc.tile_pool(name="ff1", bufs=1))
            facc_pool = ph5.enter_context(tc.tile_pool(name="facc", bufs=1))
            w1_pool = ph5.enter_context(tc.tile_pool(name="w1s", bufs=2))
            w2_pool = ph5.enter_context(tc.tile_pool(name="w2s", bufs=2))
            fst_pool = ph5.enter_context(tc.tile_pool(name="fst", bufs=3))
            pF = ph5.enter_context(tc.tile_pool(name="pF", bufs=4, space="PSUM"))
            ffacc = [facc_pool.tile([P, NQ], BF, tag=f"facc{m}", name=f"ffacc{m}")
                     for m in range(KC)]
            for half in range(2):
                ff1 = []
                for m in range(16):
                    mm_i = half * 16 + m
                    w1m = w1_pool.tile([P, KC, P], BF, tag="w1m", name="w1m")
                    nc.sync.dma_start(
                        out=w1m[:],
                        in_=w1_d[mm_i].rearrange("p (k m) -> p k m", k=KC))
                    f = ff1_pool.tile([P, NQ], BF, tag=f"f{m}", name=f"f{m}")
                    pss = [pF.tile([P, 512], F32, tag="proj", name="ps")
                           for _ in range(2)]
                    for kc in range(KC):
                        for tch in range(2):
                            MM(pss[tch][:], w1m[:, kc, :],
                               ln2T[kc][:, tch * 512:(tch + 1) * 512],
                               start=(kc == 0), stop=(kc == KC - 1), skip=True,
                               reuse=(tch > 0))
                    for tch in range(2):
                        s = slice(tch * 512, (tch + 1) * 512)
                        # relu(x + b1) eviction
                        nc.vector.tensor_scalar(f[:, s], pss[tch][:],
                                                b1t[:, mm_i:mm_i + 1],
                                                0.0, ALU.add, ALU.max)
                    ff1.append(f)
                for mc in range(KC):
                    w2m = w2_pool.tile([P, 16, P], BF, tag="w2m", name="w2m")
                    nc.sync.dma_start(
                        out=w2m[:],
                        in_=w2_d[mc][:, half * 2048:(half + 1) * 2048]
                        .rearrange("p (k m) -> p k m", k=16))
                    pss = [pF.tile([P, 512], F32, tag="proj", name="ps")
                           for _ in range(2)]
                    for kt in range(16):
                        for tch in range(2):
                            MM(pss[tch][:], w2m[:, kt, :],
                               ff1[kt][:, tch * 512:(tch + 1) * 512],
                               start=(kt == 0), stop=(kt == 15), skip=True,
                               reuse=(tch > 0))
                    for tch in range(2):
                        s = slice(tch * 512, (tch + 1) * 512)
                        if half == 0:
                            nc.scalar.activation(ffacc[mc][:, s], pss[tch][:],
                                                 AF.Copy)
                        else:
                            o = fst_pool.tile([P, 512], F32, tag="fo", name="fo")
                            nc.vector.tensor_scalar(o[:], pss[tch][:],
                                                    b2t[:, mc:mc + 1],
                                                    None, ALU.add)
                            nc.vector.tensor_tensor(o[:], o[:], ffacc[mc][:, s],
                                                    ALU.add)
                            nc.vector.tensor_tensor(o[:], o[:], x2T[mc][:, s],
                                                    ALU.add)
                            nc.sync.dma_start(out=out_d[mc * P:(mc + 1) * P, s],
                                              in_=o[:])

    nc.compile()
    return nc, names


_CACHE = {}


def _get_built():
    if "nc" not in _CACHE:
        _CACHE["nc"], _CACHE["names"] = _build()
    return _CACHE["nc"], _CACHE["names"]


def _host_inputs(x, Wq, Wk, Wv, Wo, bo, ln1_g, ln1_b, ln2_g, ln2_b, W1, b1, W2, b2):
    """Build the 8 per-core input maps (host work = sharding/layout only)."""
    from ml_dtypes import bfloat16
    f = np.float32

    def wtile(W, nmb, nkc):
        # [mb, p, kc*P_or_512] with [mb,p,kc*w+j] = W[kc*P+p, mb*wout+j]
        kin, cout = W.shape
        wout = cout // nmb
        return np.ascontiguousarray(
            W.reshape(nkc, P, nmb, wout).transpose(2, 1, 0, 3)
            .reshape(nmb, P, nkc * wout).astype(bfloat16))

    shared = {
        "wq": wtile(np.asarray(Wq, f), NPAIR, KC),
        "wk": wtile(np.asarray(Wk, f), NPAIR, KC),
        "wv": wtile(np.asarray(Wv, f), 2, KC),
        "wo": wtile(np.asarray(Wo, f), NPAIR, KC),
        "w1": wtile(np.asarray(W1, f), 32, KC),
        "w2": wtile(np.asarray(W2, f), NPAIR, 32),
        "bot": np.ascontiguousarray(np.asarray(bo, f).reshape(KC, P).T),
        "b1t": np.ascontiguousarray(np.asarray(b1, f).reshape(32, P).T),
        "b2t": np.ascontiguousarray(np.asarray(b2, f).reshape(KC, P).T),
        "g1c": np.ascontiguousarray(np.asarray(ln1_g, f).reshape(KC, P).T),
        "be1c": np.ascontiguousarray(np.asarray(ln1_b, f).reshape(KC, P).T),
        "g2c": np.ascontiguousarray(np.asarray(ln2_g, f).reshape(KC, P).T),
        "be2c": np.ascontiguousarray(np.asarray(ln2_b, f).reshape(KC, P).T),
        "onesC": np.ones((P, 1), bfloat16),
        "onesP": np.ones((P, P), bfloat16),
    }
    kl = np.arange(P)[:, None]
    ql = np.arange(CW)[None, :]
    in_maps = []
    for c in range(8):
        b, r = c // 2, c % 2
        gs = GSETS[r]
        xTb = np.ascontiguousarray(np.asarray(x[b], f).T.astype(bfloat16))
        qcols = np.concatenate([np.arange(CW * g, CW * (g + 1)) for g in gs])
        xTq = np.ascontiguousarray(xTb[:, qcols])
        m = np.empty((NSLOT, 8, P, CW), bfloat16)
        for s in range(NSLOT):
            q0 = CW * gs[s]
            base = 0 if s == 0 else 8
            for j in range(8):
                kt = base + j
                m[s, j] = ((P * kt + kl) <= (q0 + ql)).astype(bfloat16)
        im = dict(shared)
        im["xT"] = xTb
        im["xTq"] = xTq
        im["masks"] = m
        in_maps.append(im)
    return in_maps


def _unshard(outs):
    out = np.empty((4, T, C), np.float32)
    for c in range(8):
        b, r = c // 2, c % 2
        oT = outs[c]  # (C, NQ)
        for s, g in enumerate(GSETS[r]):
            out[b, CW * g:CW * (g + 1), :] = oT[:, CW * s:CW * (s + 1)].T
    return out


def kernel(**inputs):
    from concourse.bass_utils import run_bass_kernel_spmd
    from concourse.bass_interp import get_hw_module

    args = {k: np.asarray(v, np.float32) for k, v in inputs.items()}
    in_maps_named = _host_inputs(**args)

    nc, names = _get_built()
    in_maps = [{names[k]: v for k, v in im.items()} for im in in_maps_named]

    hw = get_hw_module(nc.m)
    old = nc.m
    nc.m = hw
    try:
        res = run_bass_kernel_spmd(nc, in_maps, core_ids=list(range(8)))
    finally:
        nc.m = old
    outs = [r[names["out"]] for r in res.results]
    return _unshard(outs)


if __name__ == "__main__":
    import reference
    inp = {k: np.asarray(v) for k, v in reference.setup_inputs().items()}
    got = kernel(**inp)
    exp = np.asarray(reference.reference(**inp))
    err = np.abs(got - exp).max() / np.abs(exp).max()
    print("Relative error:", err)


# revision 22
# speedup vs baseline: 1.2028x; 1.0168x over previous
"""Trainium2 Bass kernel for a transformer MiniBlock (B=4, T=2048, C=1024, 16 heads,
causal attention, 4x FFN), sharded over 8 NeuronCores.

Sharding: core = (batch b=core//2, role r=core%2). Each core runs the full block for
1024 of its batch's 2048 tokens (two 512-token chunks, balanced for causal work:
role 0 owns chunks {0,3}, role 1 owns {1,2}), computing K/V over the full sequence
(no cross-core communication). The program is SPMD-uniform: k-window loop bounds are
per-slot maxima over roles; per-core causal masks (input data) zero the difference.

All tensors are bf16 on-chip (fp32 PSUM accumulation), which doubles effective
SBUF/DMA capacity, enables fast weight loads, and keeps rel-err ~3e-3. Activations
stay channel-major end to end; LN stats / softmax sums / broadcasts use small
ones-matmuls; the attention softmax is computed k-major with a ones-column appended
to V so denominators fall out of the AV matmul. Weights are pre-tiled on the host so
every weight DMA is fully contiguous. V stays resident in SBUF (no DRAM spill).
Even/odd head score matmuls are row-packed (tile_position) to run concurrently, and
exp is a single 1024-wide activation spanning two PSUM banks.
"""
import sys

sys.path.insert(0, "/opt/trn_rl_repo")

import numpy as np
from contextlib import ExitStack

import concourse.bacc as bacc
import concourse.mybir as mybir
import concourse.tile as tile
from concourse.tile import add_dep_helper

F32 = mybir.dt.float32
BF = mybir.dt.bfloat16
AF = mybir.ActivationFunctionType
ALU = mybir.AluOpType

P = 128
T = 2048          # full sequence
C = 1024          # embedding
NQ = 1024         # query tokens per core
H4 = 4096         # ffn hidden
NPAIR = 8         # head pairs (2 heads of 64 dims = 128 channels)
KC = C // P       # 8 channel tiles
NSLOT = 2         # 512-token query chunks per core
CW = 512          # chunk width
NKT_PROG = [8, 16]            # k-tiles per slot (program constant, max over roles)
GSETS = [[0, 3], [1, 2]]      # global 512-chunk index per slot, per role
LN_EPS = 1e-5
SCALE = float(64) ** -0.5     # head_size^-0.5 = 0.125


def _build():
    nc = bacc.Bacc(None, target_bir_lowering=False, debug=False)
    names = {}

    class _PE:
        """All matmuls go through here. A nosync dep chain pins PE issue
        order to program order, which makes ldweights=False (stationary
        reuse across consecutive matmuls) safe from scheduler interleaving."""
        prev = None

        @classmethod
        def mm(cls, out, stat, mov, start, stop, skip=False, reuse=False):
            inst = nc.tensor.matmul(out, stat, mov, start=start, stop=stop,
                                    skip_group_check=skip)
            # NOTE: walrus ignores ldweights=False (LDWEIGHTS is always
            # emitted per matmul) and a forced PE program-order dep chain
            # measured 40% slower on HW, so this helper is a plain matmul.
            return inst

    MM = _PE.mm
    with tile.TileContext(nc) as tc, ExitStack() as top:
        dram = top.enter_context(tc.tile_pool(name="io", bufs=1, space="DRAM"))

        def din(name, shape, dt=BF):
            t = dram.tile(shape, dt, kind="ExternalInput", name=f"i_{name}")
            names[name] = t.name
            return t

        xT_d = din("xT", [C, T])
        xTq_d = din("xTq", [C, NQ])
        wq_d = din("wq", [NPAIR, P, KC * P])
        wk_d = din("wk", [NPAIR, P, KC * P])
        wv_d = din("wv", [2, P, KC * 512])
        wo_d = din("wo", [NPAIR, P, KC * P])
        w1_d = din("w1", [32, P, KC * P])
        w2_d = din("w2", [NPAIR, P, 32 * P])
        masks_d = din("masks", [NSLOT, 8, P, CW])
        bot_d = din("bot", [P, KC], F32)
        b1t_d = din("b1t", [P, 32], F32)
        b2t_d = din("b2t", [P, KC], F32)
        g1_d = din("g1c", [P, KC], F32)
        be1_d = din("be1c", [P, KC], F32)
        g2_d = din("g2c", [P, KC], F32)
        be2_d = din("be2c", [P, KC], F32)
        onesC_d = din("onesC", [P, 1])
        onesP_d = din("onesP", [P, P])

        out_d = dram.tile([C, NQ], F32, kind="ExternalOutput", name="o_out")
        names["out"] = out_d.name

        # ---- persistent small sbuf ----
        pers = top.enter_context(tc.tile_pool(name="pers", bufs=1))
        onesC = pers.tile([P, 1], BF, tag="onesC")
        nc.sync.dma_start(out=onesC[:], in_=onesC_d[:])
        onesP = pers.tile([P, P], BF, tag="onesP")
        nc.sync.dma_start(out=onesP[:], in_=onesP_d[:])
        bot = pers.tile([P, KC], F32, tag="bot")
        nc.sync.dma_start(out=bot[:], in_=bot_d[:])
        b1t = pers.tile([P, 32], F32, tag="b1t")
        nc.sync.dma_start(out=b1t[:], in_=b1t_d[:])
        b2t = pers.tile([P, KC], F32, tag="b2t")
        nc.sync.dma_start(out=b2t[:], in_=b2t_d[:])
        lncol = {}
        for nm, d in [("g1", g1_d), ("be1", be1_d), ("g2", g2_d), ("be2", be2_d)]:
            t = pers.tile([P, KC], F32, tag=f"ln_{nm}", name=f"ln_{nm}")
            nc.sync.dma_start(out=t[:], in_=d[:])
            lncol[nm] = t
        zero_col = pers.tile([P, 1], F32, tag="zero_col")
        nc.vector.memset(zero_col[:], 0.0)
        eps_col = pers.tile([P, 1], F32, tag="eps_col")
        nc.vector.memset(eps_col[:], LN_EPS)

        # =====================================================================
        # transposed-layout layernorm, bf16 (in place unless out_tiles given)
        # =====================================================================
        def layernorm_T(xtiles, n, gname, bname, out_tag, out_tiles=None):
            ctx = ExitStack()
            work = ctx.enter_context(tc.tile_pool(name=f"lnw_{out_tag}", bufs=2))
            stat = ctx.enter_context(tc.tile_pool(name=f"lns_{out_tag}", bufs=1))
            pL = ctx.enter_context(tc.tile_pool(name=f"lnp_{out_tag}", bufs=1,
                                                space="PSUM"))
            pB = ctx.enter_context(tc.tile_pool(name=f"lnb_{out_tag}", bufs=1,
                                                space="PSUM"))
            nn = n // 512
            mu_row = stat.tile([1, n], BF, tag="mu_row")
            msq_row = stat.tile([1, n], BF, tag="msq_row")
            for i in range(nn):
                s = slice(i * 512, (i + 1) * 512)
                ps_sum = pL.tile([1, 512], F32, tag="lsum", name="ps_sum")
                ps_sq = pL.tile([1, 512], F32, tag="lsq", name="ps_sq")
                for kc in range(KC):
                    sq = work.tile([P, 512], BF, tag="sq", name="sq")
                    nc.vector.tensor_tensor(sq[:], xtiles[kc][:, s],
                                            xtiles[kc][:, s], ALU.mult)
                    MM(ps_sum[:], onesC[:], xtiles[kc][:, s],
                       start=(kc == 0), stop=(kc == KC - 1), skip=True,
                       reuse=not (i == 0 and kc == 0))
                    MM(ps_sq[:], onesC[:], sq[:],
                       start=(kc == 0), stop=(kc == KC - 1), skip=True,
                       reuse=True)
                nc.scalar.activation(mu_row[:, s], ps_sum[:], AF.Copy, scale=1.0 / C)
                nc.scalar.activation(msq_row[:, s], ps_sq[:], AF.Copy, scale=1.0 / C)
            mu_b = stat.tile([P, n], BF, tag="mu_b")
            rstd_b = stat.tile([P, n], BF, tag="rstd_b")
            for i in range(nn):
                s = slice(i * 512, (i + 1) * 512)
                psb = pB.tile([P, 1024], F32, tag="bc", name="psb")
                MM(psb[:, 0:512], onesP[0:1, :], mu_row[:, s],
                   start=True, stop=True, skip=True, reuse=(i != 0))
                MM(psb[:, 512:1024], onesP[0:1, :], msq_row[:, s],
                   start=True, stop=True, skip=True, reuse=True)
                nc.vector.tensor_copy(mu_b[:, s], psb[:, 0:512])
                mu2 = work.tile([P, 512], BF, tag="mu2", name="mu2")
                nc.vector.tensor_tensor(mu2[:], mu_b[:, s], mu_b[:, s], ALU.mult)
                var = work.tile([P, 512], F32, tag="var", name="var")
                nc.vector.tensor_tensor(var[:], psb[:, 512:1024], mu2[:],
                                        ALU.subtract)
                nc.scalar.activation(var[:], var[:], AF.Ln, bias=eps_col[:])
                nc.scalar.activation(rstd_b[:, s], var[:], AF.Exp, scale=-0.5)
            outs = []
            for kc in range(KC):
                o = out_tiles[kc] if out_tiles is not None else xtiles[kc]
                nc.vector.tensor_tensor(o[:], xtiles[kc][:], mu_b[:], ALU.subtract)
                nc.vector.tensor_tensor(o[:], o[:], rstd_b[:], ALU.mult)
                nc.vector.tensor_scalar(o[:], o[:], lncol[gname][:, kc:kc + 1],
                                        lncol[bname][:, kc:kc + 1], ALU.mult, ALU.add)
                outs.append(o)
            ctx.close()
            return outs

        # =====================================================================
        # Phase 1: q path — LN1 of the core's own query columns, project
        # =====================================================================
        p_xTq = top.enter_context(tc.tile_pool(name="p_xTq", bufs=1))
        p_oT = top.enter_context(tc.tile_pool(name="p_oT", bufs=1))
        oT = [p_oT.tile([P, NSLOT, CW], BF, tag=f"oT{m}", name=f"oT{m}")
              for m in range(NPAIR)]
        att_ctx = ExitStack()
        p_qT = att_ctx.enter_context(tc.tile_pool(name="p_qT", bufs=1))
        xTq = []
        for kc in range(KC):
            tq = p_xTq.tile([P, NQ], BF, tag=f"xTq{kc}", name=f"xTq{kc}")
            nc.sync.dma_start(out=tq[:], in_=xTq_d[kc * P:(kc + 1) * P, :])
            xTq.append(tq)
        qT = []

        # =====================================================================
        # Phase 2: k and v paths — LN1 over the full sequence (Q projection is
        # issued right after LN1T so its matmuls overlap the LN1T apply ops)
        # =====================================================================
        p_kT = att_ctx.enter_context(tc.tile_pool(name="p_kT", bufs=1))
        p_v65 = att_ctx.enter_context(tc.tile_pool(name="p_v65", bufs=1))
        v65 = p_v65.tile([P, T // P, NPAIR, 2, 65], BF, tag="v65", name="v65")
        nc.vector.memset(v65[:, :, :, :, 64:65], 1.0)
        kT = []
        with ExitStack() as phk:
            p_xT = phk.enter_context(tc.tile_pool(name="p_xT", bufs=1))
            xT = []
            for kc in range(KC):
                t = p_xT.tile([P, T], BF, tag=f"xT{kc}", name=f"xT{kc}")
                nc.sync.dma_start(out=t[:], in_=xT_d[kc * P:(kc + 1) * P, :])
                xT.append(t)
            p_ln1q = phk.enter_context(tc.tile_pool(name="p_ln1q", bufs=1))
            ln1q_tiles = [p_ln1q.tile([P, NQ], BF, tag=f"ln1q{kc}",
                                      name=f"ln1q{kc}")
                          for kc in range(KC)]
            ln1q = layernorm_T(xTq, NQ, "g1", "be1", "ln1q",
                               out_tiles=ln1q_tiles)

            with ExitStack() as phqw:
                wqp = phqw.enter_context(tc.tile_pool(name="wqp", bufs=2))
                pA = phqw.enter_context(tc.tile_pool(name="pAq", bufs=4,
                                                     space="PSUM"))

                def _qproj(m):
                    wqm = wqp.tile([P, KC, P], BF, tag="wqm", name="wqm")
                    nc.sync.dma_start(
                        out=wqm[:], in_=wq_d[m].rearrange("p (k m) -> p k m", k=KC))
                    qt = p_qT.tile([P, NQ], BF, tag=f"qT{m}", name=f"qT{m}")
                    pss = [pA.tile([P, 512], F32, tag="proj", name="ps")
                           for _ in range(2)]
                    for kc in range(KC):
                        for nq in range(2):
                            MM(pss[nq][:], wqm[:, kc, :],
                               ln1q[kc][:, nq * 512:(nq + 1) * 512],
                               start=(kc == 0), stop=(kc == KC - 1), skip=True,
                               reuse=(nq > 0))
                    for nq in range(2):
                        nc.scalar.activation(qt[:, nq * 512:(nq + 1) * 512],
                                             pss[nq][:], AF.Copy)
                    qT.append(qt)

                # first half of Q fills the xT-DMA wait; second half fills
                # the LN1T apply window
                for m in range(4):
                    _qproj(m)
                ln1T = layernorm_T(xT, T, "g1", "be1", "ln1T")
                for m in range(4, NPAIR):
                    _qproj(m)

            with ExitStack() as phkw:
                wkp = phkw.enter_context(tc.tile_pool(name="wkp", bufs=2))
                pA = phkw.enter_context(tc.tile_pool(name="pAk", bufs=8,
                                                     space="PSUM"))
                for m in range(NPAIR):
                    wkm = wkp.tile([P, KC, P], BF, tag="wkm", name="wkm")
                    nc.sync.dma_start(
                        out=wkm[:], in_=wk_d[m].rearrange("p (k m) -> p k m", k=KC))
                    kt_t = p_kT.tile([P, T], BF, tag=f"kT{m}", name=f"kT{m}")
                    pss = [pA.tile([P, 512], F32, tag="proj", name="ps")
                           for _ in range(4)]
                    for kc in range(KC):
                        for n in range(4):
                            MM(pss[n][:], wkm[:, kc, :],
                               ln1T[kc][:, n * 512:(n + 1) * 512],
                               start=(kc == 0), stop=(kc == KC - 1), skip=True,
                               reuse=(n > 0))
                    for n in range(4):
                        nc.scalar.activation(kt_t[:, n * 512:(n + 1) * 512],
                                             pss[n][:], AF.Copy)
                    kT.append(kt_t)

            # v: token-major into resident v65 (ones col prefilled); one
            # stationary (ln1T chunk) feeds both output-dim halves
            with ExitStack() as phv:
                wvp = phv.enter_context(tc.tile_pool(name="wvp", bufs=1))
                pA = phv.enter_context(tc.tile_pool(name="pAv", bufs=4,
                                                    space="PSUM"))
                wvt = []
                for n in range(2):
                    wvn = wvp.tile([P, KC, 512], BF, tag=f"wvn{n}", name=f"wvn{n}")
                    nc.sync.dma_start(
                        out=wvn[:], in_=wv_d[n].rearrange("p (k d) -> p k d", k=KC))
                    wvt.append(wvn)
                for tt in range(T // P):
                    pss = [pA.tile([P, 512], F32, tag="proj", name="ps")
                           for _ in range(2)]
                    for kc in range(KC):
                        for n in range(2):
                            MM(pss[n][:], ln1T[kc][:, tt * P:(tt + 1) * P],
                               wvt[n][:, kc, :],
                               start=(kc == 0), stop=(kc == KC - 1), skip=True,
                               reuse=(n > 0))
                    for n in range(2):
                        nc.vector.tensor_copy(
                            v65[:, tt, 4 * n:4 * (n + 1), :, 0:64],
                            pss[n][:].rearrange("p (pr par d) -> p pr par d",
                                                pr=4, par=2))

        # =====================================================================
        # Phase 3: attention (ln1T freed; masks/avn fit above qT/kT/v65)
        # =====================================================================
        with ExitStack() as ph3:
            p_mask = ph3.enter_context(tc.tile_pool(name="p_mask", bufs=1))
            masks = {}
            for s in range(NSLOT):
                for j in range(8):
                    mt = p_mask.tile([P, CW], BF, tag=f"mask{s}_{j}",
                                     name=f"mask{s}_{j}")
                    nc.sync.dma_start(out=mt[:], in_=masks_d[s, j])
                    masks[(s, j)] = mt
            womp = ph3.enter_context(tc.tile_pool(name="womp", bufs=1))
            womt = []
            for m in range(NPAIR):
                w = womp.tile([P, KC, P], BF, tag=f"wom{m}", name=f"wom{m}")
                nc.sync.dma_start(
                    out=w[:], in_=wo_d[m].rearrange("p (k m) -> p k m", k=KC))
                womt.append(w)
            avn_pool = ph3.enter_context(tc.tile_pool(name="avn", bufs=2))
            sm_pool = ph3.enter_context(tc.tile_pool(name="sm", bufs=3))
            pt_pool = ph3.enter_context(tc.tile_pool(name="pt", bufs=4))

            psc_pool = ph3.enter_context(
                tc.tile_pool(name="psc", bufs=2, space="PSUM"))
            pav_pool = ph3.enter_context(
                tc.tile_pool(name="pav", bufs=2, space="PSUM"))
            for s in range(NSLOT):
                nkt = NKT_PROG[s]
                mask_base = 0 if s == 0 else 8
                qs = slice(s * CW, (s + 1) * CW)
                avn = {}
                avt = {}

                def _normalize(pair):
                    # evict, broadcast sums, 1/d = exp(-ln d), scale; issued
                    # one pair behind so its matmuls never wait on evictions
                    av = avt.pop(pair)
                    an = sm_pool.tile([65, 1024], BF, tag="an", name="an")
                    nc.vector.tensor_copy(an[:], av[0:65, :])
                    bc = psc_pool.tile([64, 1024], F32, tag="sc", name="bc")
                    MM(bc[:, 0:512], onesP[64:65, 0:64],
                       an[64:65, 0:512], start=True, stop=True, skip=True)
                    MM(bc[:, 512:1024], onesP[64:65, 0:64],
                       an[64:65, 512:1024], start=True, stop=True, skip=True,
                       reuse=True)
                    rec = sm_pool.tile([64, 1024], BF, tag="rec", name="rec")
                    lnd = sm_pool.tile([64, 1024], F32, tag="lnd", name="lnd")
                    nc.scalar.activation(lnd[:], bc[:], AF.Ln)
                    nc.scalar.activation(rec[:], lnd[:], AF.Exp, scale=-1.0)
                    anp = avn_pool.tile([P, CW], BF, tag=f"avn{pair}",
                                        name=f"avn{pair}")
                    nc.vector.tensor_tensor(anp[0:64, :], an[0:64, 0:512],
                                            rec[:, 0:512], ALU.mult)
                    tmo = sm_pool.tile([64, CW], BF, tag="tmo", name="tmo")
                    nc.vector.tensor_tensor(tmo[:], an[0:64, 512:1024],
                                            rec[:, 512:1024], ALU.mult)
                    nc.sync.dma_start(out=anp[64:128, :], in_=tmo[:])
                    avn[pair] = anp

                for pair in range(NPAIR):
                    av = pav_pool.tile([65, 1024], F32, tag="av", name="av")
                    avt[pair] = av
                    for kt in range(nkt):
                        kws = slice(kt * P, (kt + 1) * P)
                        psc = psc_pool.tile([P, 1024], F32, tag="sc",
                                            name="psc")
                        MM(psc[:, 0:512], kT[pair][0:64, kws],
                           qT[pair][0:64, qs], start=True, stop=True,
                           skip=True)
                        MM(psc[:, 512:1024], kT[pair][64:128, kws],
                           qT[pair][64:128, qs], start=True, stop=True,
                           skip=True)
                        pt = pt_pool.tile([P, 1024], BF, tag="pt", name="pt")
                        nc.scalar.activation(pt[:], psc[:], AF.Exp,
                                             bias=zero_col[:], scale=SCALE)
                        jm = kt - mask_base
                        if 0 <= jm < 8:
                            mt = masks[(s, jm)]
                            nc.vector.tensor_tensor(pt[:, 0:512], pt[:, 0:512],
                                                    mt[:], ALU.mult)
                            nc.vector.tensor_tensor(pt[:, 512:1024],
                                                    pt[:, 512:1024],
                                                    mt[:], ALU.mult)
                        st = (kt == 0)
                        sp = (kt == nkt - 1)
                        MM(av[0:65, 0:512], v65[:, kt, pair, 0, :],
                           pt[:, 0:512], start=st, stop=sp, skip=True)
                        MM(av[0:65, 512:1024], v65[:, kt, pair, 1, :],
                           pt[:, 512:1024], start=st, stop=sp, skip=True)
                    if pair > 0:
                        _normalize(pair - 1)
                _normalize(NPAIR - 1)
                # Wo for this slot (bias bo folded into eviction); psum
                # shares the score slot so both slots pipeline in 8 banks
                for m in range(NPAIR):
                    ps = psc_pool.tile([P, CW], F32, tag="sc", name="wops")
                    for k in range(NPAIR):
                        MM(ps[:], womt[m][:, k, :], avn[k][:],
                           start=(k == 0), stop=(k == NPAIR - 1), skip=True)
                    nc.vector.tensor_scalar(oT[m][:, s, :], ps[:],
                                            bot[:, m:m + 1], None, ALU.add)

        att_ctx.close()

        # =====================================================================
        # Phase 4: x2 = oT + xTq ; LN2 (not in place)
        # =====================================================================
        p_x2t = top.enter_context(tc.tile_pool(name="p_x2t", bufs=1))
        p_ln2T = top.enter_context(tc.tile_pool(name="p_ln2T", bufs=1))
        x2T = []
        ln2T_tiles = []
        for kc in range(KC):
            x2 = p_x2t.tile([P, NQ], BF, tag=f"x2t{kc}", name=f"x2t{kc}")
            nc.vector.tensor_tensor(
                x2[:], xTq[kc][:],
                oT[kc][:].rearrange("p s w -> p (s w)"), ALU.add)
            x2T.append(x2)
            lt = p_ln2T.tile([P, NQ], BF, tag=f"ln2T{kc}", name=f"ln2T{kc}")
            ln2T_tiles.append(lt)
        ln2T = layernorm_T(x2T, NQ, "g2", "be2", "ln2T", out_tiles=ln2T_tiles)

        # =====================================================================
        # Phase 5: FFN in two hidden-dim halves
        # =====================================================================
        with ExitStack() as ph5:
            ff1_pool = ph5.enter_context(tc.tile_pool(name="ff1", bufs=1))
            facc_pool = ph5.enter_context(tc.tile_pool(name="facc", bufs=1))
            w1_pool = ph5.enter_context(tc.tile_pool(name="w1s", bufs=2))
            w2_pool = ph5.enter_context(tc.tile_pool(name="w2s", bufs=2))
            fst_pool = ph5.enter_context(tc.tile_pool(name="fst", bufs=3))
            pF = ph5.enter_context(tc.tile_pool(name="pF", bufs=4, space="PSUM"))
            ffacc = [facc_pool.tile([P, NQ], BF, tag=f"facc{m}", name=f"ffacc{m}")
                     for m in range(KC)]
            for half in range(2):
                ff1 = []
                for m in range(16):
                    mm_i = half * 16 + m
                    w1m = w1_pool.tile([P, KC, P], BF, tag="w1m", name="w1m")
                    nc.sync.dma_start(
                        out=w1m[:],
                        in_=w1_d[mm_i].rearrange("p (k m) -> p k m", k=KC))
                    f = ff1_pool.tile([P, NQ], BF, tag=f"f{m}", name=f"f{m}")
                    pss = [pF.tile([P, 512], F32, tag="proj", name="ps")
                           for _ in range(2)]
                    for kc in range(KC):
                        for tch in range(2):
                            MM(pss[tch][:], w1m[:, kc, :],
                               ln2T[kc][:, tch * 512:(tch + 1) * 512],
                               start=(kc == 0), stop=(kc == KC - 1), skip=True,
                               reuse=(tch > 0))
                    for tch in range(2):
                        s = slice(tch * 512, (tch + 1) * 512)
                        # relu(x + b1) eviction
                        nc.vector.tensor_scalar(f[:, s], pss[tch][:],
                                                b1t[:, mm_i:mm_i + 1],
                                                0.0, ALU.add, ALU.max)
                    ff1.append(f)
                for mc in range(KC):
                    w2m = w2_pool.tile([P, 16, P], BF, tag="w2m", name="w2m")
                    nc.sync.dma_start(
                        out=w2m[:],
                        in_=w2_d[mc][:, half * 2048:(half + 1) * 2048]
                        .rearrange("p (k m) -> p k m", k=16))
                    pss = [pF.tile([P, 512], F32, tag="proj", name="ps")
                           for _ in range(2)]
                    for kt in range(16):
                        for tch in range(2):
                            MM(pss[tch][:], w2m[:, kt, :],
                               ff1[kt][:, tch * 512:(tch + 1) * 512],
                               start=(kt == 0), stop=(kt == 15), skip=True,
                               reuse=(tch > 0))
                    for tch in range(2):
                        s = slice(tch * 512, (tch + 1) * 512)
                        if half == 0:
                            nc.scalar.activation(ffacc[mc][:, s], pss[tch][:],
                                                 AF.Copy)
                        else:
                            o = fst_pool.tile([P, 512], F32, tag="fo", name="fo")
                            nc.vector.tensor_scalar(o[:], pss[tch][:],
                                                    b2t[:, mc:mc + 1],
                                                    None, ALU.add)
                            nc.vector.tensor_tensor(o[:], o[:], ffacc[mc][:, s],
                                                    ALU.add)
                            nc.vector.tensor_tensor(o[:], o[:], x2T[mc][:, s],
                                                    ALU.add)
                            nc.sync.dma_start(out=out_d[mc * P:(mc + 1) * P, s],
                                              in_=o[:])

    nc.compile()
    return nc, names


_CACHE = {}


def _get_built():
    if "nc" not in _CACHE:
        _CACHE["nc"], _CACHE["names"] = _build()
    return _CACHE["nc"], _CACHE["names"]


def _host_inputs(x, Wq, Wk, Wv, Wo, bo, ln1_g, ln1_b, ln2_g, ln2_b, W1, b1, W2, b2):
    """Build the 8 per-core input maps (host work = sharding/layout only)."""
    from ml_dtypes import bfloat16
    f = np.float32

    def wtile(W, nmb, nkc):
        # [mb, p, kc*P_or_512] with [mb,p,kc*w+j] = W[kc*P+p, mb*wout+j]
        kin, cout = W.shape
        wout = cout // nmb
        return np.ascontiguousarray(
            W.reshape(nkc, P, nmb, wout).transpose(2, 1, 0, 3)
            .reshape(nmb, P, nkc * wout).astype(bfloat16))

    shared = {
        "wq": wtile(np.asarray(Wq, f), NPAIR, KC),
        "wk": wtile(np.asarray(Wk, f), NPAIR, KC),
        "wv": wtile(np.asarray(Wv, f), 2, KC),
        "wo": wtile(np.asarray(Wo, f), NPAIR, KC),
        "w1": wtile(np.asarray(W1, f), 32, KC),
        "w2": wtile(np.asarray(W2, f), NPAIR, 32),
        "bot": np.ascontiguousarray(np.asarray(bo, f).reshape(KC, P).T),
        "b1t": np.ascontiguousarray(np.asarray(b1, f).reshape(32, P).T),
        "b2t": np.ascontiguousarray(np.asarray(b2, f).reshape(KC, P).T),
        "g1c": np.ascontiguousarray(np.asarray(ln1_g, f).reshape(KC, P).T),
        "be1c": np.ascontiguousarray(np.asarray(ln1_b, f).reshape(KC, P).T),
        "g2c": np.ascontiguousarray(np.asarray(ln2_g, f).reshape(KC, P).T),
        "be2c": np.ascontiguousarray(np.asarray(ln2_b, f).reshape(KC, P).T),
        "onesC": np.ones((P, 1), bfloat16),
        "onesP": np.ones((P, P), bfloat16),
    }
    kl = np.arange(P)[:, None]
    ql = np.arange(CW)[None, :]
    in_maps = []
    for c in range(8):
        b, r = c // 2, c % 2
        gs = GSETS[r]
        xTb = np.ascontiguousarray(np.asarray(x[b], f).T.astype(bfloat16))
        qcols = np.concatenate([np.arange(CW * g, CW * (g + 1)) for g in gs])
        xTq = np.ascontiguousarray(xTb[:, qcols])
        m = np.empty((NSLOT, 8, P, CW), bfloat16)
        for s in range(NSLOT):
            q0 = CW * gs[s]
            base = 0 if s == 0 else 8
            for j in range(8):
                kt = base + j
                m[s, j] = ((P * kt + kl) <= (q0 + ql)).astype(bfloat16)
        im = dict(shared)
        im["xT"] = xTb
        im["xTq"] = xTq
        im["masks"] = m
        in_maps.append(im)
    return in_maps


def _unshard(outs):
    out = np.empty((4, T, C), np.float32)
    for c in range(8):
        b, r = c // 2, c % 2
        oT = outs[c]  # (C, NQ)
        for s, g in enumerate(GSETS[r]):
            out[b, CW * g:CW * (g + 1), :] = oT[:, CW * s:CW * (s + 1)].T
    return out


def kernel(**inputs):
    from concourse.bass_utils import run_bass_kernel_spmd
    from concourse.bass_interp import get_hw_module

    args = {k: np.asarray(v, np.float32) for k, v in inputs.items()}
    in_maps_named = _host_inputs(**args)

    nc, names = _get_built()
    in_maps = [{names[k]: v for k, v in im.items()} for im in in_maps_named]

    hw = get_hw_module(nc.m)
    old = nc.m
    nc.m = hw
    try:
        res = run_bass_kernel_spmd(nc, in_maps, core_ids=list(range(8)))
    finally:
        nc.m = old
    outs = [r[names["out"]] for r in res.results]
    return _unshard(outs)


if __name__ == "__main__":
    import reference
    inp = {k: np.asarray(v) for k, v in reference.setup_inputs().items()}
    got = kernel(**inp)
    exp = np.asarray(reference.reference(**inp))
    err = np.abs(got - exp).max() / np.abs(exp).max()
    print("Relative error:", err)


# revision 23
# speedup vs baseline: 1.2057x; 1.0024x over previous
"""Trainium2 Bass kernel for a transformer MiniBlock (B=4, T=2048, C=1024, 16 heads,
causal attention, 4x FFN), sharded over 8 NeuronCores.

Sharding: core = (batch b=core//2, role r=core%2). Each core runs the full block for
1024 of its batch's 2048 tokens (two 512-token chunks, balanced for causal work:
role 0 owns chunks {0,3}, role 1 owns {1,2}), computing K/V over the full sequence
(no cross-core communication). The program is SPMD-uniform: k-window loop bounds are
per-slot maxima over roles; per-core causal masks (input data) zero the difference.

All tensors are bf16 on-chip (fp32 PSUM accumulation), which doubles effective
SBUF/DMA capacity, enables fast weight loads, and keeps rel-err ~3e-3. Activations
stay channel-major end to end; LN stats / softmax sums / broadcasts use small
ones-matmuls; the attention softmax is computed k-major with a ones-column appended
to V so denominators fall out of the AV matmul. Weights are pre-tiled on the host so
every weight DMA is fully contiguous. V stays resident in SBUF (no DRAM spill).
Even/odd head score matmuls are row-packed (tile_position) to run concurrently, and
exp is a single 1024-wide activation spanning two PSUM banks.
"""
import sys

sys.path.insert(0, "/opt/trn_rl_repo")

import numpy as np
from contextlib import ExitStack

import concourse.bacc as bacc
import concourse.mybir as mybir
import concourse.tile as tile
from concourse.tile import add_dep_helper

F32 = mybir.dt.float32
BF = mybir.dt.bfloat16
AF = mybir.ActivationFunctionType
ALU = mybir.AluOpType

P = 128
T = 2048          # full sequence
C = 1024          # embedding
NQ = 1024         # query tokens per core
H4 = 4096         # ffn hidden
NPAIR = 8         # head pairs (2 heads of 64 dims = 128 channels)
KC = C // P       # 8 channel tiles
NSLOT = 2         # 512-token query chunks per core
CW = 512          # chunk width
NKT_PROG = [8, 16]            # k-tiles per slot (program constant, max over roles)
GSETS = [[0, 3], [1, 2]]      # global 512-chunk index per slot, per role
LN_EPS = 1e-5
SCALE = float(64) ** -0.5     # head_size^-0.5 = 0.125


def _build():
    nc = bacc.Bacc(None, target_bir_lowering=False, debug=False)
    names = {}

    class _PE:
        """All matmuls go through here. A nosync dep chain pins PE issue
        order to program order, which makes ldweights=False (stationary
        reuse across consecutive matmuls) safe from scheduler interleaving."""
        prev = None

        @classmethod
        def mm(cls, out, stat, mov, start, stop, skip=False, reuse=False):
            inst = nc.tensor.matmul(out, stat, mov, start=start, stop=stop,
                                    skip_group_check=skip)
            # NOTE: walrus ignores ldweights=False (LDWEIGHTS is always
            # emitted per matmul) and a forced PE program-order dep chain
            # measured 40% slower on HW, so this helper is a plain matmul.
            return inst

    MM = _PE.mm
    with tile.TileContext(nc) as tc, ExitStack() as top:
        dram = top.enter_context(tc.tile_pool(name="io", bufs=1, space="DRAM"))

        def din(name, shape, dt=BF):
            t = dram.tile(shape, dt, kind="ExternalInput", name=f"i_{name}")
            names[name] = t.name
            return t

        xT_d = din("xT", [C, T])
        xTq_d = din("xTq", [C, NQ])
        wq_d = din("wq", [NPAIR, P, KC * P])
        wk_d = din("wk", [NPAIR, P, KC * P])
        wv_d = din("wv", [2, P, KC * 512])
        wo_d = din("wo", [NPAIR, P, KC * P])
        w1_d = din("w1", [32, P, KC * P])
        w2_d = din("w2", [NPAIR, P, 32 * P])
        masks_d = din("masks", [NSLOT, 8, P, CW])
        bot_d = din("bot", [P, KC], F32)
        b1t_d = din("b1t", [P, 32], F32)
        b2t_d = din("b2t", [P, KC], F32)
        g1_d = din("g1c", [P, KC], F32)
        be1_d = din("be1c", [P, KC], F32)
        g2_d = din("g2c", [P, KC], F32)
        be2_d = din("be2c", [P, KC], F32)
        onesC_d = din("onesC", [P, 1])
        onesP_d = din("onesP", [P, P])

        out_d = dram.tile([C, NQ], F32, kind="ExternalOutput", name="o_out")
        names["out"] = out_d.name

        # ---- persistent small sbuf ----
        pers = top.enter_context(tc.tile_pool(name="pers", bufs=1))
        onesC = pers.tile([P, 1], BF, tag="onesC")
        nc.sync.dma_start(out=onesC[:], in_=onesC_d[:])
        onesP = pers.tile([P, P], BF, tag="onesP")
        nc.sync.dma_start(out=onesP[:], in_=onesP_d[:])
        bot = pers.tile([P, KC], F32, tag="bot")
        nc.sync.dma_start(out=bot[:], in_=bot_d[:])
        b1t = pers.tile([P, 32], F32, tag="b1t")
        nc.sync.dma_start(out=b1t[:], in_=b1t_d[:])
        b2t = pers.tile([P, KC], F32, tag="b2t")
        nc.sync.dma_start(out=b2t[:], in_=b2t_d[:])
        lncol = {}
        for nm, d in [("g1", g1_d), ("be1", be1_d), ("g2", g2_d), ("be2", be2_d)]:
            t = pers.tile([P, KC], F32, tag=f"ln_{nm}", name=f"ln_{nm}")
            nc.sync.dma_start(out=t[:], in_=d[:])
            lncol[nm] = t
        zero_col = pers.tile([P, 1], F32, tag="zero_col")
        nc.vector.memset(zero_col[:], 0.0)
        eps_col = pers.tile([P, 1], F32, tag="eps_col")
        nc.vector.memset(eps_col[:], LN_EPS)

        # =====================================================================
        # transposed-layout layernorm, bf16 (in place unless out_tiles given)
        # =====================================================================
        def layernorm_T(xtiles, n, gname, bname, out_tag, out_tiles=None):
            ctx = ExitStack()
            work = ctx.enter_context(tc.tile_pool(name=f"lnw_{out_tag}", bufs=2))
            stat = ctx.enter_context(tc.tile_pool(name=f"lns_{out_tag}", bufs=1))
            pL = ctx.enter_context(tc.tile_pool(name=f"lnp_{out_tag}", bufs=1,
                                                space="PSUM"))
            pB = ctx.enter_context(tc.tile_pool(name=f"lnb_{out_tag}", bufs=1,
                                                space="PSUM"))
            nn = n // 512
            mu_row = stat.tile([1, n], BF, tag="mu_row")
            msq_row = stat.tile([1, n], BF, tag="msq_row")
            for i in range(nn):
                s = slice(i * 512, (i + 1) * 512)
                ps_sum = pL.tile([1, 512], F32, tag="lsum", name="ps_sum")
                ps_sq = pL.tile([1, 512], F32, tag="lsq", name="ps_sq")
                for kc in range(KC):
                    sq = work.tile([P, 512], BF, tag="sq", name="sq")
                    nc.vector.tensor_tensor(sq[:], xtiles[kc][:, s],
                                            xtiles[kc][:, s], ALU.mult)
                    MM(ps_sum[:], onesC[:], xtiles[kc][:, s],
                       start=(kc == 0), stop=(kc == KC - 1), skip=True,
                       reuse=not (i == 0 and kc == 0))
                    MM(ps_sq[:], onesC[:], sq[:],
                       start=(kc == 0), stop=(kc == KC - 1), skip=True,
                       reuse=True)
                nc.scalar.activation(mu_row[:, s], ps_sum[:], AF.Copy, scale=1.0 / C)
                nc.scalar.activation(msq_row[:, s], ps_sq[:], AF.Copy, scale=1.0 / C)
            mu_b = stat.tile([P, n], BF, tag="mu_b")
            rstd_b = stat.tile([P, n], BF, tag="rstd_b")
            for i in range(nn):
                s = slice(i * 512, (i + 1) * 512)
                psb = pB.tile([P, 1024], F32, tag="bc", name="psb")
                MM(psb[:, 0:512], onesP[0:1, :], mu_row[:, s],
                   start=True, stop=True, skip=True, reuse=(i != 0))
                MM(psb[:, 512:1024], onesP[0:1, :], msq_row[:, s],
                   start=True, stop=True, skip=True, reuse=True)
                nc.vector.tensor_copy(mu_b[:, s], psb[:, 0:512])
                mu2 = work.tile([P, 512], BF, tag="mu2", name="mu2")
                nc.vector.tensor_tensor(mu2[:], mu_b[:, s], mu_b[:, s], ALU.mult)
                var = work.tile([P, 512], F32, tag="var", name="var")
                nc.vector.tensor_tensor(var[:], psb[:, 512:1024], mu2[:],
                                        ALU.subtract)
                nc.scalar.activation(var[:], var[:], AF.Ln, bias=eps_col[:])
                nc.scalar.activation(rstd_b[:, s], var[:], AF.Exp, scale=-0.5)
            outs = []
            for kc in range(KC):
                o = out_tiles[kc] if out_tiles is not None else xtiles[kc]
                nc.vector.tensor_tensor(o[:], xtiles[kc][:], mu_b[:], ALU.subtract)
                nc.vector.tensor_tensor(o[:], o[:], rstd_b[:], ALU.mult)
                nc.vector.tensor_scalar(o[:], o[:], lncol[gname][:, kc:kc + 1],
                                        lncol[bname][:, kc:kc + 1], ALU.mult, ALU.add)
                outs.append(o)
            ctx.close()
            return outs

        # =====================================================================
        # Phase 1: q path — LN1 of the core's own query columns, project
        # =====================================================================
        p_xTq = top.enter_context(tc.tile_pool(name="p_xTq", bufs=1))
        p_oT = top.enter_context(tc.tile_pool(name="p_oT", bufs=1))
        oT = [p_oT.tile([P, NSLOT, CW], BF, tag=f"oT{m}", name=f"oT{m}")
              for m in range(NPAIR)]
        att_ctx = ExitStack()
        p_qT = att_ctx.enter_context(tc.tile_pool(name="p_qT", bufs=1))
        xTq = []
        for kc in range(KC):
            tq = p_xTq.tile([P, NQ], BF, tag=f"xTq{kc}", name=f"xTq{kc}")
            nc.sync.dma_start(out=tq[:], in_=xTq_d[kc * P:(kc + 1) * P, :])
            xTq.append(tq)
        qT = []

        # =====================================================================
        # Phase 2: k and v paths — LN1 over the full sequence (Q projection is
        # issued right after LN1T so its matmuls overlap the LN1T apply ops)
        # =====================================================================
        p_kT = att_ctx.enter_context(tc.tile_pool(name="p_kT", bufs=1))
        p_v65 = att_ctx.enter_context(tc.tile_pool(name="p_v65", bufs=1))
        v65 = p_v65.tile([P, T // P, NPAIR, 2, 65], BF, tag="v65", name="v65")
        nc.vector.memset(v65[:, :, :, :, 64:65], 1.0)
        kT = []
        with ExitStack() as phk:
            p_xT = phk.enter_context(tc.tile_pool(name="p_xT", bufs=1))
            xT = []
            for kc in range(KC):
                t = p_xT.tile([P, T], BF, tag=f"xT{kc}", name=f"xT{kc}")
                nc.sync.dma_start(out=t[:], in_=xT_d[kc * P:(kc + 1) * P, :])
                xT.append(t)
            p_ln1q = phk.enter_context(tc.tile_pool(name="p_ln1q", bufs=1))
            ln1q_tiles = [p_ln1q.tile([P, NQ], BF, tag=f"ln1q{kc}",
                                      name=f"ln1q{kc}")
                          for kc in range(KC)]
            ln1q = layernorm_T(xTq, NQ, "g1", "be1", "ln1q",
                               out_tiles=ln1q_tiles)

            with ExitStack() as phqw:
                wqp = phqw.enter_context(tc.tile_pool(name="wqp", bufs=2))
                pA = phqw.enter_context(tc.tile_pool(name="pAq", bufs=4,
                                                     space="PSUM"))

                def _qproj(m):
                    wqm = wqp.tile([P, KC, P], BF, tag="wqm", name="wqm")
                    nc.sync.dma_start(
                        out=wqm[:], in_=wq_d[m].rearrange("p (k m) -> p k m", k=KC))
                    qt = p_qT.tile([P, NQ], BF, tag=f"qT{m}", name=f"qT{m}")
                    pss = [pA.tile([P, 512], F32, tag="proj", name="ps")
                           for _ in range(2)]
                    for kc in range(KC):
                        for nq in range(2):
                            MM(pss[nq][:], wqm[:, kc, :],
                               ln1q[kc][:, nq * 512:(nq + 1) * 512],
                               start=(kc == 0), stop=(kc == KC - 1), skip=True,
                               reuse=(nq > 0))
                    for nq in range(2):
                        nc.scalar.activation(qt[:, nq * 512:(nq + 1) * 512],
                                             pss[nq][:], AF.Copy)
                    qT.append(qt)

                # first half of Q fills the xT-DMA wait; second half fills
                # the LN1T apply window
                for m in range(4):
                    _qproj(m)
                ln1T = layernorm_T(xT, T, "g1", "be1", "ln1T")
                for m in range(4, NPAIR):
                    _qproj(m)

            with ExitStack() as phkw:
                wkp = phkw.enter_context(tc.tile_pool(name="wkp", bufs=2))
                pKV = phkw.enter_context(tc.tile_pool(name="pAkv", bufs=8,
                                                      space="PSUM"))
                pA = pKV
                for m in range(NPAIR):
                    wkm = wkp.tile([P, KC, P], BF, tag="wkm", name="wkm")
                    nc.sync.dma_start(
                        out=wkm[:], in_=wk_d[m].rearrange("p (k m) -> p k m", k=KC))
                    kt_t = p_kT.tile([P, T], BF, tag=f"kT{m}", name=f"kT{m}")
                    pss = [pA.tile([P, 512], F32, tag="proj", name="ps")
                           for _ in range(4)]
                    for kc in range(KC):
                        for n in range(4):
                            MM(pss[n][:], wkm[:, kc, :],
                               ln1T[kc][:, n * 512:(n + 1) * 512],
                               start=(kc == 0), stop=(kc == KC - 1), skip=True,
                               reuse=(n > 0))
                    for n in range(4):
                        nc.scalar.activation(kt_t[:, n * 512:(n + 1) * 512],
                                             pss[n][:], AF.Copy)
                    kT.append(kt_t)

                # v: token-major into resident v65 (ones col prefilled);
                # one stationary (ln1T chunk) feeds both output-dim halves.
                # Shares the K psum pool so the K->V handoff has no pool
                # close/reopen serialization.
                wvp = phkw.enter_context(tc.tile_pool(name="wvp", bufs=1))
                pA = pKV
                wvt = []
                for n in range(2):
                    wvn = wvp.tile([P, KC, 512], BF, tag=f"wvn{n}", name=f"wvn{n}")
                    nc.sync.dma_start(
                        out=wvn[:], in_=wv_d[n].rearrange("p (k d) -> p k d", k=KC))
                    wvt.append(wvn)
                for tt in range(T // P):
                    pss = [pA.tile([P, 512], F32, tag="proj", name="ps")
                           for _ in range(2)]
                    for kc in range(KC):
                        for n in range(2):
                            MM(pss[n][:], ln1T[kc][:, tt * P:(tt + 1) * P],
                               wvt[n][:, kc, :],
                               start=(kc == 0), stop=(kc == KC - 1), skip=True,
                               reuse=(n > 0))
                    for n in range(2):
                        nc.vector.tensor_copy(
                            v65[:, tt, 4 * n:4 * (n + 1), :, 0:64],
                            pss[n][:].rearrange("p (pr par d) -> p pr par d",
                                                pr=4, par=2))

        # =====================================================================
        # Phase 3: attention (ln1T freed; masks/avn fit above qT/kT/v65)
        # =====================================================================
        with ExitStack() as ph3:
            p_mask = ph3.enter_context(tc.tile_pool(name="p_mask", bufs=1))
            masks = {}
            for s in range(NSLOT):
                for j in range(8):
                    mt = p_mask.tile([P, CW], BF, tag=f"mask{s}_{j}",
                                     name=f"mask{s}_{j}")
                    nc.sync.dma_start(out=mt[:], in_=masks_d[s, j])
                    masks[(s, j)] = mt
            womp = ph3.enter_context(tc.tile_pool(name="womp", bufs=1))
            womt = []
            for m in range(NPAIR):
                w = womp.tile([P, KC, P], BF, tag=f"wom{m}", name=f"wom{m}")
                nc.sync.dma_start(
                    out=w[:], in_=wo_d[m].rearrange("p (k m) -> p k m", k=KC))
                womt.append(w)
            avn_pool = ph3.enter_context(tc.tile_pool(name="avn", bufs=2))
            sm_pool = ph3.enter_context(tc.tile_pool(name="sm", bufs=3))
            pt_pool = ph3.enter_context(tc.tile_pool(name="pt", bufs=4))

            psc_pool = ph3.enter_context(
                tc.tile_pool(name="psc", bufs=2, space="PSUM"))
            pav_pool = ph3.enter_context(
                tc.tile_pool(name="pav", bufs=2, space="PSUM"))
            for s in range(NSLOT):
                nkt = NKT_PROG[s]
                mask_base = 0 if s == 0 else 8
                qs = slice(s * CW, (s + 1) * CW)
                avn = {}
                avt = {}

                def _normalize(pair):
                    # evict, broadcast sums, 1/d = exp(-ln d), scale; issued
                    # one pair behind so its matmuls never wait on evictions
                    av = avt.pop(pair)
                    an = sm_pool.tile([65, 1024], BF, tag="an", name="an")
                    nc.vector.tensor_copy(an[:], av[0:65, :])
                    bc = psc_pool.tile([64, 1024], F32, tag="sc", name="bc")
                    MM(bc[:, 0:512], onesP[64:65, 0:64],
                       an[64:65, 0:512], start=True, stop=True, skip=True)
                    MM(bc[:, 512:1024], onesP[64:65, 0:64],
                       an[64:65, 512:1024], start=True, stop=True, skip=True,
                       reuse=True)
                    rec = sm_pool.tile([64, 1024], BF, tag="rec", name="rec")
                    lnd = sm_pool.tile([64, 1024], F32, tag="lnd", name="lnd")
                    nc.scalar.activation(lnd[:], bc[:], AF.Ln)
                    nc.scalar.activation(rec[:], lnd[:], AF.Exp, scale=-1.0)
                    anp = avn_pool.tile([P, CW], BF, tag=f"avn{pair}",
                                        name=f"avn{pair}")
                    nc.vector.tensor_tensor(anp[0:64, :], an[0:64, 0:512],
                                            rec[:, 0:512], ALU.mult)
                    tmo = sm_pool.tile([64, CW], BF, tag="tmo", name="tmo")
                    nc.vector.tensor_tensor(tmo[:], an[0:64, 512:1024],
                                            rec[:, 512:1024], ALU.mult)
                    nc.sync.dma_start(out=anp[64:128, :], in_=tmo[:])
                    avn[pair] = anp

                for pair in range(NPAIR):
                    av = pav_pool.tile([65, 1024], F32, tag="av", name="av")
                    avt[pair] = av
                    for kt in range(nkt):
                        kws = slice(kt * P, (kt + 1) * P)
                        psc = psc_pool.tile([P, 1024], F32, tag="sc",
                                            name="psc")
                        MM(psc[:, 0:512], kT[pair][0:64, kws],
                           qT[pair][0:64, qs], start=True, stop=True,
                           skip=True)
                        MM(psc[:, 512:1024], kT[pair][64:128, kws],
                           qT[pair][64:128, qs], start=True, stop=True,
                           skip=True)
                        pt = pt_pool.tile([P, 1024], BF, tag="pt", name="pt")
                        nc.scalar.activation(pt[:], psc[:], AF.Exp,
                                             bias=zero_col[:], scale=SCALE)
                        jm = kt - mask_base
                        if 0 <= jm < 8:
                            mt = masks[(s, jm)]
                            nc.vector.tensor_tensor(pt[:, 0:512], pt[:, 0:512],
                                                    mt[:], ALU.mult)
                            nc.vector.tensor_tensor(pt[:, 512:1024],
                                                    pt[:, 512:1024],
                                                    mt[:], ALU.mult)
                        st = (kt == 0)
                        sp = (kt == nkt - 1)
                        MM(av[0:65, 0:512], v65[:, kt, pair, 0, :],
                           pt[:, 0:512], start=st, stop=sp, skip=True)
                        MM(av[0:65, 512:1024], v65[:, kt, pair, 1, :],
                           pt[:, 512:1024], start=st, stop=sp, skip=True)
                    if pair > 0:
                        _normalize(pair - 1)
                _normalize(NPAIR - 1)
                # Wo for this slot (bias bo folded into eviction); psum
                # shares the score slot so both slots pipeline in 8 banks
                for m in range(NPAIR):
                    ps = psc_pool.tile([P, CW], F32, tag="sc", name="wops")
                    for k in range(NPAIR):
                        MM(ps[:], womt[m][:, k, :], avn[k][:],
                           start=(k == 0), stop=(k == NPAIR - 1), skip=True)
                    nc.vector.tensor_scalar(oT[m][:, s, :], ps[:],
                                            bot[:, m:m + 1], None, ALU.add)

        att_ctx.close()

        # =====================================================================
        # Phase 4: x2 = oT + xTq ; LN2 (not in place)
        # =====================================================================
        p_x2t = top.enter_context(tc.tile_pool(name="p_x2t", bufs=1))
        p_ln2T = top.enter_context(tc.tile_pool(name="p_ln2T", bufs=1))
        x2T = []
        ln2T_tiles = []
        for kc in range(KC):
            x2 = p_x2t.tile([P, NQ], BF, tag=f"x2t{kc}", name=f"x2t{kc}")
            nc.vector.tensor_tensor(
                x2[:], xTq[kc][:],
                oT[kc][:].rearrange("p s w -> p (s w)"), ALU.add)
            x2T.append(x2)
            lt = p_ln2T.tile([P, NQ], BF, tag=f"ln2T{kc}", name=f"ln2T{kc}")
            ln2T_tiles.append(lt)
        ln2T = layernorm_T(x2T, NQ, "g2", "be2", "ln2T", out_tiles=ln2T_tiles)

        # =====================================================================
        # Phase 5: FFN in two hidden-dim halves
        # =====================================================================
        with ExitStack() as ph5:
            ff1_pool = ph5.enter_context(tc.tile_pool(name="ff1", bufs=1))
            facc_pool = ph5.enter_context(tc.tile_pool(name="facc", bufs=1))
            w1_pool = ph5.enter_context(tc.tile_pool(name="w1s", bufs=2))
            w2_pool = ph5.enter_context(tc.tile_pool(name="w2s", bufs=2))
            fst_pool = ph5.enter_context(tc.tile_pool(name="fst", bufs=3))
            pF = ph5.enter_context(tc.tile_pool(name="pF", bufs=4, space="PSUM"))
            ffacc = [facc_pool.tile([P, NQ], BF, tag=f"facc{m}", name=f"ffacc{m}")
                     for m in range(KC)]
            for half in range(2):
                ff1 = []
                for m in range(16):
                    mm_i = half * 16 + m
                    w1m = w1_pool.tile([P, KC, P], BF, tag="w1m", name="w1m")
                    nc.sync.dma_start(
                        out=w1m[:],
                        in_=w1_d[mm_i].rearrange("p (k m) -> p k m", k=KC))
                    f = ff1_pool.tile([P, NQ], BF, tag=f"f{m}", name=f"f{m}")
                    pss = [pF.tile([P, 512], F32, tag="proj", name="ps")
                           for _ in range(2)]
                    for kc in range(KC):
                        for tch in range(2):
                            MM(pss[tch][:], w1m[:, kc, :],
                               ln2T[kc][:, tch * 512:(tch + 1) * 512],
                               start=(kc == 0), stop=(kc == KC - 1), skip=True,
                               reuse=(tch > 0))
                    for tch in range(2):
                        s = slice(tch * 512, (tch + 1) * 512)
                        # relu(x + b1) eviction
                        nc.vector.tensor_scalar(f[:, s], pss[tch][:],
                                                b1t[:, mm_i:mm_i + 1],
                                                0.0, ALU.add, ALU.max)
                    ff1.append(f)
                for mc in range(KC):
                    w2m = w2_pool.tile([P, 16, P], BF, tag="w2m", name="w2m")
                    nc.sync.dma_start(
                        out=w2m[:],
                        in_=w2_d[mc][:, half * 2048:(half + 1) * 2048]
                        .rearrange("p (k m) -> p k m", k=16))
                    pss = [pF.tile([P, 512], F32, tag="proj", name="ps")
                           for _ in range(2)]
                    for kt in range(16):
                        for tch in range(2):
                            MM(pss[tch][:], w2m[:, kt, :],
                               ff1[kt][:, tch * 512:(tch + 1) * 512],
                               start=(kt == 0), stop=(kt == 15), skip=True,
                               reuse=(tch > 0))
                    for tch in range(2):
                        s = slice(tch * 512, (tch + 1) * 512)
                        if half == 0:
                            nc.scalar.activation(ffacc[mc][:, s], pss[tch][:],
                                                 AF.Copy)
                        else:
                            o = fst_pool.tile([P, 512], F32, tag="fo", name="fo")
                            nc.vector.tensor_scalar(o[:], pss[tch][:],
                                                    b2t[:, mc:mc + 1],
                                                    None, ALU.add)
                            nc.vector.tensor_tensor(o[:], o[:], ffacc[mc][:, s],
                                                    ALU.add)
                            nc.vector.tensor_tensor(o[:], o[:], x2T[mc][:, s],
                                                    ALU.add)
                            nc.sync.dma_start(out=out_d[mc * P:(mc + 1) * P, s],
                                              in_=o[:])

    nc.compile()
    return nc, names


_CACHE = {}


def _get_built():
    if "nc" not in _CACHE:
        _CACHE["nc"], _CACHE["names"] = _build()
    return _CACHE["nc"], _CACHE["names"]


def _host_inputs(x, Wq, Wk, Wv, Wo, bo, ln1_g, ln1_b, ln2_g, ln2_b, W1, b1, W2, b2):
    """Build the 8 per-core input maps (host work = sharding/layout only)."""
    from ml_dtypes import bfloat16
    f = np.float32

    def wtile(W, nmb, nkc):
        # [mb, p, kc*P_or_512] with [mb,p,kc*w+j] = W[kc*P+p, mb*wout+j]
        kin, cout = W.shape
        wout = cout // nmb
        return np.ascontiguousarray(
            W.reshape(nkc, P, nmb, wout).transpose(2, 1, 0, 3)
            .reshape(nmb, P, nkc * wout).astype(bfloat16))

    shared = {
        "wq": wtile(np.asarray(Wq, f), NPAIR, KC),
        "wk": wtile(np.asarray(Wk, f), NPAIR, KC),
        "wv": wtile(np.asarray(Wv, f), 2, KC),
        "wo": wtile(np.asarray(Wo, f), NPAIR, KC),
        "w1": wtile(np.asarray(W1, f), 32, KC),
        "w2": wtile(np.asarray(W2, f), NPAIR, 32),
        "bot": np.ascontiguousarray(np.asarray(bo, f).reshape(KC, P).T),
        "b1t": np.ascontiguousarray(np.asarray(b1, f).reshape(32, P).T),
        "b2t": np.ascontiguousarray(np.asarray(b2, f).reshape(KC, P).T),
        "g1c": np.ascontiguousarray(np.asarray(ln1_g, f).reshape(KC, P).T),
        "be1c": np.ascontiguousarray(np.asarray(ln1_b, f).reshape(KC, P).T),
        "g2c": np.ascontiguousarray(np.asarray(ln2_g, f).reshape(KC, P).T),
        "be2c": np.ascontiguousarray(np.asarray(ln2_b, f).reshape(KC, P).T),
        "onesC": np.ones((P, 1), bfloat16),
        "onesP": np.ones((P, P), bfloat16),
    }
    kl = np.arange(P)[:, None]
    ql = np.arange(CW)[None, :]
    in_maps = []
    for c in range(8):
        b, r = c // 2, c % 2
        gs = GSETS[r]
        xTb = np.ascontiguousarray(np.asarray(x[b], f).T.astype(bfloat16))
        qcols = np.concatenate([np.arange(CW * g, CW * (g + 1)) for g in gs])
        xTq = np.ascontiguousarray(xTb[:, qcols])
        m = np.empty((NSLOT, 8, P, CW), bfloat16)
        for s in range(NSLOT):
            q0 = CW * gs[s]
            base = 0 if s == 0 else 8
            for j in range(8):
                kt = base + j
                m[s, j] = ((P * kt + kl) <= (q0 + ql)).astype(bfloat16)
        im = dict(shared)
        im["xT"] = xTb
        im["xTq"] = xTq
        im["masks"] = m
        in_maps.append(im)
    return in_maps


def _unshard(outs):
    out = np.empty((4, T, C), np.float32)
    for c in range(8):
        b, r = c // 2, c % 2
        oT = outs[c]  # (C, NQ)
        for s, g in enumerate(GSETS[r]):
            out[b, CW * g:CW * (g + 1), :] = oT[:, CW * s:CW * (s + 1)].T
    return out


def kernel(**inputs):
    from concourse.bass_utils import run_bass_kernel_spmd
    from concourse.bass_interp import get_hw_module

    args = {k: np.asarray(v, np.float32) for k, v in inputs.items()}
    in_maps_named = _host_inputs(**args)

    nc, names = _get_built()
    in_maps = [{names[k]: v for k, v in im.items()} for im in in_maps_named]

    hw = get_hw_module(nc.m)
    old = nc.m
    nc.m = hw
    try:
        res = run_bass_kernel_spmd(nc, in_maps, core_ids=list(range(8)))
    finally:
        nc.m = old
    outs = [r[names["out"]] for r in res.results]
    return _unshard(outs)


if __name__ == "__main__":
    import reference
    inp = {k: np.asarray(v) for k, v in reference.setup_inputs().items()}
    got = kernel(**inp)
    exp = np.asarray(reference.reference(**inp))
    err = np.abs(got - exp).max() / np.abs(exp).max()
    print("Relative error:", err)


# revision 25
# speedup vs baseline: 1.2148x; 1.0075x over previous
"""Trainium2 Bass kernel for a transformer MiniBlock (B=4, T=2048, C=1024, 16 heads,
causal attention, 4x FFN), sharded over 8 NeuronCores.

Sharding: core = (batch b=core//2, role r=core%2). Each core runs the full block for
1024 of its batch's 2048 tokens (two 512-token chunks, balanced for causal work:
role 0 owns chunks {0,3}, role 1 owns {1,2}), computing K/V over the full sequence
(no cross-core communication). The program is SPMD-uniform: k-window loop bounds are
per-slot maxima over roles; per-core causal masks (input data) zero the difference.

All tensors are bf16 on-chip (fp32 PSUM accumulation), which doubles effective
SBUF/DMA capacity, enables fast weight loads, and keeps rel-err ~3e-3. Activations
stay channel-major end to end; LN stats / softmax sums / broadcasts use small
ones-matmuls; the attention softmax is computed k-major with a ones-column appended
to V so denominators fall out of the AV matmul. Weights are pre-tiled on the host so
every weight DMA is fully contiguous. V stays resident in SBUF (no DRAM spill).
Even/odd head score matmuls are row-packed (tile_position) to run concurrently, and
exp is a single 1024-wide activation spanning two PSUM banks.
"""
import sys

sys.path.insert(0, "/opt/trn_rl_repo")

import numpy as np
from contextlib import ExitStack

import concourse.bacc as bacc
import concourse.mybir as mybir
import concourse.tile as tile
from concourse.tile import add_dep_helper

F32 = mybir.dt.float32
BF = mybir.dt.bfloat16
AF = mybir.ActivationFunctionType
ALU = mybir.AluOpType

P = 128
T = 2048          # full sequence
C = 1024          # embedding
NQ = 1024         # query tokens per core
H4 = 4096         # ffn hidden
NPAIR = 8         # head pairs (2 heads of 64 dims = 128 channels)
KC = C // P       # 8 channel tiles
NSLOT = 2         # 512-token query chunks per core
CW = 512          # chunk width
NKT_PROG = [8, 16]            # k-tiles per slot (program constant, max over roles)
GSETS = [[0, 3], [1, 2]]      # global 512-chunk index per slot, per role
LN_EPS = 1e-5
SCALE = float(64) ** -0.5     # head_size^-0.5 = 0.125


def _build():
    nc = bacc.Bacc(None, target_bir_lowering=False, debug=False)
    names = {}

    class _PE:
        """All matmuls go through here. A nosync dep chain pins PE issue
        order to program order, which makes ldweights=False (stationary
        reuse across consecutive matmuls) safe from scheduler interleaving."""
        prev = None

        @classmethod
        def mm(cls, out, stat, mov, start, stop, skip=False, reuse=False):
            inst = nc.tensor.matmul(out, stat, mov, start=start, stop=stop,
                                    skip_group_check=skip)
            # NOTE: walrus ignores ldweights=False (LDWEIGHTS is always
            # emitted per matmul) and a forced PE program-order dep chain
            # measured 40% slower on HW, so this helper is a plain matmul.
            return inst

    MM = _PE.mm
    with tile.TileContext(nc) as tc, ExitStack() as top:
        dram = top.enter_context(tc.tile_pool(name="io", bufs=1, space="DRAM"))

        def din(name, shape, dt=BF):
            t = dram.tile(shape, dt, kind="ExternalInput", name=f"i_{name}")
            names[name] = t.name
            return t

        xT_d = din("xT", [C, T])
        xTq_d = din("xTq", [C, NQ])
        wq_d = din("wq", [NPAIR, P, KC * P])
        wk_d = din("wk", [NPAIR, P, KC * P])
        wv_d = din("wv", [2, P, KC * 512])
        wo_d = din("wo", [NPAIR, P, KC * P])
        w1_d = din("w1", [32, P, KC * P])
        w2_d = din("w2", [NPAIR, P, 32 * P])
        masks_d = din("masks", [NSLOT, 8, P, CW])
        bot_d = din("bot", [P, KC], F32)
        b1t_d = din("b1t", [P, 32], F32)
        b2t_d = din("b2t", [P, KC], F32)
        g1_d = din("g1c", [P, KC], F32)
        be1_d = din("be1c", [P, KC], F32)
        g2_d = din("g2c", [P, KC], F32)
        be2_d = din("be2c", [P, KC], F32)
        onesC_d = din("onesC", [P, 1])
        onesP_d = din("onesP", [P, P])

        out_d = dram.tile([C, NQ], F32, kind="ExternalOutput", name="o_out")
        names["out"] = out_d.name

        # ---- persistent small sbuf ----
        pers = top.enter_context(tc.tile_pool(name="pers", bufs=1))
        onesC = pers.tile([P, 1], BF, tag="onesC")
        nc.sync.dma_start(out=onesC[:], in_=onesC_d[:])
        onesP = pers.tile([P, P], BF, tag="onesP")
        nc.sync.dma_start(out=onesP[:], in_=onesP_d[:])
        bot = pers.tile([P, KC], F32, tag="bot")
        nc.sync.dma_start(out=bot[:], in_=bot_d[:])
        b1t = pers.tile([P, 32], F32, tag="b1t")
        nc.sync.dma_start(out=b1t[:], in_=b1t_d[:])
        b2t = pers.tile([P, KC], F32, tag="b2t")
        nc.sync.dma_start(out=b2t[:], in_=b2t_d[:])
        lncol = {}
        for nm, d in [("g1", g1_d), ("be1", be1_d), ("g2", g2_d), ("be2", be2_d)]:
            t = pers.tile([P, KC], F32, tag=f"ln_{nm}", name=f"ln_{nm}")
            nc.sync.dma_start(out=t[:], in_=d[:])
            lncol[nm] = t
        zero_col = pers.tile([P, 1], F32, tag="zero_col")
        nc.vector.memset(zero_col[:], 0.0)
        eps_col = pers.tile([P, 1], F32, tag="eps_col")
        nc.vector.memset(eps_col[:], LN_EPS)

        # =====================================================================
        # transposed-layout layernorm, bf16 (in place unless out_tiles given)
        # =====================================================================
        def layernorm_T(xtiles, n, gname, bname, out_tag, out_tiles=None):
            ctx = ExitStack()
            work = ctx.enter_context(tc.tile_pool(name=f"lnw_{out_tag}", bufs=2))
            stat = ctx.enter_context(tc.tile_pool(name=f"lns_{out_tag}", bufs=1))
            pL = ctx.enter_context(tc.tile_pool(name=f"lnp_{out_tag}", bufs=1,
                                                space="PSUM"))
            pB = ctx.enter_context(tc.tile_pool(name=f"lnb_{out_tag}", bufs=1,
                                                space="PSUM"))
            nn = n // 512
            mu_row = stat.tile([1, n], BF, tag="mu_row")
            msq_row = stat.tile([1, n], BF, tag="msq_row")
            for i in range(nn):
                s = slice(i * 512, (i + 1) * 512)
                ps_sum = pL.tile([1, 512], F32, tag="lsum", name="ps_sum")
                ps_sq = pL.tile([1, 512], F32, tag="lsq", name="ps_sq")
                for kc in range(KC):
                    sq = work.tile([P, 512], BF, tag="sq", name="sq")
                    nc.vector.tensor_tensor(sq[:], xtiles[kc][:, s],
                                            xtiles[kc][:, s], ALU.mult)
                    MM(ps_sum[:], onesC[:], xtiles[kc][:, s],
                       start=(kc == 0), stop=(kc == KC - 1), skip=True,
                       reuse=not (i == 0 and kc == 0))
                    MM(ps_sq[:], onesC[:], sq[:],
                       start=(kc == 0), stop=(kc == KC - 1), skip=True,
                       reuse=True)
                nc.scalar.activation(mu_row[:, s], ps_sum[:], AF.Copy, scale=1.0 / C)
                nc.scalar.activation(msq_row[:, s], ps_sq[:], AF.Copy, scale=1.0 / C)
            mu_b = stat.tile([P, n], BF, tag="mu_b")
            rstd_b = stat.tile([P, n], BF, tag="rstd_b")
            for i in range(nn):
                s = slice(i * 512, (i + 1) * 512)
                psb = pB.tile([P, 1024], F32, tag="bc", name="psb")
                MM(psb[:, 0:512], onesP[0:1, :], mu_row[:, s],
                   start=True, stop=True, skip=True, reuse=(i != 0))
                MM(psb[:, 512:1024], onesP[0:1, :], msq_row[:, s],
                   start=True, stop=True, skip=True, reuse=True)
                nc.vector.tensor_copy(mu_b[:, s], psb[:, 0:512])
                mu2 = work.tile([P, 512], BF, tag="mu2", name="mu2")
                nc.vector.tensor_tensor(mu2[:], mu_b[:, s], mu_b[:, s], ALU.mult)
                var = work.tile([P, 512], F32, tag="var", name="var")
                nc.vector.tensor_tensor(var[:], psb[:, 512:1024], mu2[:],
                                        ALU.subtract)
                nc.scalar.activation(var[:], var[:], AF.Ln, bias=eps_col[:])
                nc.scalar.activation(rstd_b[:, s], var[:], AF.Exp, scale=-0.5)
            outs = []
            for kc in range(KC):
                o = out_tiles[kc] if out_tiles is not None else xtiles[kc]
                nc.vector.tensor_tensor(o[:], xtiles[kc][:], mu_b[:], ALU.subtract)
                nc.vector.tensor_tensor(o[:], o[:], rstd_b[:], ALU.mult)
                nc.vector.tensor_scalar(o[:], o[:], lncol[gname][:, kc:kc + 1],
                                        lncol[bname][:, kc:kc + 1], ALU.mult, ALU.add)
                outs.append(o)
            ctx.close()
            return outs

        # =====================================================================
        # Phase 1: q path — LN1 of the core's own query columns, project
        # =====================================================================
        p_xTq = top.enter_context(tc.tile_pool(name="p_xTq", bufs=1))
        p_oT = top.enter_context(tc.tile_pool(name="p_oT", bufs=1))
        oT = [p_oT.tile([P, NSLOT, CW], BF, tag=f"oT{m}", name=f"oT{m}")
              for m in range(NPAIR)]
        p_x2t = top.enter_context(tc.tile_pool(name="p_x2t", bufs=1))
        x2T = [p_x2t.tile([P, NQ], BF, tag=f"x2t{kc}", name=f"x2t{kc}")
               for kc in range(KC)]
        att_ctx = ExitStack()
        p_qT = att_ctx.enter_context(tc.tile_pool(name="p_qT", bufs=1))
        xTq = []
        for kc in range(KC):
            tq = p_xTq.tile([P, NQ], BF, tag=f"xTq{kc}", name=f"xTq{kc}")
            nc.sync.dma_start(out=tq[:], in_=xTq_d[kc * P:(kc + 1) * P, :])
            xTq.append(tq)
        qT = []

        # =====================================================================
        # Phase 2: k and v paths — LN1 over the full sequence (Q projection is
        # issued right after LN1T so its matmuls overlap the LN1T apply ops)
        # =====================================================================
        p_kT = att_ctx.enter_context(tc.tile_pool(name="p_kT", bufs=1))
        p_v65 = att_ctx.enter_context(tc.tile_pool(name="p_v65", bufs=1))
        v65 = p_v65.tile([P, T // P, NPAIR, 2, 65], BF, tag="v65", name="v65")
        nc.vector.memset(v65[:, :, :, :, 64:65], 1.0)
        kT = []
        with ExitStack() as phk:
            p_xT = phk.enter_context(tc.tile_pool(name="p_xT", bufs=1))
            xT = []
            for kc in range(KC):
                t = p_xT.tile([P, T], BF, tag=f"xT{kc}", name=f"xT{kc}")
                nc.sync.dma_start(out=t[:], in_=xT_d[kc * P:(kc + 1) * P, :])
                xT.append(t)
            p_ln1q = phk.enter_context(tc.tile_pool(name="p_ln1q", bufs=1))
            ln1q_tiles = [p_ln1q.tile([P, NQ], BF, tag=f"ln1q{kc}",
                                      name=f"ln1q{kc}")
                          for kc in range(KC)]
            ln1q = layernorm_T(xTq, NQ, "g1", "be1", "ln1q",
                               out_tiles=ln1q_tiles)

            with ExitStack() as phqw:
                wqp = phqw.enter_context(tc.tile_pool(name="wqp", bufs=2))
                pA = phqw.enter_context(tc.tile_pool(name="pAq", bufs=4,
                                                     space="PSUM"))

                def _qproj(m):
                    wqm = wqp.tile([P, KC, P], BF, tag="wqm", name="wqm")
                    nc.sync.dma_start(
                        out=wqm[:], in_=wq_d[m].rearrange("p (k m) -> p k m", k=KC))
                    qt = p_qT.tile([P, NQ], BF, tag=f"qT{m}", name=f"qT{m}")
                    pss = [pA.tile([P, 512], F32, tag="proj", name="ps")
                           for _ in range(2)]
                    for kc in range(KC):
                        for nq in range(2):
                            MM(pss[nq][:], wqm[:, kc, :],
                               ln1q[kc][:, nq * 512:(nq + 1) * 512],
                               start=(kc == 0), stop=(kc == KC - 1), skip=True,
                               reuse=(nq > 0))
                    for nq in range(2):
                        nc.scalar.activation(qt[:, nq * 512:(nq + 1) * 512],
                                             pss[nq][:], AF.Copy)
                    qT.append(qt)

                # first half of Q fills the xT-DMA wait; second half fills
                # the LN1T apply window
                for m in range(4):
                    _qproj(m)
                ln1T = layernorm_T(xT, T, "g1", "be1", "ln1T")
                for m in range(4, NPAIR):
                    _qproj(m)

            with ExitStack() as phkw:
                wkp = phkw.enter_context(tc.tile_pool(name="wkp", bufs=2))
                pKV = phkw.enter_context(tc.tile_pool(name="pAkv", bufs=8,
                                                      space="PSUM"))
                pA = pKV
                for m in range(NPAIR):
                    wkm = wkp.tile([P, KC, P], BF, tag="wkm", name="wkm")
                    nc.sync.dma_start(
                        out=wkm[:], in_=wk_d[m].rearrange("p (k m) -> p k m", k=KC))
                    kt_t = p_kT.tile([P, T], BF, tag=f"kT{m}", name=f"kT{m}")
                    pss = [pA.tile([P, 512], F32, tag="proj", name="ps")
                           for _ in range(4)]
                    for kc in range(KC):
                        for n in range(4):
                            MM(pss[n][:], wkm[:, kc, :],
                               ln1T[kc][:, n * 512:(n + 1) * 512],
                               start=(kc == 0), stop=(kc == KC - 1), skip=True,
                               reuse=(n > 0))
                    for n in range(4):
                        nc.scalar.activation(kt_t[:, n * 512:(n + 1) * 512],
                                             pss[n][:], AF.Copy)
                    kT.append(kt_t)

                # v: token-major into resident v65 (ones col prefilled);
                # one stationary (ln1T chunk) feeds both output-dim halves.
                # Shares the K psum pool so the K->V handoff has no pool
                # close/reopen serialization.
                wvp = phkw.enter_context(tc.tile_pool(name="wvp", bufs=1))
                pA = pKV
                wvt = []
                for n in range(2):
                    wvn = wvp.tile([P, KC, 512], BF, tag=f"wvn{n}", name=f"wvn{n}")
                    nc.sync.dma_start(
                        out=wvn[:], in_=wv_d[n].rearrange("p (k d) -> p k d", k=KC))
                    wvt.append(wvn)
                for tt in range(T // P):
                    pss = [pA.tile([P, 512], F32, tag="proj", name="ps")
                           for _ in range(2)]
                    for kc in range(KC):
                        for n in range(2):
                            MM(pss[n][:], ln1T[kc][:, tt * P:(tt + 1) * P],
                               wvt[n][:, kc, :],
                               start=(kc == 0), stop=(kc == KC - 1), skip=True,
                               reuse=(n > 0))
                    for n in range(2):
                        nc.vector.tensor_copy(
                            v65[:, tt, 4 * n:4 * (n + 1), :, 0:64],
                            pss[n][:].rearrange("p (pr par d) -> p pr par d",
                                                pr=4, par=2))

        # =====================================================================
        # Phase 3: attention (ln1T freed; masks/avn fit above qT/kT/v65)
        # =====================================================================
        with ExitStack() as ph3:
            p_mask = ph3.enter_context(tc.tile_pool(name="p_mask", bufs=1))
            masks = {}
            for s in range(NSLOT):
                for j in range(8):
                    mt = p_mask.tile([P, CW], BF, tag=f"mask{s}_{j}",
                                     name=f"mask{s}_{j}")
                    nc.sync.dma_start(out=mt[:], in_=masks_d[s, j])
                    masks[(s, j)] = mt
            womp = ph3.enter_context(tc.tile_pool(name="womp", bufs=1))
            womt = []
            for m in range(NPAIR):
                w = womp.tile([P, KC, P], BF, tag=f"wom{m}", name=f"wom{m}")
                nc.sync.dma_start(
                    out=w[:], in_=wo_d[m].rearrange("p (k m) -> p k m", k=KC))
                womt.append(w)
            avn_pool = ph3.enter_context(tc.tile_pool(name="avn", bufs=2))
            sm_pool = ph3.enter_context(tc.tile_pool(name="sm", bufs=2))
            pt_pool = ph3.enter_context(tc.tile_pool(name="pt", bufs=4))

            psc_pool = ph3.enter_context(
                tc.tile_pool(name="psc", bufs=2, space="PSUM"))
            pav_pool = ph3.enter_context(
                tc.tile_pool(name="pav", bufs=2, space="PSUM"))
            for s in range(NSLOT):
                nkt = NKT_PROG[s]
                mask_base = 0 if s == 0 else 8
                qs = slice(s * CW, (s + 1) * CW)
                avn = {}
                avt = {}

                def _normalize(pair):
                    # evict, broadcast sums, 1/d = exp(-ln d), scale; issued
                    # one pair behind so its matmuls never wait on evictions
                    av = avt.pop(pair)
                    an = sm_pool.tile([65, 1024], BF, tag="an", name="an")
                    nc.vector.tensor_copy(an[:], av[0:65, :])
                    bc = psc_pool.tile([64, 1024], F32, tag="sc", name="bc")
                    MM(bc[:, 0:512], onesP[64:65, 0:64],
                       an[64:65, 0:512], start=True, stop=True, skip=True)
                    MM(bc[:, 512:1024], onesP[64:65, 0:64],
                       an[64:65, 512:1024], start=True, stop=True, skip=True,
                       reuse=True)
                    rec = sm_pool.tile([64, 1024], BF, tag="rec", name="rec")
                    lnd = sm_pool.tile([64, 1024], F32, tag="lnd", name="lnd")
                    nc.scalar.activation(lnd[:], bc[:], AF.Ln)
                    nc.scalar.activation(rec[:], lnd[:], AF.Exp, scale=-1.0)
                    anp = avn_pool.tile([P, CW], BF, tag=f"avn{pair}",
                                        name=f"avn{pair}")
                    nc.vector.tensor_tensor(anp[0:64, :], an[0:64, 0:512],
                                            rec[:, 0:512], ALU.mult)
                    tmo = sm_pool.tile([64, CW], BF, tag="tmo", name="tmo")
                    nc.vector.tensor_tensor(tmo[:], an[0:64, 512:1024],
                                            rec[:, 512:1024], ALU.mult)
                    nc.sync.dma_start(out=anp[64:128, :], in_=tmo[:])
                    avn[pair] = anp

                for pair in range(NPAIR):
                    av = pav_pool.tile([65, 1024], F32, tag="av", name="av")
                    avt[pair] = av
                    for kt in range(nkt):
                        kws = slice(kt * P, (kt + 1) * P)
                        psc = psc_pool.tile([P, 1024], F32, tag="sc",
                                            name="psc")
                        MM(psc[:, 0:512], kT[pair][0:64, kws],
                           qT[pair][0:64, qs], start=True, stop=True,
                           skip=True)
                        MM(psc[:, 512:1024], kT[pair][64:128, kws],
                           qT[pair][64:128, qs], start=True, stop=True,
                           skip=True)
                        pt = pt_pool.tile([P, 1024], BF, tag="pt", name="pt")
                        nc.scalar.activation(pt[:], psc[:], AF.Exp,
                                             scale=SCALE)
                        jm = kt - mask_base
                        if 0 <= jm < 8:
                            mt = masks[(s, jm)]
                            nc.vector.tensor_tensor(pt[:, 0:512], pt[:, 0:512],
                                                    mt[:], ALU.mult)
                            nc.vector.tensor_tensor(pt[:, 512:1024],
                                                    pt[:, 512:1024],
                                                    mt[:], ALU.mult)
                        st = (kt == 0)
                        sp = (kt == nkt - 1)
                        MM(av[0:65, 0:512], v65[:, kt, pair, 0, :],
                           pt[:, 0:512], start=st, stop=sp, skip=True)
                        MM(av[0:65, 512:1024], v65[:, kt, pair, 1, :],
                           pt[:, 512:1024], start=st, stop=sp, skip=True)
                    if pair > 0:
                        _normalize(pair - 1)
                _normalize(NPAIR - 1)
                # Wo for this slot (bias bo folded into eviction); psum
                # shares the score slot so both slots pipeline in 8 banks
                for m in range(NPAIR):
                    ps = psc_pool.tile([P, CW], F32, tag="sc", name="wops")
                    for k in range(NPAIR):
                        MM(ps[:], womt[m][:, k, :], avn[k][:],
                           start=(k == 0), stop=(k == NPAIR - 1), skip=True)
                    nc.vector.tensor_scalar(oT[m][:, s, :], ps[:],
                                            bot[:, m:m + 1], None, ALU.add)
                    if s == 1:
                        # residual add interleaves with the remaining Wo
                        # matmuls so LN2 stats can start sooner
                        nc.vector.tensor_tensor(
                            x2T[m][:], xTq[m][:],
                            oT[m][:].rearrange("p s w -> p (s w)"), ALU.add)

        att_ctx.close()

        # =====================================================================
        # Phase 4: x2 = oT + xTq ; LN2 (not in place)
        # =====================================================================
        p_ln2T = top.enter_context(tc.tile_pool(name="p_ln2T", bufs=1))
        ln2T_tiles = [p_ln2T.tile([P, NQ], BF, tag=f"ln2T{kc}", name=f"ln2T{kc}")
                      for kc in range(KC)]
        ln2T = layernorm_T(x2T, NQ, "g2", "be2", "ln2T", out_tiles=ln2T_tiles)

        # =====================================================================
        # Phase 5: FFN in two hidden-dim halves
        # =====================================================================
        with ExitStack() as ph5:
            ff1_pool = ph5.enter_context(tc.tile_pool(name="ff1", bufs=1))
            facc_pool = ph5.enter_context(tc.tile_pool(name="facc", bufs=1))
            w1_pool = ph5.enter_context(tc.tile_pool(name="w1s", bufs=2))
            w2_pool = ph5.enter_context(tc.tile_pool(name="w2s", bufs=2))
            fst_pool = ph5.enter_context(tc.tile_pool(name="fst", bufs=3))
            pF = ph5.enter_context(tc.tile_pool(name="pF", bufs=4, space="PSUM"))
            ffacc = [facc_pool.tile([P, NQ], BF, tag=f"facc{m}", name=f"ffacc{m}")
                     for m in range(KC)]
            for half in range(2):
                ff1 = []
                for m in range(16):
                    mm_i = half * 16 + m
                    w1m = w1_pool.tile([P, KC, P], BF, tag="w1m", name="w1m")
                    nc.sync.dma_start(
                        out=w1m[:],
                        in_=w1_d[mm_i].rearrange("p (k m) -> p k m", k=KC))
                    f = ff1_pool.tile([P, NQ], BF, tag=f"f{m}", name=f"f{m}")
                    pss = [pF.tile([P, 512], F32, tag="proj", name="ps")
                           for _ in range(2)]
                    for kc in range(KC):
                        for tch in range(2):
                            MM(pss[tch][:], w1m[:, kc, :],
                               ln2T[kc][:, tch * 512:(tch + 1) * 512],
                               start=(kc == 0), stop=(kc == KC - 1), skip=True,
                               reuse=(tch > 0))
                    for tch in range(2):
                        s = slice(tch * 512, (tch + 1) * 512)
                        # relu(x + b1) eviction
                        nc.vector.tensor_scalar(f[:, s], pss[tch][:],
                                                b1t[:, mm_i:mm_i + 1],
                                                0.0, ALU.add, ALU.max)
                    ff1.append(f)
                for mc in range(KC):
                    w2m = w2_pool.tile([P, 16, P], BF, tag="w2m", name="w2m")
                    nc.sync.dma_start(
                        out=w2m[:],
                        in_=w2_d[mc][:, half * 2048:(half + 1) * 2048]
                        .rearrange("p (k m) -> p k m", k=16))
                    pss = [pF.tile([P, 512], F32, tag="proj", name="ps")
                           for _ in range(2)]
                    for kt in range(16):
                        for tch in range(2):
                            MM(pss[tch][:], w2m[:, kt, :],
                               ff1[kt][:, tch * 512:(tch + 1) * 512],
                               start=(kt == 0), stop=(kt == 15), skip=True,
                               reuse=(tch > 0))
                    for tch in range(2):
                        s = slice(tch * 512, (tch + 1) * 512)
                        if half == 0:
                            nc.scalar.activation(ffacc[mc][:, s], pss[tch][:],
                                                 AF.Copy)
                        else:
                            o = fst_pool.tile([P, 512], F32, tag="fo", name="fo")
                            nc.vector.tensor_scalar(o[:], pss[tch][:],
                                                    b2t[:, mc:mc + 1],
                                                    None, ALU.add)
                            nc.vector.tensor_tensor(o[:], o[:], ffacc[mc][:, s],
                                                    ALU.add)
                            nc.vector.tensor_tensor(o[:], o[:], x2T[mc][:, s],
                                                    ALU.add)
                            nc.sync.dma_start(out=out_d[mc * P:(mc + 1) * P, s],
                                              in_=o[:])

    nc.compile()
    return nc, names


_CACHE = {}


def _get_built():
    if "nc" not in _CACHE:
        _CACHE["nc"], _CACHE["names"] = _build()
    return _CACHE["nc"], _CACHE["names"]


def _host_inputs(x, Wq, Wk, Wv, Wo, bo, ln1_g, ln1_b, ln2_g, ln2_b, W1, b1, W2, b2):
    """Build the 8 per-core input maps (host work = sharding/layout only)."""
    from ml_dtypes import bfloat16
    f = np.float32

    def wtile(W, nmb, nkc):
        # [mb, p, kc*P_or_512] with [mb,p,kc*w+j] = W[kc*P+p, mb*wout+j]
        kin, cout = W.shape
        wout = cout // nmb
        return np.ascontiguousarray(
            W.reshape(nkc, P, nmb, wout).transpose(2, 1, 0, 3)
            .reshape(nmb, P, nkc * wout).astype(bfloat16))

    shared = {
        "wq": wtile(np.asarray(Wq, f), NPAIR, KC),
        "wk": wtile(np.asarray(Wk, f), NPAIR, KC),
        "wv": wtile(np.asarray(Wv, f), 2, KC),
        "wo": wtile(np.asarray(Wo, f), NPAIR, KC),
        "w1": wtile(np.asarray(W1, f), 32, KC),
        "w2": wtile(np.asarray(W2, f), NPAIR, 32),
        "bot": np.ascontiguousarray(np.asarray(bo, f).reshape(KC, P).T),
        "b1t": np.ascontiguousarray(np.asarray(b1, f).reshape(32, P).T),
        "b2t": np.ascontiguousarray(np.asarray(b2, f).reshape(KC, P).T),
        "g1c": np.ascontiguousarray(np.asarray(ln1_g, f).reshape(KC, P).T),
        "be1c": np.ascontiguousarray(np.asarray(ln1_b, f).reshape(KC, P).T),
        "g2c": np.ascontiguousarray(np.asarray(ln2_g, f).reshape(KC, P).T),
        "be2c": np.ascontiguousarray(np.asarray(ln2_b, f).reshape(KC, P).T),
        "onesC": np.ones((P, 1), bfloat16),
        "onesP": np.ones((P, P), bfloat16),
    }
    kl = np.arange(P)[:, None]
    ql = np.arange(CW)[None, :]
    in_maps = []
    for c in range(8):
        b, r = c // 2, c % 2
        gs = GSETS[r]
        xTb = np.ascontiguousarray(np.asarray(x[b], f).T.astype(bfloat16))
        qcols = np.concatenate([np.arange(CW * g, CW * (g + 1)) for g in gs])
        xTq = np.ascontiguousarray(xTb[:, qcols])
        m = np.empty((NSLOT, 8, P, CW), bfloat16)
        for s in range(NSLOT):
            q0 = CW * gs[s]
            base = 0 if s == 0 else 8
            for j in range(8):
                kt = base + j
                m[s, j] = ((P * kt + kl) <= (q0 + ql)).astype(bfloat16)
        im = dict(shared)
        im["xT"] = xTb
        im["xTq"] = xTq
        im["masks"] = m
        in_maps.append(im)
    return in_maps


def _unshard(outs):
    out = np.empty((4, T, C), np.float32)
    for c in range(8):
        b, r = c // 2, c % 2
        oT = outs[c]  # (C, NQ)
        for s, g in enumerate(GSETS[r]):
            out[b, CW * g:CW * (g + 1), :] = oT[:, CW * s:CW * (s + 1)].T
    return out


def kernel(**inputs):
    from concourse.bass_utils import run_bass_kernel_spmd
    from concourse.bass_interp import get_hw_module

    args = {k: np.asarray(v, np.float32) for k, v in inputs.items()}
    in_maps_named = _host_inputs(**args)

    nc, names = _get_built()
    in_maps = [{names[k]: v for k, v in im.items()} for im in in_maps_named]

    hw = get_hw_module(nc.m)
    old = nc.m
    nc.m = hw
    try:
        res = run_bass_kernel_spmd(nc, in_maps, core_ids=list(range(8)))
    finally:
        nc.m = old
    outs = [r[names["out"]] for r in res.results]
    return _unshard(outs)


if __name__ == "__main__":
    import reference
    inp = {k: np.asarray(v) for k, v in reference.setup_inputs().items()}
    got = kernel(**inp)
    exp = np.asarray(reference.reference(**inp))
    err = np.abs(got - exp).max() / np.abs(exp).max()
    print("Relative error:", err)


# revision 26
# speedup vs baseline: 1.2165x; 1.0014x over previous
"""Trainium2 Bass kernel for a transformer MiniBlock (B=4, T=2048, C=1024, 16 heads,
causal attention, 4x FFN), sharded over 8 NeuronCores.

Sharding: core = (batch b=core//2, role r=core%2). Each core runs the full block for
1024 of its batch's 2048 tokens (two 512-token chunks, balanced for causal work:
role 0 owns chunks {0,3}, role 1 owns {1,2}), computing K/V over the full sequence
(no cross-core communication). The program is SPMD-uniform: k-window loop bounds are
per-slot maxima over roles; per-core causal masks (input data) zero the difference.

All tensors are bf16 on-chip (fp32 PSUM accumulation), which doubles effective
SBUF/DMA capacity, enables fast weight loads, and keeps rel-err ~3e-3. Activations
stay channel-major end to end; LN stats / softmax sums / broadcasts use small
ones-matmuls; the attention softmax is computed k-major with a ones-column appended
to V so denominators fall out of the AV matmul. Weights are pre-tiled on the host so
every weight DMA is fully contiguous. V stays resident in SBUF (no DRAM spill).
Even/odd head score matmuls are row-packed (tile_position) to run concurrently, and
exp is a single 1024-wide activation spanning two PSUM banks.
"""
import sys

sys.path.insert(0, "/opt/trn_rl_repo")

import numpy as np
from contextlib import ExitStack

import concourse.bacc as bacc
import concourse.mybir as mybir
import concourse.tile as tile
from concourse.tile import add_dep_helper

F32 = mybir.dt.float32
BF = mybir.dt.bfloat16
AF = mybir.ActivationFunctionType
ALU = mybir.AluOpType

P = 128
T = 2048          # full sequence
C = 1024          # embedding
NQ = 1024         # query tokens per core
H4 = 4096         # ffn hidden
NPAIR = 8         # head pairs (2 heads of 64 dims = 128 channels)
KC = C // P       # 8 channel tiles
NSLOT = 2         # 512-token query chunks per core
CW = 512          # chunk width
NKT_PROG = [8, 16]            # k-tiles per slot (program constant, max over roles)
GSETS = [[0, 3], [1, 2]]      # global 512-chunk index per slot, per role
LN_EPS = 1e-5
SCALE = float(64) ** -0.5     # head_size^-0.5 = 0.125


def _build():
    nc = bacc.Bacc(None, target_bir_lowering=False, debug=False)
    names = {}

    class _PE:
        """All matmuls go through here. A nosync dep chain pins PE issue
        order to program order, which makes ldweights=False (stationary
        reuse across consecutive matmuls) safe from scheduler interleaving."""
        prev = None

        @classmethod
        def mm(cls, out, stat, mov, start, stop, skip=False, reuse=False):
            inst = nc.tensor.matmul(out, stat, mov, start=start, stop=stop,
                                    skip_group_check=skip)
            # NOTE: walrus ignores ldweights=False (LDWEIGHTS is always
            # emitted per matmul) and a forced PE program-order dep chain
            # measured 40% slower on HW, so this helper is a plain matmul.
            return inst

    MM = _PE.mm
    with tile.TileContext(nc) as tc, ExitStack() as top:
        dram = top.enter_context(tc.tile_pool(name="io", bufs=1, space="DRAM"))

        def din(name, shape, dt=BF):
            t = dram.tile(shape, dt, kind="ExternalInput", name=f"i_{name}")
            names[name] = t.name
            return t

        xT_d = din("xT", [C, T])
        xTq_d = din("xTq", [C, NQ])
        wq_d = din("wq", [NPAIR, P, KC * P])
        wk_d = din("wk", [NPAIR, P, KC * P])
        wv_d = din("wv", [2, P, KC * 512])
        wo_d = din("wo", [NPAIR, P, KC * P])
        w1_d = din("w1", [32, P, KC * P])
        w2_d = din("w2", [NPAIR, P, 32 * P])
        masks_d = din("masks", [NSLOT, 8, P, CW])
        bot_d = din("bot", [P, KC], F32)
        b1t_d = din("b1t", [P, 32], F32)
        b2t_d = din("b2t", [P, KC], F32)
        g1_d = din("g1c", [P, KC], F32)
        be1_d = din("be1c", [P, KC], F32)
        g2_d = din("g2c", [P, KC], F32)
        be2_d = din("be2c", [P, KC], F32)
        onesC_d = din("onesC", [P, 1])
        onesP_d = din("onesP", [P, P])

        out_d = dram.tile([C, NQ], F32, kind="ExternalOutput", name="o_out")
        names["out"] = out_d.name

        # ---- persistent small sbuf ----
        pers = top.enter_context(tc.tile_pool(name="pers", bufs=1))
        onesC = pers.tile([P, 1], BF, tag="onesC")
        nc.sync.dma_start(out=onesC[:], in_=onesC_d[:])
        onesP = pers.tile([P, P], BF, tag="onesP")
        nc.sync.dma_start(out=onesP[:], in_=onesP_d[:])
        bot = pers.tile([P, KC], F32, tag="bot")
        nc.sync.dma_start(out=bot[:], in_=bot_d[:])
        b1t = pers.tile([P, 32], F32, tag="b1t")
        nc.sync.dma_start(out=b1t[:], in_=b1t_d[:])
        b2t = pers.tile([P, KC], F32, tag="b2t")
        nc.sync.dma_start(out=b2t[:], in_=b2t_d[:])
        lncol = {}
        for nm, d in [("g1", g1_d), ("be1", be1_d), ("g2", g2_d), ("be2", be2_d)]:
            t = pers.tile([P, KC], F32, tag=f"ln_{nm}", name=f"ln_{nm}")
            nc.sync.dma_start(out=t[:], in_=d[:])
            lncol[nm] = t
        zero_col = pers.tile([P, 1], F32, tag="zero_col")
        nc.vector.memset(zero_col[:], 0.0)
        eps_col = pers.tile([P, 1], F32, tag="eps_col")
        nc.vector.memset(eps_col[:], LN_EPS)

        # =====================================================================
        # transposed-layout layernorm, bf16 (in place unless out_tiles given)
        # =====================================================================
        def layernorm_T(xtiles, n, gname, bname, out_tag, out_tiles=None):
            ctx = ExitStack()
            work = ctx.enter_context(tc.tile_pool(name=f"lnw_{out_tag}", bufs=2))
            stat = ctx.enter_context(tc.tile_pool(name=f"lns_{out_tag}", bufs=1))
            pL = ctx.enter_context(tc.tile_pool(name=f"lnp_{out_tag}", bufs=1,
                                                space="PSUM"))
            pB = ctx.enter_context(tc.tile_pool(name=f"lnb_{out_tag}", bufs=1,
                                                space="PSUM"))
            nn = n // 512
            mu_row = stat.tile([1, n], BF, tag="mu_row")
            msq_row = stat.tile([1, n], BF, tag="msq_row")
            for i in range(nn):
                s = slice(i * 512, (i + 1) * 512)
                ps_sum = pL.tile([1, 512], F32, tag="lsum", name="ps_sum")
                ps_sq = pL.tile([1, 512], F32, tag="lsq", name="ps_sq")
                for kc in range(KC):
                    sq = work.tile([P, 512], BF, tag="sq", name="sq")
                    nc.vector.tensor_tensor(sq[:], xtiles[kc][:, s],
                                            xtiles[kc][:, s], ALU.mult)
                    MM(ps_sum[:], onesC[:], xtiles[kc][:, s],
                       start=(kc == 0), stop=(kc == KC - 1), skip=True,
                       reuse=not (i == 0 and kc == 0))
                    MM(ps_sq[:], onesC[:], sq[:],
                       start=(kc == 0), stop=(kc == KC - 1), skip=True,
                       reuse=True)
                nc.scalar.activation(mu_row[:, s], ps_sum[:], AF.Copy, scale=1.0 / C)
                nc.scalar.activation(msq_row[:, s], ps_sq[:], AF.Copy, scale=1.0 / C)
            mu_b = stat.tile([P, n], BF, tag="mu_b")
            rstd_b = stat.tile([P, n], BF, tag="rstd_b")
            for i in range(nn):
                s = slice(i * 512, (i + 1) * 512)
                psb = pB.tile([P, 1024], F32, tag="bc", name="psb")
                MM(psb[:, 0:512], onesP[0:1, :], mu_row[:, s],
                   start=True, stop=True, skip=True, reuse=(i != 0))
                MM(psb[:, 512:1024], onesP[0:1, :], msq_row[:, s],
                   start=True, stop=True, skip=True, reuse=True)
                nc.vector.tensor_copy(mu_b[:, s], psb[:, 0:512])
                mu2 = work.tile([P, 512], BF, tag="mu2", name="mu2")
                nc.vector.tensor_tensor(mu2[:], mu_b[:, s], mu_b[:, s], ALU.mult)
                var = work.tile([P, 512], F32, tag="var", name="var")
                nc.vector.tensor_tensor(var[:], psb[:, 512:1024], mu2[:],
                                        ALU.subtract)
                nc.scalar.activation(var[:], var[:], AF.Ln, bias=eps_col[:])
                nc.scalar.activation(rstd_b[:, s], var[:], AF.Exp, scale=-0.5)
            outs = []
            for kc in range(KC):
                o = out_tiles[kc] if out_tiles is not None else xtiles[kc]
                nc.vector.tensor_tensor(o[:], xtiles[kc][:], mu_b[:], ALU.subtract)
                nc.vector.tensor_tensor(o[:], o[:], rstd_b[:], ALU.mult)
                nc.vector.tensor_scalar(o[:], o[:], lncol[gname][:, kc:kc + 1],
                                        lncol[bname][:, kc:kc + 1], ALU.mult, ALU.add)
                outs.append(o)
            ctx.close()
            return outs

        # =====================================================================
        # Phase 1: q path — LN1 of the core's own query columns, project
        # =====================================================================
        p_xTq = top.enter_context(tc.tile_pool(name="p_xTq", bufs=1))
        p_oT = top.enter_context(tc.tile_pool(name="p_oT", bufs=1))
        oT = [p_oT.tile([P, NSLOT, CW], BF, tag=f"oT{m}", name=f"oT{m}")
              for m in range(NPAIR)]
        p_x2t = top.enter_context(tc.tile_pool(name="p_x2t", bufs=1))
        x2T = [p_x2t.tile([P, NQ], BF, tag=f"x2t{kc}", name=f"x2t{kc}")
               for kc in range(KC)]
        att_ctx = ExitStack()
        p_qT = att_ctx.enter_context(tc.tile_pool(name="p_qT", bufs=1))
        xTq = []
        for kc in range(KC):
            tq = p_xTq.tile([P, NQ], BF, tag=f"xTq{kc}", name=f"xTq{kc}")
            nc.sync.dma_start(out=tq[:], in_=xTq_d[kc * P:(kc + 1) * P, :])
            xTq.append(tq)
        qT = []

        # =====================================================================
        # Phase 2: k and v paths — LN1 over the full sequence (Q projection is
        # issued right after LN1T so its matmuls overlap the LN1T apply ops)
        # =====================================================================
        p_kT = att_ctx.enter_context(tc.tile_pool(name="p_kT", bufs=1))
        p_v65 = att_ctx.enter_context(tc.tile_pool(name="p_v65", bufs=1))
        v65 = p_v65.tile([P, T // P, NPAIR, 2, 65], BF, tag="v65", name="v65")
        nc.vector.memset(v65[:, :, :, :, 64:65], 1.0)
        kT = []
        with ExitStack() as phk:
            p_xT = phk.enter_context(tc.tile_pool(name="p_xT", bufs=1))
            xT = []
            for kc in range(KC):
                t = p_xT.tile([P, T], BF, tag=f"xT{kc}", name=f"xT{kc}")
                nc.sync.dma_start(out=t[:], in_=xT_d[kc * P:(kc + 1) * P, :])
                xT.append(t)
            p_ln1q = phk.enter_context(tc.tile_pool(name="p_ln1q", bufs=1))
            ln1q_tiles = [p_ln1q.tile([P, NQ], BF, tag=f"ln1q{kc}",
                                      name=f"ln1q{kc}")
                          for kc in range(KC)]
            ln1q = layernorm_T(xTq, NQ, "g1", "be1", "ln1q",
                               out_tiles=ln1q_tiles)

            with ExitStack() as phqw:
                wqp = phqw.enter_context(tc.tile_pool(name="wqp", bufs=2))
                pA = phqw.enter_context(tc.tile_pool(name="pAq", bufs=4,
                                                     space="PSUM"))

                def _qproj(m):
                    wqm = wqp.tile([P, KC, P], BF, tag="wqm", name="wqm")
                    nc.sync.dma_start(
                        out=wqm[:], in_=wq_d[m].rearrange("p (k m) -> p k m", k=KC))
                    qt = p_qT.tile([P, NQ], BF, tag=f"qT{m}", name=f"qT{m}")
                    pss = [pA.tile([P, 512], F32, tag="proj", name="ps")
                           for _ in range(2)]
                    for kc in range(KC):
                        for nq in range(2):
                            MM(pss[nq][:], wqm[:, kc, :],
                               ln1q[kc][:, nq * 512:(nq + 1) * 512],
                               start=(kc == 0), stop=(kc == KC - 1), skip=True,
                               reuse=(nq > 0))
                    for nq in range(2):
                        nc.scalar.activation(qt[:, nq * 512:(nq + 1) * 512],
                                             pss[nq][:], AF.Copy)
                    qT.append(qt)

                # first half of Q fills the xT-DMA wait; second half fills
                # the LN1T apply window
                for m in range(4):
                    _qproj(m)
                ln1T = layernorm_T(xT, T, "g1", "be1", "ln1T")
                for m in range(4, NPAIR):
                    _qproj(m)

            with ExitStack() as phkw:
                wkp = phkw.enter_context(tc.tile_pool(name="wkp", bufs=2))
                pKV = phkw.enter_context(tc.tile_pool(name="pAkv", bufs=8,
                                                      space="PSUM"))
                pA = pKV
                for m in range(NPAIR):
                    wkm = wkp.tile([P, KC, P], BF, tag="wkm", name="wkm")
                    nc.sync.dma_start(
                        out=wkm[:], in_=wk_d[m].rearrange("p (k m) -> p k m", k=KC))
                    kt_t = p_kT.tile([P, T], BF, tag=f"kT{m}", name=f"kT{m}")
                    pss = [pA.tile([P, 512], F32, tag="proj", name="ps")
                           for _ in range(4)]
                    for kc in range(KC):
                        for n in range(4):
                            MM(pss[n][:], wkm[:, kc, :],
                               ln1T[kc][:, n * 512:(n + 1) * 512],
                               start=(kc == 0), stop=(kc == KC - 1), skip=True,
                               reuse=(n > 0))
                    for n in range(4):
                        nc.scalar.activation(kt_t[:, n * 512:(n + 1) * 512],
                                             pss[n][:], AF.Copy)
                    kT.append(kt_t)

                # v: token-major into resident v65 (ones col prefilled);
                # one stationary (ln1T chunk) feeds both output-dim halves.
                # Shares the K psum pool so the K->V handoff has no pool
                # close/reopen serialization.
                wvp = phkw.enter_context(tc.tile_pool(name="wvp", bufs=1))
                pA = pKV
                wvt = []
                for n in range(2):
                    wvn = wvp.tile([P, KC, 512], BF, tag=f"wvn{n}", name=f"wvn{n}")
                    nc.sync.dma_start(
                        out=wvn[:], in_=wv_d[n].rearrange("p (k d) -> p k d", k=KC))
                    wvt.append(wvn)
                for tt in range(T // P):
                    pss = [pA.tile([P, 512], F32, tag="proj", name="ps")
                           for _ in range(2)]
                    for kc in range(KC):
                        for n in range(2):
                            MM(pss[n][:], ln1T[kc][:, tt * P:(tt + 1) * P],
                               wvt[n][:, kc, :],
                               start=(kc == 0), stop=(kc == KC - 1), skip=True,
                               reuse=(n > 0))
                    for n in range(2):
                        nc.vector.tensor_copy(
                            v65[:, tt, 4 * n:4 * (n + 1), :, 0:64],
                            pss[n][:].rearrange("p (pr par d) -> p pr par d",
                                                pr=4, par=2))

        # =====================================================================
        # Phase 3: attention (ln1T freed; masks/avn fit above qT/kT/v65)
        # =====================================================================
        with ExitStack() as ph3:
            p_mask = ph3.enter_context(tc.tile_pool(name="p_mask", bufs=1))
            masks = {}
            for s in range(NSLOT):
                for j in range(8):
                    mt = p_mask.tile([P, CW], BF, tag=f"mask{s}_{j}",
                                     name=f"mask{s}_{j}")
                    nc.sync.dma_start(out=mt[:], in_=masks_d[s, j])
                    masks[(s, j)] = mt
            womp = ph3.enter_context(tc.tile_pool(name="womp", bufs=1))
            womt = []
            for m in range(NPAIR):
                w = womp.tile([P, KC, P], BF, tag=f"wom{m}", name=f"wom{m}")
                nc.sync.dma_start(
                    out=w[:], in_=wo_d[m].rearrange("p (k m) -> p k m", k=KC))
                womt.append(w)
            avn_pool = ph3.enter_context(tc.tile_pool(name="avn", bufs=2))
            sm_pool = ph3.enter_context(tc.tile_pool(name="sm", bufs=2))
            pt_pool = ph3.enter_context(tc.tile_pool(name="pt", bufs=4))

            psc_pool = ph3.enter_context(
                tc.tile_pool(name="psc", bufs=3, space="PSUM"))
            pav_pool = ph3.enter_context(
                tc.tile_pool(name="pav", bufs=1, space="PSUM"))
            for s in range(NSLOT):
                nkt = NKT_PROG[s]
                mask_base = 0 if s == 0 else 8
                qs = slice(s * CW, (s + 1) * CW)
                avn = {}
                avt = {}

                def _normalize(pair):
                    # evict, broadcast sums, 1/d = exp(-ln d), scale; issued
                    # one pair behind so its matmuls never wait on evictions
                    av = avt.pop(pair)
                    an = sm_pool.tile([65, 1024], BF, tag="an", name="an")
                    nc.vector.tensor_copy(an[:], av[0:65, :])
                    bc = psc_pool.tile([64, 1024], F32, tag="sc", name="bc")
                    MM(bc[:, 0:512], onesP[64:65, 0:64],
                       an[64:65, 0:512], start=True, stop=True, skip=True)
                    MM(bc[:, 512:1024], onesP[64:65, 0:64],
                       an[64:65, 512:1024], start=True, stop=True, skip=True,
                       reuse=True)
                    rec = sm_pool.tile([64, 1024], BF, tag="rec", name="rec")
                    lnd = sm_pool.tile([64, 1024], F32, tag="lnd", name="lnd")
                    nc.scalar.activation(lnd[:], bc[:], AF.Ln)
                    nc.scalar.activation(rec[:], lnd[:], AF.Exp, scale=-1.0)
                    anp = avn_pool.tile([P, CW], BF, tag=f"avn{pair}",
                                        name=f"avn{pair}")
                    nc.vector.tensor_tensor(anp[0:64, :], an[0:64, 0:512],
                                            rec[:, 0:512], ALU.mult)
                    tmo = sm_pool.tile([64, CW], BF, tag="tmo", name="tmo")
                    nc.vector.tensor_tensor(tmo[:], an[0:64, 512:1024],
                                            rec[:, 512:1024], ALU.mult)
                    nc.sync.dma_start(out=anp[64:128, :], in_=tmo[:])
                    avn[pair] = anp

                for pair in range(NPAIR):
                    av = pav_pool.tile([65, 1024], F32, tag="av", name="av")
                    avt[pair] = av
                    for kt in range(nkt):
                        kws = slice(kt * P, (kt + 1) * P)
                        psc = psc_pool.tile([P, 1024], F32, tag="sc",
                                            name="psc")
                        MM(psc[:, 0:512], kT[pair][0:64, kws],
                           qT[pair][0:64, qs], start=True, stop=True,
                           skip=True)
                        MM(psc[:, 512:1024], kT[pair][64:128, kws],
                           qT[pair][64:128, qs], start=True, stop=True,
                           skip=True)
                        pt = pt_pool.tile([P, 1024], BF, tag="pt", name="pt")
                        nc.scalar.activation(pt[:], psc[:], AF.Exp,
                                             scale=SCALE)
                        jm = kt - mask_base
                        if 0 <= jm < 8:
                            mt = masks[(s, jm)]
                            nc.vector.tensor_tensor(pt[:, 0:512], pt[:, 0:512],
                                                    mt[:], ALU.mult)
                            nc.vector.tensor_tensor(pt[:, 512:1024],
                                                    pt[:, 512:1024],
                                                    mt[:], ALU.mult)
                        st = (kt == 0)
                        sp = (kt == nkt - 1)
                        MM(av[0:65, 0:512], v65[:, kt, pair, 0, :],
                           pt[:, 0:512], start=st, stop=sp, skip=True)
                        MM(av[0:65, 512:1024], v65[:, kt, pair, 1, :],
                           pt[:, 512:1024], start=st, stop=sp, skip=True)
                    if pair > 0:
                        _normalize(pair - 1)
                _normalize(NPAIR - 1)
                # Wo for this slot (bias bo folded into eviction); psum
                # shares the score slot so both slots pipeline in 8 banks
                for m in range(NPAIR):
                    ps = psc_pool.tile([P, CW], F32, tag="sc", name="wops")
                    for k in range(NPAIR):
                        MM(ps[:], womt[m][:, k, :], avn[k][:],
                           start=(k == 0), stop=(k == NPAIR - 1), skip=True)
                    nc.vector.tensor_scalar(oT[m][:, s, :], ps[:],
                                            bot[:, m:m + 1], None, ALU.add)
                    if s == 1:
                        # residual add interleaves with the remaining Wo
                        # matmuls so LN2 stats can start sooner
                        nc.vector.tensor_tensor(
                            x2T[m][:], xTq[m][:],
                            oT[m][:].rearrange("p s w -> p (s w)"), ALU.add)

        att_ctx.close()

        # =====================================================================
        # Phase 4: x2 = oT + xTq ; LN2 (not in place)
        # =====================================================================
        p_ln2T = top.enter_context(tc.tile_pool(name="p_ln2T", bufs=1))
        ln2T_tiles = [p_ln2T.tile([P, NQ], BF, tag=f"ln2T{kc}", name=f"ln2T{kc}")
                      for kc in range(KC)]
        ln2T = layernorm_T(x2T, NQ, "g2", "be2", "ln2T", out_tiles=ln2T_tiles)

        # =====================================================================
        # Phase 5: FFN in two hidden-dim halves
        # =====================================================================
        with ExitStack() as ph5:
            ff1_pool = ph5.enter_context(tc.tile_pool(name="ff1", bufs=1))
            facc_pool = ph5.enter_context(tc.tile_pool(name="facc", bufs=1))
            w1_pool = ph5.enter_context(tc.tile_pool(name="w1s", bufs=2))
            w2_pool = ph5.enter_context(tc.tile_pool(name="w2s", bufs=2))
            fst_pool = ph5.enter_context(tc.tile_pool(name="fst", bufs=3))
            pF = ph5.enter_context(tc.tile_pool(name="pF", bufs=4, space="PSUM"))
            ffacc = [facc_pool.tile([P, NQ], BF, tag=f"facc{m}", name=f"ffacc{m}")
                     for m in range(KC)]
            for half in range(2):
                ff1 = []
                for m in range(16):
                    mm_i = half * 16 + m
                    w1m = w1_pool.tile([P, KC, P], BF, tag="w1m", name="w1m")
                    nc.sync.dma_start(
                        out=w1m[:],
                        in_=w1_d[mm_i].rearrange("p (k m) -> p k m", k=KC))
                    f = ff1_pool.tile([P, NQ], BF, tag=f"f{m}", name=f"f{m}")
                    pss = [pF.tile([P, 512], F32, tag="proj", name="ps")
                           for _ in range(2)]
                    for kc in range(KC):
                        for tch in range(2):
                            MM(pss[tch][:], w1m[:, kc, :],
                               ln2T[kc][:, tch * 512:(tch + 1) * 512],
                               start=(kc == 0), stop=(kc == KC - 1), skip=True,
                               reuse=(tch > 0))
                    for tch in range(2):
                        s = slice(tch * 512, (tch + 1) * 512)
                        # relu(x + b1) eviction
                        nc.vector.tensor_scalar(f[:, s], pss[tch][:],
                                                b1t[:, mm_i:mm_i + 1],
                                                0.0, ALU.add, ALU.max)
                    ff1.append(f)
                for mc in range(KC):
                    w2m = w2_pool.tile([P, 16, P], BF, tag="w2m", name="w2m")
                    nc.sync.dma_start(
                        out=w2m[:],
                        in_=w2_d[mc][:, half * 2048:(half + 1) * 2048]
                        .rearrange("p (k m) -> p k m", k=16))
                    pss = [pF.tile([P, 512], F32, tag="proj", name="ps")
                           for _ in range(2)]
                    for kt in range(16):
                        for tch in range(2):
                            MM(pss[tch][:], w2m[:, kt, :],
                               ff1[kt][:, tch * 512:(tch + 1) * 512],
                               start=(kt == 0), stop=(kt == 15), skip=True,
                               reuse=(tch > 0))
                    for tch in range(2):
                        s = slice(tch * 512, (tch + 1) * 512)
                        if half == 0:
                            nc.scalar.activation(ffacc[mc][:, s], pss[tch][:],
                                                 AF.Copy)
                        else:
                            o = fst_pool.tile([P, 512], F32, tag="fo", name="fo")
                            nc.vector.tensor_scalar(o[:], pss[tch][:],
                                                    b2t[:, mc:mc + 1],
                                                    None, ALU.add)
                            nc.vector.tensor_tensor(o[:], o[:], ffacc[mc][:, s],
                                                    ALU.add)
                            nc.vector.tensor_tensor(o[:], o[:], x2T[mc][:, s],
                                                    ALU.add)
                            nc.sync.dma_start(out=out_d[mc * P:(mc + 1) * P, s],
                                              in_=o[:])

    nc.compile()
    return nc, names


_CACHE = {}


def _get_built():
    if "nc" not in _CACHE:
        _CACHE["nc"], _CACHE["names"] = _build()
    return _CACHE["nc"], _CACHE["names"]


def _host_inputs(x, Wq, Wk, Wv, Wo, bo, ln1_g, ln1_b, ln2_g, ln2_b, W1, b1, W2, b2):
    """Build the 8 per-core input maps (host work = sharding/layout only)."""
    from ml_dtypes import bfloat16
    f = np.float32

    def wtile(W, nmb, nkc):
        # [mb, p, kc*P_or_512] with [mb,p,kc*w+j] = W[kc*P+p, mb*wout+j]
        kin, cout = W.shape
        wout = cout // nmb
        return np.ascontiguousarray(
            W.reshape(nkc, P, nmb, wout).transpose(2, 1, 0, 3)
            .reshape(nmb, P, nkc * wout).astype(bfloat16))

    shared = {
        "wq": wtile(np.asarray(Wq, f), NPAIR, KC),
        "wk": wtile(np.asarray(Wk, f), NPAIR, KC),
        "wv": wtile(np.asarray(Wv, f), 2, KC),
        "wo": wtile(np.asarray(Wo, f), NPAIR, KC),
        "w1": wtile(np.asarray(W1, f), 32, KC),
        "w2": wtile(np.asarray(W2, f), NPAIR, 32),
        "bot": np.ascontiguousarray(np.asarray(bo, f).reshape(KC, P).T),
        "b1t": np.ascontiguousarray(np.asarray(b1, f).reshape(32, P).T),
        "b2t": np.ascontiguousarray(np.asarray(b2, f).reshape(KC, P).T),
        "g1c": np.ascontiguousarray(np.asarray(ln1_g, f).reshape(KC, P).T),
        "be1c": np.ascontiguousarray(np.asarray(ln1_b, f).reshape(KC, P).T),
        "g2c": np.ascontiguousarray(np.asarray(ln2_g, f).reshape(KC, P).T),
        "be2c": np.ascontiguousarray(np.asarray(ln2_b, f).reshape(KC, P).T),
        "onesC": np.ones((P, 1), bfloat16),
        "onesP": np.ones((P, P), bfloat16),
    }
    kl = np.arange(P)[:, None]
    ql = np.arange(CW)[None, :]
    in_maps = []
    for c in range(8):
        b, r = c // 2, c % 2
        gs = GSETS[r]
        xTb = np.ascontiguousarray(np.asarray(x[b], f).T.astype(bfloat16))
        qcols = np.concatenate([np.arange(CW * g, CW * (g + 1)) for g in gs])
        xTq = np.ascontiguousarray(xTb[:, qcols])
        m = np.empty((NSLOT, 8, P, CW), bfloat16)
        for s in range(NSLOT):
            q0 = CW * gs[s]
            base = 0 if s == 0 else 8
            for j in range(8):
                kt = base + j
                m[s, j] = ((P * kt + kl) <= (q0 + ql)).astype(bfloat16)
        im = dict(shared)
        im["xT"] = xTb
        im["xTq"] = xTq
        im["masks"] = m
        in_maps.append(im)
    return in_maps


def _unshard(outs):
    out = np.empty((4, T, C), np.float32)
    for c in range(8):
        b, r = c // 2, c % 2
        oT = outs[c]  # (C, NQ)
        for s, g in enumerate(GSETS[r]):
            out[b, CW * g:CW * (g + 1), :] = oT[:, CW * s:CW * (s + 1)].T
    return out


def kernel(**inputs):
    from concourse.bass_utils import run_bass_kernel_spmd
    from concourse.bass_interp import get_hw_module

    args = {k: np.asarray(v, np.float32) for k, v in inputs.items()}
    in_maps_named = _host_inputs(**args)

    nc, names = _get_built()
    in_maps = [{names[k]: v for k, v in im.items()} for im in in_maps_named]

    hw = get_hw_module(nc.m)
    old = nc.m
    nc.m = hw
    try:
        res = run_bass_kernel_spmd(nc, in_maps, core_ids=list(range(8)))
    finally:
        nc.m = old
    outs = [r[names["out"]] for r in res.results]
    return _unshard(outs)


if __name__ == "__main__":
    import reference
    inp = {k: np.asarray(v) for k, v in reference.setup_inputs().items()}
    got = kernel(**inp)
    exp = np.asarray(reference.reference(**inp))
    err = np.abs(got - exp).max() / np.abs(exp).max()
    print("Relative error:", err)


# revision 27
# speedup vs baseline: 1.2429x; 1.0217x over previous
"""Trainium2 Bass kernel for a transformer MiniBlock (B=4, T=2048, C=1024, 16 heads,
causal attention, 4x FFN), sharded over 8 NeuronCores.

Sharding: core = (batch b=core//2, role r=core%2). Each core runs the full block for
1024 of its batch's 2048 tokens (two 512-token chunks, balanced for causal work:
role 0 owns chunks {0,3}, role 1 owns {1,2}), computing K/V over the full sequence
(no cross-core communication). The program is SPMD-uniform: k-window loop bounds are
per-slot maxima over roles; per-core causal masks (input data) zero the difference.

All tensors are bf16 on-chip (fp32 PSUM accumulation), which doubles effective
SBUF/DMA capacity, enables fast weight loads, and keeps rel-err ~3e-3. Activations
stay channel-major end to end; LN stats / softmax sums / broadcasts use small
ones-matmuls; the attention softmax is computed k-major with a ones-column appended
to V so denominators fall out of the AV matmul. Weights are pre-tiled on the host so
every weight DMA is fully contiguous. V stays resident in SBUF (no DRAM spill).
Even/odd head score matmuls are row-packed (tile_position) to run concurrently, and
exp is a single 1024-wide activation spanning two PSUM banks.
"""
import sys

sys.path.insert(0, "/opt/trn_rl_repo")

import numpy as np
from contextlib import ExitStack

import concourse.bacc as bacc
import concourse.mybir as mybir
import concourse.tile as tile
from concourse.tile import add_dep_helper

F32 = mybir.dt.float32
BF = mybir.dt.bfloat16
AF = mybir.ActivationFunctionType
ALU = mybir.AluOpType

P = 128
T = 2048          # full sequence
C = 1024          # embedding
NQ = 1024         # query tokens per core
H4 = 4096         # ffn hidden
NPAIR = 8         # head pairs (2 heads of 64 dims = 128 channels)
KC = C // P       # 8 channel tiles
NSLOT = 2         # 512-token query chunks per core
CW = 512          # chunk width
NKT_PROG = [8, 16]            # k-tiles per slot (program constant, max over roles)
GSETS = [[0, 3], [1, 2]]      # global 512-chunk index per slot, per role
LN_EPS = 1e-5
SCALE = float(64) ** -0.5     # head_size^-0.5 = 0.125


def _build():
    nc = bacc.Bacc(None, target_bir_lowering=False, debug=False)
    names = {}

    class _PE:
        """All matmuls go through here. A nosync dep chain pins PE issue
        order to program order, which makes ldweights=False (stationary
        reuse across consecutive matmuls) safe from scheduler interleaving."""
        prev = None

        @classmethod
        def mm(cls, out, stat, mov, start, stop, skip=False, reuse=False):
            inst = nc.tensor.matmul(out, stat, mov, start=start, stop=stop,
                                    skip_group_check=skip)
            # NOTE: walrus ignores ldweights=False (LDWEIGHTS is always
            # emitted per matmul) and a forced PE program-order dep chain
            # measured 40% slower on HW, so this helper is a plain matmul.
            return inst

    MM = _PE.mm
    with tile.TileContext(nc) as tc, ExitStack() as top:
        dram = top.enter_context(tc.tile_pool(name="io", bufs=1, space="DRAM"))

        def din(name, shape, dt=BF):
            t = dram.tile(shape, dt, kind="ExternalInput", name=f"i_{name}")
            names[name] = t.name
            return t

        xT_d = din("xT", [C, T])
        xTq_d = din("xTq", [C, NQ])
        wq_d = din("wq", [NPAIR, P, KC * P])
        wk_d = din("wk", [NPAIR, P, KC * P])
        wv_d = din("wv", [2, P, KC * 512])
        wo_d = din("wo", [NPAIR, P, KC * P])
        w1_d = din("w1", [32, P, KC * P])
        w2_d = din("w2", [NPAIR, P, 32 * P])
        masks_d = din("masks", [NSLOT, 8, P, CW])
        bot_d = din("bot", [P, KC], F32)
        b1t_d = din("b1t", [P, 32], F32)
        b2t_d = din("b2t", [P, KC], F32)
        g1_d = din("g1c", [P, KC], F32)
        be1_d = din("be1c", [P, KC], F32)
        g2_d = din("g2c", [P, KC], F32)
        be2_d = din("be2c", [P, KC], F32)
        onesC_d = din("onesC", [P, 1])
        onesP_d = din("onesP", [P, P])

        out_d = dram.tile([C, NQ], F32, kind="ExternalOutput", name="o_out")
        names["out"] = out_d.name

        # ---- persistent small sbuf ----
        pers = top.enter_context(tc.tile_pool(name="pers", bufs=1))
        onesC = pers.tile([P, 1], BF, tag="onesC")
        nc.sync.dma_start(out=onesC[:], in_=onesC_d[:])
        onesP = pers.tile([P, P], BF, tag="onesP")
        nc.sync.dma_start(out=onesP[:], in_=onesP_d[:])
        bot = pers.tile([P, KC], F32, tag="bot")
        nc.sync.dma_start(out=bot[:], in_=bot_d[:])
        b1t = pers.tile([P, 32], F32, tag="b1t")
        nc.sync.dma_start(out=b1t[:], in_=b1t_d[:])
        b2t = pers.tile([P, KC], F32, tag="b2t")
        nc.sync.dma_start(out=b2t[:], in_=b2t_d[:])
        lncol = {}
        for nm, d in [("g1", g1_d), ("be1", be1_d), ("g2", g2_d), ("be2", be2_d)]:
            t = pers.tile([P, KC], F32, tag=f"ln_{nm}", name=f"ln_{nm}")
            nc.sync.dma_start(out=t[:], in_=d[:])
            lncol[nm] = t
        zero_col = pers.tile([P, 1], F32, tag="zero_col")
        nc.vector.memset(zero_col[:], 0.0)
        eps_col = pers.tile([P, 1], F32, tag="eps_col")
        nc.vector.memset(eps_col[:], LN_EPS)

        # =====================================================================
        # transposed-layout layernorm, bf16 (in place unless out_tiles given)
        # =====================================================================
        def layernorm_T(xtiles, n, gname, bname, out_tag, out_tiles=None):
            ctx = ExitStack()
            work = ctx.enter_context(tc.tile_pool(name=f"lnw_{out_tag}", bufs=2))
            stat = ctx.enter_context(tc.tile_pool(name=f"lns_{out_tag}", bufs=1))
            pL = ctx.enter_context(tc.tile_pool(name=f"lnp_{out_tag}", bufs=1,
                                                space="PSUM"))
            pB = ctx.enter_context(tc.tile_pool(name=f"lnb_{out_tag}", bufs=1,
                                                space="PSUM"))
            nn = n // 512
            mu_row = stat.tile([1, n], BF, tag="mu_row")
            msq_row = stat.tile([1, n], BF, tag="msq_row")
            for i in range(nn):
                s = slice(i * 512, (i + 1) * 512)
                ps_sum = pL.tile([1, 512], F32, tag="lsum", name="ps_sum")
                ps_sq = pL.tile([1, 512], F32, tag="lsq", name="ps_sq")
                for kc in range(KC):
                    sq = work.tile([P, 512], BF, tag="sq", name="sq")
                    nc.vector.tensor_tensor(sq[:], xtiles[kc][:, s],
                                            xtiles[kc][:, s], ALU.mult)
                    MM(ps_sum[:], onesC[:], xtiles[kc][:, s],
                       start=(kc == 0), stop=(kc == KC - 1), skip=True,
                       reuse=not (i == 0 and kc == 0))
                    MM(ps_sq[:], onesC[:], sq[:],
                       start=(kc == 0), stop=(kc == KC - 1), skip=True,
                       reuse=True)
                nc.scalar.activation(mu_row[:, s], ps_sum[:], AF.Copy, scale=1.0 / C)
                nc.scalar.activation(msq_row[:, s], ps_sq[:], AF.Copy, scale=1.0 / C)
            mu_b = stat.tile([P, n], BF, tag="mu_b")
            rstd_b = stat.tile([P, n], BF, tag="rstd_b")
            for i in range(nn):
                s = slice(i * 512, (i + 1) * 512)
                psb = pB.tile([P, 1024], F32, tag="bc", name="psb")
                MM(psb[:, 0:512], onesP[0:1, :], mu_row[:, s],
                   start=True, stop=True, skip=True, reuse=(i != 0))
                MM(psb[:, 512:1024], onesP[0:1, :], msq_row[:, s],
                   start=True, stop=True, skip=True, reuse=True)
                nc.vector.tensor_copy(mu_b[:, s], psb[:, 0:512])
                mu2 = work.tile([P, 512], BF, tag="mu2", name="mu2")
                nc.vector.tensor_tensor(mu2[:], mu_b[:, s], mu_b[:, s], ALU.mult)
                var = work.tile([P, 512], F32, tag="var", name="var")
                nc.vector.tensor_tensor(var[:], psb[:, 512:1024], mu2[:],
                                        ALU.subtract)
                nc.scalar.activation(var[:], var[:], AF.Ln, bias=eps_col[:])
                nc.scalar.activation(rstd_b[:, s], var[:], AF.Exp, scale=-0.5)
            outs = []
            for kc in range(KC):
                o = out_tiles[kc] if out_tiles is not None else xtiles[kc]
                nc.vector.tensor_tensor(o[:], xtiles[kc][:], mu_b[:], ALU.subtract)
                nc.vector.tensor_tensor(o[:], o[:], rstd_b[:], ALU.mult)
                nc.vector.tensor_scalar(o[:], o[:], lncol[gname][:, kc:kc + 1],
                                        lncol[bname][:, kc:kc + 1], ALU.mult, ALU.add)
                outs.append(o)
            ctx.close()
            return outs

        # =====================================================================
        # Phase 1: q path — LN1 of the core's own query columns, project
        # =====================================================================
        p_xTq = top.enter_context(tc.tile_pool(name="p_xTq", bufs=1))
        p_oT = top.enter_context(tc.tile_pool(name="p_oT", bufs=1))
        oT = [p_oT.tile([P, NSLOT, CW], BF, tag=f"oT{m}", name=f"oT{m}")
              for m in range(NPAIR)]
        p_x2t = top.enter_context(tc.tile_pool(name="p_x2t", bufs=1))
        x2T = [p_x2t.tile([P, NQ], BF, tag=f"x2t{kc}", name=f"x2t{kc}")
               for kc in range(KC)]
        att_ctx = ExitStack()
        p_qT = att_ctx.enter_context(tc.tile_pool(name="p_qT", bufs=1))
        xTq = []
        for kc in range(KC):
            tq = p_xTq.tile([P, NQ], BF, tag=f"xTq{kc}", name=f"xTq{kc}")
            nc.sync.dma_start(out=tq[:], in_=xTq_d[kc * P:(kc + 1) * P, :])
            xTq.append(tq)
        qT = []

        # =====================================================================
        # Phase 2: k and v paths — LN1 over the full sequence (Q projection is
        # issued right after LN1T so its matmuls overlap the LN1T apply ops)
        # =====================================================================
        p_kT = att_ctx.enter_context(tc.tile_pool(name="p_kT", bufs=1))
        p_v65 = att_ctx.enter_context(tc.tile_pool(name="p_v65", bufs=1))
        v65 = p_v65.tile([P, T // P, NPAIR, 2, 65], BF, tag="v65", name="v65")
        nc.vector.memset(v65[:, :, :, :, 64:65], 1.0)
        kT = []
        with ExitStack() as phk:
            p_xT = phk.enter_context(tc.tile_pool(name="p_xT", bufs=1))
            xT = []
            for kc in range(KC):
                t = p_xT.tile([P, T], BF, tag=f"xT{kc}", name=f"xT{kc}")
                nc.sync.dma_start(out=t[:], in_=xT_d[kc * P:(kc + 1) * P, :])
                xT.append(t)
            p_ln1q = phk.enter_context(tc.tile_pool(name="p_ln1q", bufs=1))
            ln1q_tiles = [p_ln1q.tile([P, NQ], BF, tag=f"ln1q{kc}",
                                      name=f"ln1q{kc}")
                          for kc in range(KC)]
            ln1q = layernorm_T(xTq, NQ, "g1", "be1", "ln1q",
                               out_tiles=ln1q_tiles)

            with ExitStack() as phqw:
                wqp = phqw.enter_context(tc.tile_pool(name="wqp", bufs=2))
                pA = phqw.enter_context(tc.tile_pool(name="pAq", bufs=4,
                                                     space="PSUM"))

                def _qproj(m):
                    wqm = wqp.tile([P, KC, P], BF, tag="wqm", name="wqm")
                    nc.sync.dma_start(
                        out=wqm[:], in_=wq_d[m].rearrange("p (k m) -> p k m", k=KC))
                    qt = p_qT.tile([P, NQ], BF, tag=f"qT{m}", name=f"qT{m}")
                    pss = [pA.tile([P, 512], F32, tag="proj", name="ps")
                           for _ in range(2)]
                    for kc in range(KC):
                        for nq in range(2):
                            MM(pss[nq][:], wqm[:, kc, :],
                               ln1q[kc][:, nq * 512:(nq + 1) * 512],
                               start=(kc == 0), stop=(kc == KC - 1), skip=True,
                               reuse=(nq > 0))
                    for nq in range(2):
                        nc.scalar.activation(qt[:, nq * 512:(nq + 1) * 512],
                                             pss[nq][:], AF.Copy)
                    qT.append(qt)

                # first half of Q fills the xT-DMA wait; second half fills
                # the LN1T apply window
                for m in range(4):
                    _qproj(m)
                ln1T = layernorm_T(xT, T, "g1", "be1", "ln1T")
                for m in range(4, NPAIR):
                    _qproj(m)

            with ExitStack() as phkw:
                wkp = phkw.enter_context(tc.tile_pool(name="wkp", bufs=2))
                pKV = phkw.enter_context(tc.tile_pool(name="pAkv", bufs=8,
                                                      space="PSUM"))
                pA = pKV
                for m in range(NPAIR):
                    wkm = wkp.tile([P, KC, P], BF, tag="wkm", name="wkm")
                    nc.sync.dma_start(
                        out=wkm[:], in_=wk_d[m].rearrange("p (k m) -> p k m", k=KC))
                    kt_t = p_kT.tile([P, T], BF, tag=f"kT{m}", name=f"kT{m}")
                    pss = [pA.tile([P, 512], F32, tag="proj", name="ps")
                           for _ in range(4)]
                    for kc in range(KC):
                        for n in range(4):
                            MM(pss[n][:], wkm[:, kc, :],
                               ln1T[kc][:, n * 512:(n + 1) * 512],
                               start=(kc == 0), stop=(kc == KC - 1), skip=True,
                               reuse=(n > 0))
                    for n in range(4):
                        s = slice(n * 512, (n + 1) * 512)
                        if n % 2 == 0:
                            nc.scalar.activation(kt_t[:, s], pss[n][:], AF.Copy)
                        else:
                            nc.vector.tensor_copy(kt_t[:, s], pss[n][:])
                    kT.append(kt_t)

                # v: token-major into resident v65 (ones col prefilled);
                # one stationary (ln1T chunk) feeds both output-dim halves.
                # Shares the K psum pool so the K->V handoff has no pool
                # close/reopen serialization.
                wvp = phkw.enter_context(tc.tile_pool(name="wvp", bufs=1))
                pA = pKV
                wvt = []
                for n in range(2):
                    wvn = wvp.tile([P, KC, 512], BF, tag=f"wvn{n}", name=f"wvn{n}")
                    nc.sync.dma_start(
                        out=wvn[:], in_=wv_d[n].rearrange("p (k d) -> p k d", k=KC))
                    wvt.append(wvn)
                for tt in range(T // P):
                    pss = [pA.tile([P, 512], F32, tag="proj", name="ps")
                           for _ in range(2)]
                    for kc in range(KC):
                        for n in range(2):
                            MM(pss[n][:], ln1T[kc][:, tt * P:(tt + 1) * P],
                               wvt[n][:, kc, :],
                               start=(kc == 0), stop=(kc == KC - 1), skip=True,
                               reuse=(n > 0))
                    for n in range(2):
                        nc.vector.tensor_copy(
                            v65[:, tt, 4 * n:4 * (n + 1), :, 0:64],
                            pss[n][:].rearrange("p (pr par d) -> p pr par d",
                                                pr=4, par=2))

        # =====================================================================
        # Phase 3: attention (ln1T freed; masks/avn fit above qT/kT/v65)
        # =====================================================================
        with ExitStack() as ph3:
            p_mask = ph3.enter_context(tc.tile_pool(name="p_mask", bufs=1))
            masks = {}
            for s in range(NSLOT):
                for j in range(8):
                    mt = p_mask.tile([P, CW], BF, tag=f"mask{s}_{j}",
                                     name=f"mask{s}_{j}")
                    nc.sync.dma_start(out=mt[:], in_=masks_d[s, j])
                    masks[(s, j)] = mt
            womp = ph3.enter_context(tc.tile_pool(name="womp", bufs=1))
            womt = []
            for m in range(NPAIR):
                w = womp.tile([P, KC, P], BF, tag=f"wom{m}", name=f"wom{m}")
                nc.sync.dma_start(
                    out=w[:], in_=wo_d[m].rearrange("p (k m) -> p k m", k=KC))
                womt.append(w)
            avn_pool = ph3.enter_context(tc.tile_pool(name="avn", bufs=2))
            sm_pool = ph3.enter_context(tc.tile_pool(name="sm", bufs=2))
            pt_pool = ph3.enter_context(tc.tile_pool(name="pt", bufs=6))

            psc_pool = ph3.enter_context(
                tc.tile_pool(name="psc", bufs=3, space="PSUM"))
            pav_pool = ph3.enter_context(
                tc.tile_pool(name="pav", bufs=1, space="PSUM"))
            for s in range(NSLOT):
                nkt = NKT_PROG[s]
                mask_base = 0 if s == 0 else 8
                qs = slice(s * CW, (s + 1) * CW)
                avn = {}
                avt = {}

                def _normalize(pair):
                    # evict, broadcast sums, 1/d = exp(-ln d), scale; issued
                    # one pair behind so its matmuls never wait on evictions
                    av = avt.pop(pair)
                    an = sm_pool.tile([65, 1024], BF, tag="an", name="an")
                    nc.vector.tensor_copy(an[:], av[0:65, :])
                    bc = psc_pool.tile([64, 1024], F32, tag="sc", name="bc")
                    MM(bc[:, 0:512], onesP[64:65, 0:64],
                       an[64:65, 0:512], start=True, stop=True, skip=True)
                    MM(bc[:, 512:1024], onesP[64:65, 0:64],
                       an[64:65, 512:1024], start=True, stop=True, skip=True,
                       reuse=True)
                    rec = sm_pool.tile([64, 1024], BF, tag="rec", name="rec")
                    lnd = sm_pool.tile([64, 1024], F32, tag="lnd", name="lnd")
                    nc.scalar.activation(lnd[:], bc[:], AF.Ln)
                    nc.scalar.activation(rec[:], lnd[:], AF.Exp, scale=-1.0)
                    anp = avn_pool.tile([P, CW], BF, tag=f"avn{pair}",
                                        name=f"avn{pair}")
                    nc.vector.tensor_tensor(anp[0:64, :], an[0:64, 0:512],
                                            rec[:, 0:512], ALU.mult)
                    tmo = sm_pool.tile([64, CW], BF, tag="tmo", name="tmo")
                    nc.vector.tensor_tensor(tmo[:], an[0:64, 512:1024],
                                            rec[:, 512:1024], ALU.mult)
                    nc.sync.dma_start(out=anp[64:128, :], in_=tmo[:])
                    avn[pair] = anp

                for pair in range(NPAIR):
                    av = pav_pool.tile([65, 1024], F32, tag="av", name="av")
                    avt[pair] = av
                    for kt in range(nkt):
                        kws = slice(kt * P, (kt + 1) * P)
                        psc = psc_pool.tile([P, 1024], F32, tag="sc",
                                            name="psc")
                        MM(psc[:, 0:512], kT[pair][0:64, kws],
                           qT[pair][0:64, qs], start=True, stop=True,
                           skip=True)
                        MM(psc[:, 512:1024], kT[pair][64:128, kws],
                           qT[pair][64:128, qs], start=True, stop=True,
                           skip=True)
                        pt = pt_pool.tile([P, 1024], BF, tag="pt", name="pt")
                        nc.scalar.activation(pt[:], psc[:], AF.Exp,
                                             scale=SCALE)
                        jm = kt - mask_base
                        if 0 <= jm < 8:
                            mt = masks[(s, jm)]
                            nc.vector.tensor_tensor(pt[:, 0:512], pt[:, 0:512],
                                                    mt[:], ALU.mult)
                            nc.vector.tensor_tensor(pt[:, 512:1024],
                                                    pt[:, 512:1024],
                                                    mt[:], ALU.mult)
                        st = (kt == 0)
                        sp = (kt == nkt - 1)
                        MM(av[0:65, 0:512], v65[:, kt, pair, 0, :],
                           pt[:, 0:512], start=st, stop=sp, skip=True)
                        MM(av[0:65, 512:1024], v65[:, kt, pair, 1, :],
                           pt[:, 512:1024], start=st, stop=sp, skip=True)
                    if pair > 0:
                        _normalize(pair - 1)
                _normalize(NPAIR - 1)
                # Wo for this slot (bias bo folded into eviction); psum
                # shares the score slot so both slots pipeline in 8 banks
                for m in range(NPAIR):
                    ps = psc_pool.tile([P, CW], F32, tag="sc", name="wops")
                    for k in range(NPAIR):
                        MM(ps[:], womt[m][:, k, :], avn[k][:],
                           start=(k == 0), stop=(k == NPAIR - 1), skip=True)
                    nc.vector.tensor_scalar(oT[m][:, s, :], ps[:],
                                            bot[:, m:m + 1], None, ALU.add)
                    if s == 1:
                        # residual add interleaves with the remaining Wo
                        # matmuls so LN2 stats can start sooner
                        nc.vector.tensor_tensor(
                            x2T[m][:], xTq[m][:],
                            oT[m][:].rearrange("p s w -> p (s w)"), ALU.add)

        att_ctx.close()

        # =====================================================================
        # Phase 4: x2 = oT + xTq ; LN2 (not in place)
        # =====================================================================
        p_ln2T = top.enter_context(tc.tile_pool(name="p_ln2T", bufs=1))
        ln2T_tiles = [p_ln2T.tile([P, NQ], BF, tag=f"ln2T{kc}", name=f"ln2T{kc}")
                      for kc in range(KC)]
        ln2T = layernorm_T(x2T, NQ, "g2", "be2", "ln2T", out_tiles=ln2T_tiles)

        # =====================================================================
        # Phase 5: FFN in two hidden-dim halves
        # =====================================================================
        with ExitStack() as ph5:
            ff1_pool = ph5.enter_context(tc.tile_pool(name="ff1", bufs=1))
            facc_pool = ph5.enter_context(tc.tile_pool(name="facc", bufs=1))
            w1_pool = ph5.enter_context(tc.tile_pool(name="w1s", bufs=2))
            w2_pool = ph5.enter_context(tc.tile_pool(name="w2s", bufs=2))
            fst_pool = ph5.enter_context(tc.tile_pool(name="fst", bufs=3))
            pF = ph5.enter_context(tc.tile_pool(name="pF", bufs=4, space="PSUM"))
            ffacc = [facc_pool.tile([P, NQ], BF, tag=f"facc{m}", name=f"ffacc{m}")
                     for m in range(KC)]
            for half in range(2):
                ff1 = []
                for m in range(16):
                    mm_i = half * 16 + m
                    w1m = w1_pool.tile([P, KC, P], BF, tag="w1m", name="w1m")
                    nc.sync.dma_start(
                        out=w1m[:],
                        in_=w1_d[mm_i].rearrange("p (k m) -> p k m", k=KC))
                    f = ff1_pool.tile([P, NQ], BF, tag=f"f{m}", name=f"f{m}")
                    pss = [pF.tile([P, 512], F32, tag="proj", name="ps")
                           for _ in range(2)]
                    for kc in range(KC):
                        for tch in range(2):
                            MM(pss[tch][:], w1m[:, kc, :],
                               ln2T[kc][:, tch * 512:(tch + 1) * 512],
                               start=(kc == 0), stop=(kc == KC - 1), skip=True,
                               reuse=(tch > 0))
                    for tch in range(2):
                        s = slice(tch * 512, (tch + 1) * 512)
                        # relu(x + b1) eviction
                        nc.vector.tensor_scalar(f[:, s], pss[tch][:],
                                                b1t[:, mm_i:mm_i + 1],
                                                0.0, ALU.add, ALU.max)
                    ff1.append(f)
                for mc in range(KC):
                    w2m = w2_pool.tile([P, 16, P], BF, tag="w2m", name="w2m")
                    nc.sync.dma_start(
                        out=w2m[:],
                        in_=w2_d[mc][:, half * 2048:(half + 1) * 2048]
                        .rearrange("p (k m) -> p k m", k=16))
                    pss = [pF.tile([P, 512], F32, tag="proj", name="ps")
                           for _ in range(2)]
                    for kt in range(16):
                        for tch in range(2):
                            MM(pss[tch][:], w2m[:, kt, :],
                               ff1[kt][:, tch * 512:(tch + 1) * 512],
                               start=(kt == 0), stop=(kt == 15), skip=True,
                               reuse=(tch > 0))
                    for tch in range(2):
                        s = slice(tch * 512, (tch + 1) * 512)
                        if half == 0:
                            nc.scalar.activation(ffacc[mc][:, s], pss[tch][:],
                                                 AF.Copy)
                        else:
                            o = fst_pool.tile([P, 512], F32, tag="fo", name="fo")
                            nc.vector.tensor_scalar(o[:], pss[tch][:],
                                                    b2t[:, mc:mc + 1],
                                                    None, ALU.add)
                            nc.vector.tensor_tensor(o[:], o[:], ffacc[mc][:, s],
                                                    ALU.add)
                            nc.vector.tensor_tensor(o[:], o[:], x2T[mc][:, s],
                                                    ALU.add)
                            nc.sync.dma_start(out=out_d[mc * P:(mc + 1) * P, s],
                                              in_=o[:])

    nc.compile()
    return nc, names


_CACHE = {}


def _get_built():
    if "nc" not in _CACHE:
        _CACHE["nc"], _CACHE["names"] = _build()
    return _CACHE["nc"], _CACHE["names"]


def _host_inputs(x, Wq, Wk, Wv, Wo, bo, ln1_g, ln1_b, ln2_g, ln2_b, W1, b1, W2, b2):
    """Build the 8 per-core input maps (host work = sharding/layout only)."""
    from ml_dtypes import bfloat16
    f = np.float32

    def wtile(W, nmb, nkc):
        # [mb, p, kc*P_or_512] with [mb,p,kc*w+j] = W[kc*P+p, mb*wout+j]
        kin, cout = W.shape
        wout = cout // nmb
        return np.ascontiguousarray(
            W.reshape(nkc, P, nmb, wout).transpose(2, 1, 0, 3)
            .reshape(nmb, P, nkc * wout).astype(bfloat16))

    shared = {
        "wq": wtile(np.asarray(Wq, f), NPAIR, KC),
        "wk": wtile(np.asarray(Wk, f), NPAIR, KC),
        "wv": wtile(np.asarray(Wv, f), 2, KC),
        "wo": wtile(np.asarray(Wo, f), NPAIR, KC),
        "w1": wtile(np.asarray(W1, f), 32, KC),
        "w2": wtile(np.asarray(W2, f), NPAIR, 32),
        "bot": np.ascontiguousarray(np.asarray(bo, f).reshape(KC, P).T),
        "b1t": np.ascontiguousarray(np.asarray(b1, f).reshape(32, P).T),
        "b2t": np.ascontiguousarray(np.asarray(b2, f).reshape(KC, P).T),
        "g1c": np.ascontiguousarray(np.asarray(ln1_g, f).reshape(KC, P).T),
        "be1c": np.ascontiguousarray(np.asarray(ln1_b, f).reshape(KC, P).T),
        "g2c": np.ascontiguousarray(np.asarray(ln2_g, f).reshape(KC, P).T),
        "be2c": np.ascontiguousarray(np.asarray(ln2_b, f).reshape(KC, P).T),
        "onesC": np.ones((P, 1), bfloat16),
        "onesP": np.ones((P, P), bfloat16),
    }
    kl = np.arange(P)[:, None]
    ql = np.arange(CW)[None, :]
    in_maps = []
    for c in range(8):
        b, r = c // 2, c % 2
        gs = GSETS[r]
        xTb = np.ascontiguousarray(np.asarray(x[b], f).T.astype(bfloat16))
        qcols = np.concatenate([np.arange(CW * g, CW * (g + 1)) for g in gs])
        xTq = np.ascontiguousarray(xTb[:, qcols])
        m = np.empty((NSLOT, 8, P, CW), bfloat16)
        for s in range(NSLOT):
            q0 = CW * gs[s]
            base = 0 if s == 0 else 8
            for j in range(8):
                kt = base + j
                m[s, j] = ((P * kt + kl) <= (q0 + ql)).astype(bfloat16)
        im = dict(shared)
        im["xT"] = xTb
        im["xTq"] = xTq
        im["masks"] = m
        in_maps.append(im)
    return in_maps


def _unshard(outs):
    out = np.empty((4, T, C), np.float32)
    for c in range(8):
        b, r = c // 2, c % 2
        oT = outs[c]  # (C, NQ)
        for s, g in enumerate(GSETS[r]):
            out[b, CW * g:CW * (g + 1), :] = oT[:, CW * s:CW * (s + 1)].T
    return out


def kernel(**inputs):
    from concourse.bass_utils import run_bass_kernel_spmd
    from concourse.bass_interp import get_hw_module

    args = {k: np.asarray(v, np.float32) for k, v in inputs.items()}
    in_maps_named = _host_inputs(**args)

    nc, names = _get_built()
    in_maps = [{names[k]: v for k, v in im.items()} for im in in_maps_named]

    hw = get_hw_module(nc.m)
    old = nc.m
    nc.m = hw
    try:
        res = run_bass_kernel_spmd(nc, in_maps, core_ids=list(range(8)))
    finally:
        nc.m = old
    outs = [r[names["out"]] for r in res.results]
    return _unshard(outs)


if __name__ == "__main__":
    import reference
    inp = {k: np.asarray(v) for k, v in reference.setup_inputs().items()}
    got = kernel(**inp)
    exp = np.asarray(reference.reference(**inp))
    err = np.abs(got - exp).max() / np.abs(exp).max()
    print("Relative error:", err)


# revision 31
# speedup vs baseline: 1.2469x; 1.0032x over previous
"""Trainium2 Bass kernel for a transformer MiniBlock (B=4, T=2048, C=1024, 16 heads,
causal attention, 4x FFN), sharded over 8 NeuronCores.

Sharding: core = (batch b=core//2, role r=core%2). Each core runs the full block for
1024 of its batch's 2048 tokens (two 512-token chunks, balanced for causal work:
role 0 owns chunks {0,3}, role 1 owns {1,2}), computing K/V over the full sequence
(no cross-core communication). The program is SPMD-uniform: k-window loop bounds are
per-slot maxima over roles; per-core causal masks (input data) zero the difference.

All tensors are bf16 on-chip (fp32 PSUM accumulation), which doubles effective
SBUF/DMA capacity, enables fast weight loads, and keeps rel-err ~3e-3. Activations
stay channel-major end to end; LN stats / softmax sums / broadcasts use small
ones-matmuls; the attention softmax is computed k-major with a ones-column appended
to V so denominators fall out of the AV matmul. Weights are pre-tiled on the host so
every weight DMA is fully contiguous. V stays resident in SBUF (no DRAM spill).
Even/odd head score matmuls are row-packed (tile_position) to run concurrently, and
exp is a single 1024-wide activation spanning two PSUM banks.
"""
import sys

sys.path.insert(0, "/opt/trn_rl_repo")

import numpy as np
from contextlib import ExitStack

import concourse.bacc as bacc
import concourse.mybir as mybir
import concourse.tile as tile
from concourse.tile import add_dep_helper

F32 = mybir.dt.float32
BF = mybir.dt.bfloat16
AF = mybir.ActivationFunctionType
ALU = mybir.AluOpType

P = 128
T = 2048          # full sequence
C = 1024          # embedding
NQ = 1024         # query tokens per core
H4 = 4096         # ffn hidden
NPAIR = 8         # head pairs (2 heads of 64 dims = 128 channels)
KC = C // P       # 8 channel tiles
NSLOT = 2         # 512-token query chunks per core
CW = 512          # chunk width
NKT_PROG = [8, 16]            # k-tiles per slot (program constant, max over roles)
GSETS = [[0, 3], [1, 2]]      # global 512-chunk index per slot, per role
LN_EPS = 1e-5
SCALE = float(64) ** -0.5     # head_size^-0.5 = 0.125


def _build():
    nc = bacc.Bacc(None, target_bir_lowering=False, debug=False)
    names = {}

    class _PE:
        """All matmuls go through here. A nosync dep chain pins PE issue
        order to program order, which makes ldweights=False (stationary
        reuse across consecutive matmuls) safe from scheduler interleaving."""
        prev = None

        @classmethod
        def mm(cls, out, stat, mov, start, stop, skip=False, reuse=False):
            inst = nc.tensor.matmul(out, stat, mov, start=start, stop=stop,
                                    skip_group_check=skip)
            # NOTE: walrus ignores ldweights=False (LDWEIGHTS is always
            # emitted per matmul) and a forced PE program-order dep chain
            # measured 40% slower on HW, so this helper is a plain matmul.
            return inst

    MM = _PE.mm
    with tile.TileContext(nc) as tc, ExitStack() as top:
        dram = top.enter_context(tc.tile_pool(name="io", bufs=1, space="DRAM"))

        def din(name, shape, dt=BF):
            t = dram.tile(shape, dt, kind="ExternalInput", name=f"i_{name}")
            names[name] = t.name
            return t

        xT_d = din("xT", [C, T])
        xTq_d = din("xTq", [C, NQ])
        wq_d = din("wq", [NPAIR, P, KC * P])
        wk_d = din("wk", [NPAIR, P, KC * P])
        wv_d = din("wv", [2, P, KC * 512])
        wo_d = din("wo", [NPAIR, P, KC * P])
        w1_d = din("w1", [32, P, KC * P])
        w2_d = din("w2", [NPAIR, P, 32 * P])
        masks_d = din("masks", [NSLOT, 8, P, CW])
        bot_d = din("bot", [P, KC], F32)
        b1t_d = din("b1t", [P, 32], F32)
        b2t_d = din("b2t", [P, KC], F32)
        g1_d = din("g1c", [P, KC], F32)
        be1_d = din("be1c", [P, KC], F32)
        g2_d = din("g2c", [P, KC], F32)
        be2_d = din("be2c", [P, KC], F32)
        onesC_d = din("onesC", [P, 1])
        onesP_d = din("onesP", [P, P])

        out_d = dram.tile([C, NQ], F32, kind="ExternalOutput", name="o_out")
        names["out"] = out_d.name

        # ---- persistent small sbuf ----
        pers = top.enter_context(tc.tile_pool(name="pers", bufs=1))
        onesC = pers.tile([P, 1], BF, tag="onesC")
        nc.sync.dma_start(out=onesC[:], in_=onesC_d[:])
        onesP = pers.tile([P, P], BF, tag="onesP")
        nc.sync.dma_start(out=onesP[:], in_=onesP_d[:])
        bot = pers.tile([P, KC], F32, tag="bot")
        nc.sync.dma_start(out=bot[:], in_=bot_d[:])
        b1t = pers.tile([P, 32], F32, tag="b1t")
        nc.sync.dma_start(out=b1t[:], in_=b1t_d[:])
        b2t = pers.tile([P, KC], F32, tag="b2t")
        nc.sync.dma_start(out=b2t[:], in_=b2t_d[:])
        lncol = {}
        for nm, d in [("g1", g1_d), ("be1", be1_d), ("g2", g2_d), ("be2", be2_d)]:
            t = pers.tile([P, KC], F32, tag=f"ln_{nm}", name=f"ln_{nm}")
            nc.sync.dma_start(out=t[:], in_=d[:])
            lncol[nm] = t
        zero_col = pers.tile([P, 1], F32, tag="zero_col")
        nc.vector.memset(zero_col[:], 0.0)
        eps_col = pers.tile([P, 1], F32, tag="eps_col")
        nc.vector.memset(eps_col[:], LN_EPS)

        # =====================================================================
        # transposed-layout layernorm, bf16 (in place unless out_tiles given)
        # =====================================================================
        def layernorm_T(xtiles, n, gname, bname, out_tag, out_tiles=None):
            ctx = ExitStack()
            work = ctx.enter_context(tc.tile_pool(name=f"lnw_{out_tag}", bufs=2))
            stat = ctx.enter_context(tc.tile_pool(name=f"lns_{out_tag}", bufs=1))
            pL = ctx.enter_context(tc.tile_pool(name=f"lnp_{out_tag}", bufs=1,
                                                space="PSUM"))
            pB = ctx.enter_context(tc.tile_pool(name=f"lnb_{out_tag}", bufs=1,
                                                space="PSUM"))
            nn = n // 512
            mu_row = stat.tile([1, n], BF, tag="mu_row")
            msq_row = stat.tile([1, n], BF, tag="msq_row")
            for i in range(nn):
                s = slice(i * 512, (i + 1) * 512)
                ps_sum = pL.tile([1, 512], F32, tag="lsum", name="ps_sum")
                ps_sq = pL.tile([1, 512], F32, tag="lsq", name="ps_sq")
                for kc in range(KC):
                    sq = work.tile([P, 512], BF, tag="sq", name="sq")
                    nc.vector.tensor_tensor(sq[:], xtiles[kc][:, s],
                                            xtiles[kc][:, s], ALU.mult)
                    MM(ps_sum[:], onesC[:], xtiles[kc][:, s],
                       start=(kc == 0), stop=(kc == KC - 1), skip=True,
                       reuse=not (i == 0 and kc == 0))
                    MM(ps_sq[:], onesC[:], sq[:],
                       start=(kc == 0), stop=(kc == KC - 1), skip=True,
                       reuse=True)
                nc.scalar.activation(mu_row[:, s], ps_sum[:], AF.Copy, scale=1.0 / C)
                nc.scalar.activation(msq_row[:, s], ps_sq[:], AF.Copy, scale=1.0 / C)
            mu_b = stat.tile([P, n], BF, tag="mu_b")
            rstd_b = stat.tile([P, n], BF, tag="rstd_b")
            for i in range(nn):
                s = slice(i * 512, (i + 1) * 512)
                psb = pB.tile([P, 1024], F32, tag="bc", name="psb")
                MM(psb[:, 0:512], onesP[0:1, :], mu_row[:, s],
                   start=True, stop=True, skip=True, reuse=(i != 0))
                MM(psb[:, 512:1024], onesP[0:1, :], msq_row[:, s],
                   start=True, stop=True, skip=True, reuse=True)
                nc.vector.tensor_copy(mu_b[:, s], psb[:, 0:512])
                mu2 = work.tile([P, 512], BF, tag="mu2", name="mu2")
                nc.vector.tensor_tensor(mu2[:], mu_b[:, s], mu_b[:, s], ALU.mult)
                var = work.tile([P, 512], F32, tag="var", name="var")
                nc.vector.tensor_tensor(var[:], psb[:, 512:1024], mu2[:],
                                        ALU.subtract)
                nc.scalar.activation(var[:], var[:], AF.Ln, bias=eps_col[:])
                nc.scalar.activation(rstd_b[:, s], var[:], AF.Exp, scale=-0.5)
            outs = []
            for kc in range(KC):
                o = out_tiles[kc] if out_tiles is not None else xtiles[kc]
                nc.vector.tensor_tensor(o[:], xtiles[kc][:], mu_b[:], ALU.subtract)
                nc.vector.tensor_tensor(o[:], o[:], rstd_b[:], ALU.mult)
                nc.vector.tensor_scalar(o[:], o[:], lncol[gname][:, kc:kc + 1],
                                        lncol[bname][:, kc:kc + 1], ALU.mult, ALU.add)
                outs.append(o)
            ctx.close()
            return outs

        # =====================================================================
        # Phase 1: q path — LN1 of the core's own query columns, project
        # =====================================================================
        p_xTq = top.enter_context(tc.tile_pool(name="p_xTq", bufs=1))
        p_oT = top.enter_context(tc.tile_pool(name="p_oT", bufs=1))
        oT = [p_oT.tile([P, NSLOT, CW], BF, tag=f"oT{m}", name=f"oT{m}")
              for m in range(NPAIR)]
        p_x2t = top.enter_context(tc.tile_pool(name="p_x2t", bufs=1))
        x2T = [p_x2t.tile([P, NQ], BF, tag=f"x2t{kc}", name=f"x2t{kc}")
               for kc in range(KC)]
        att_ctx = ExitStack()
        p_qT = att_ctx.enter_context(tc.tile_pool(name="p_qT", bufs=1))
        xTq = []
        for kc in range(KC):
            tq = p_xTq.tile([P, NQ], BF, tag=f"xTq{kc}", name=f"xTq{kc}")
            nc.sync.dma_start(out=tq[:], in_=xTq_d[kc * P:(kc + 1) * P, :])
            xTq.append(tq)
        qT = []

        # =====================================================================
        # Phase 2: k and v paths — LN1 over the full sequence (Q projection is
        # issued right after LN1T so its matmuls overlap the LN1T apply ops)
        # =====================================================================
        p_kT = att_ctx.enter_context(tc.tile_pool(name="p_kT", bufs=1))
        p_v65 = att_ctx.enter_context(tc.tile_pool(name="p_v65", bufs=1))
        v65 = p_v65.tile([P, T // P, NPAIR, 2, 65], BF, tag="v65", name="v65")
        nc.vector.memset(v65[:, :, :, :, 64:65], 1.0)
        kT = []
        with ExitStack() as phk:
            p_xT = phk.enter_context(tc.tile_pool(name="p_xT", bufs=1))
            xT = []
            for kc in range(KC):
                t = p_xT.tile([P, T], BF, tag=f"xT{kc}", name=f"xT{kc}")
                nc.sync.dma_start(out=t[:], in_=xT_d[kc * P:(kc + 1) * P, :])
                xT.append(t)
            p_ln1q = phk.enter_context(tc.tile_pool(name="p_ln1q", bufs=1))
            ln1q_tiles = [p_ln1q.tile([P, NQ], BF, tag=f"ln1q{kc}",
                                      name=f"ln1q{kc}")
                          for kc in range(KC)]
            ln1q = layernorm_T(xTq, NQ, "g1", "be1", "ln1q",
                               out_tiles=ln1q_tiles)

            with ExitStack() as phqw:
                wqp = phqw.enter_context(tc.tile_pool(name="wqp", bufs=2))
                pA = phqw.enter_context(tc.tile_pool(name="pAq", bufs=4,
                                                     space="PSUM"))

                def _qproj(m):
                    wqm = wqp.tile([P, KC, P], BF, tag="wqm", name="wqm")
                    nc.sync.dma_start(
                        out=wqm[:], in_=wq_d[m].rearrange("p (k m) -> p k m", k=KC))
                    qt = p_qT.tile([P, NQ], BF, tag=f"qT{m}", name=f"qT{m}")
                    pss = [pA.tile([P, 512], F32, tag="proj", name="ps")
                           for _ in range(2)]
                    for kc in range(KC):
                        for nq in range(2):
                            MM(pss[nq][:], wqm[:, kc, :],
                               ln1q[kc][:, nq * 512:(nq + 1) * 512],
                               start=(kc == 0), stop=(kc == KC - 1), skip=True,
                               reuse=(nq > 0))
                    for nq in range(2):
                        nc.scalar.activation(qt[:, nq * 512:(nq + 1) * 512],
                                             pss[nq][:], AF.Copy)
                    qT.append(qt)

                # first half of Q fills the xT-DMA wait; second half fills
                # the LN1T apply window
                for m in range(4):
                    _qproj(m)
                ln1T = layernorm_T(xT, T, "g1", "be1", "ln1T")
                for m in range(4, NPAIR):
                    _qproj(m)

            with ExitStack() as phkw:
                wkp = phkw.enter_context(tc.tile_pool(name="wkp", bufs=3))
                pKV = phkw.enter_context(tc.tile_pool(name="pAkv", bufs=8,
                                                      space="PSUM"))
                pA = pKV
                for m in range(NPAIR):
                    wkm = wkp.tile([P, KC, P], BF, tag="wkm", name="wkm")
                    nc.sync.dma_start(
                        out=wkm[:], in_=wk_d[m].rearrange("p (k m) -> p k m", k=KC))
                    kt_t = p_kT.tile([P, T], BF, tag=f"kT{m}", name=f"kT{m}")
                    pss = [pA.tile([P, 512], F32, tag="proj", name="ps")
                           for _ in range(4)]
                    for kc in range(KC):
                        for n in range(4):
                            MM(pss[n][:], wkm[:, kc, :],
                               ln1T[kc][:, n * 512:(n + 1) * 512],
                               start=(kc == 0), stop=(kc == KC - 1), skip=True,
                               reuse=(n > 0))
                    for n in range(4):
                        s = slice(n * 512, (n + 1) * 512)
                        if n % 2 == 0:
                            nc.scalar.activation(kt_t[:, s], pss[n][:], AF.Copy)
                        else:
                            nc.vector.tensor_copy(kt_t[:, s], pss[n][:])
                    kT.append(kt_t)

                # v: token-major into resident v65 (ones col prefilled);
                # one stationary (ln1T chunk) feeds both output-dim halves.
                # Shares the K psum pool so the K->V handoff has no pool
                # close/reopen serialization.
                wvp = phkw.enter_context(tc.tile_pool(name="wvp", bufs=1))
                pA = pKV
                wvt = []
                for n in range(2):
                    wvn = wvp.tile([P, KC, 512], BF, tag=f"wvn{n}", name=f"wvn{n}")
                    nc.sync.dma_start(
                        out=wvn[:], in_=wv_d[n].rearrange("p (k d) -> p k d", k=KC))
                    wvt.append(wvn)
                for tt in range(T // P):
                    pss = [pA.tile([P, 512], F32, tag="proj", name="ps")
                           for _ in range(2)]
                    for kc in range(KC):
                        for n in range(2):
                            MM(pss[n][:], ln1T[kc][:, tt * P:(tt + 1) * P],
                               wvt[n][:, kc, :],
                               start=(kc == 0), stop=(kc == KC - 1), skip=True,
                               reuse=(n > 0))
                    for n in range(2):
                        nc.vector.tensor_copy(
                            v65[:, tt, 4 * n:4 * (n + 1), :, 0:64],
                            pss[n][:].rearrange("p (pr par d) -> p pr par d",
                                                pr=4, par=2))

        # =====================================================================
        # Phase 3: attention (ln1T freed; masks/avn fit above qT/kT/v65)
        # =====================================================================
        with ExitStack() as ph3:
            p_mask = ph3.enter_context(tc.tile_pool(name="p_mask", bufs=1))
            masks = {}
            for s in range(NSLOT):
                for j in range(8):
                    mt = p_mask.tile([P, CW], BF, tag=f"mask{s}_{j}",
                                     name=f"mask{s}_{j}")
                    nc.sync.dma_start(out=mt[:], in_=masks_d[s, j])
                    masks[(s, j)] = mt
            womp = ph3.enter_context(tc.tile_pool(name="womp", bufs=1))
            womt = []
            for m in range(NPAIR):
                w = womp.tile([P, KC, P], BF, tag=f"wom{m}", name=f"wom{m}")
                nc.sync.dma_start(
                    out=w[:], in_=wo_d[m].rearrange("p (k m) -> p k m", k=KC))
                womt.append(w)
            avn_pool = ph3.enter_context(tc.tile_pool(name="avn", bufs=2))
            sm_pool = ph3.enter_context(tc.tile_pool(name="sm", bufs=2))
            pt_pool = ph3.enter_context(tc.tile_pool(name="pt", bufs=6))

            psc_pool = ph3.enter_context(
                tc.tile_pool(name="psc", bufs=3, space="PSUM"))
            pav_pool = ph3.enter_context(
                tc.tile_pool(name="pav", bufs=1, space="PSUM"))
            for s in range(NSLOT):
                nkt = NKT_PROG[s]
                mask_base = 0 if s == 0 else 8
                qs = slice(s * CW, (s + 1) * CW)
                avn = {}
                avt = {}

                def _normalize(pair):
                    # evict, broadcast sums, 1/d = exp(-ln d), scale; issued
                    # one pair behind so its matmuls never wait on evictions
                    av = avt.pop(pair)
                    an = sm_pool.tile([65, 1024], BF, tag="an", name="an")
                    nc.vector.tensor_copy(an[:], av[0:65, :])
                    bc = psc_pool.tile([64, 1024], F32, tag="sc", name="bc")
                    MM(bc[:, 0:512], onesP[64:65, 0:64],
                       an[64:65, 0:512], start=True, stop=True, skip=True)
                    MM(bc[:, 512:1024], onesP[64:65, 0:64],
                       an[64:65, 512:1024], start=True, stop=True, skip=True,
                       reuse=True)
                    rec = sm_pool.tile([64, 1024], BF, tag="rec", name="rec")
                    lnd = sm_pool.tile([64, 1024], F32, tag="lnd", name="lnd")
                    nc.scalar.activation(lnd[:], bc[:], AF.Ln)
                    nc.scalar.activation(rec[:], lnd[:], AF.Exp, scale=-1.0)
                    anp = avn_pool.tile([P, CW], BF, tag=f"avn{pair}",
                                        name=f"avn{pair}")
                    nc.vector.tensor_tensor(anp[0:64, :], an[0:64, 0:512],
                                            rec[:, 0:512], ALU.mult)
                    tmo = sm_pool.tile([64, CW], BF, tag="tmo", name="tmo")
                    nc.vector.tensor_tensor(tmo[:], an[0:64, 512:1024],
                                            rec[:, 512:1024], ALU.mult)
                    nc.sync.dma_start(out=anp[64:128, :], in_=tmo[:])
                    avn[pair] = anp

                for pair in range(NPAIR):
                    av = pav_pool.tile([65, 1024], F32, tag="av", name="av")
                    avt[pair] = av
                    for kt in range(nkt):
                        kws = slice(kt * P, (kt + 1) * P)
                        psc = psc_pool.tile([P, 1024], F32, tag="sc",
                                            name="psc")
                        MM(psc[:, 0:512], kT[pair][0:64, kws],
                           qT[pair][0:64, qs], start=True, stop=True,
                           skip=True)
                        MM(psc[:, 512:1024], kT[pair][64:128, kws],
                           qT[pair][64:128, qs], start=True, stop=True,
                           skip=True)
                        pt = pt_pool.tile([P, 1024], BF, tag="pt", name="pt")
                        nc.scalar.activation(pt[:], psc[:], AF.Exp,
                                             scale=SCALE)
                        jm = kt - mask_base
                        if 0 <= jm < 8:
                            mt = masks[(s, jm)]
                            nc.vector.tensor_tensor(pt[:, 0:512], pt[:, 0:512],
                                                    mt[:], ALU.mult)
                            nc.vector.tensor_tensor(pt[:, 512:1024],
                                                    pt[:, 512:1024],
                                                    mt[:], ALU.mult)
                        st = (kt == 0)
                        sp = (kt == nkt - 1)
                        MM(av[0:65, 0:512], v65[:, kt, pair, 0, :],
                           pt[:, 0:512], start=st, stop=sp, skip=True)
                        MM(av[0:65, 512:1024], v65[:, kt, pair, 1, :],
                           pt[:, 512:1024], start=st, stop=sp, skip=True)
                    if pair > 0:
                        _normalize(pair - 1)
                _normalize(NPAIR - 1)
                # Wo for this slot (bias bo folded into eviction); psum
                # shares the score slot so both slots pipeline in 8 banks
                for m in range(NPAIR):
                    ps = psc_pool.tile([P, CW], F32, tag="sc", name="wops")
                    for k in range(NPAIR):
                        MM(ps[:], womt[m][:, k, :], avn[k][:],
                           start=(k == 0), stop=(k == NPAIR - 1), skip=True)
                    nc.vector.tensor_scalar(oT[m][:, s, :], ps[:],
                                            bot[:, m:m + 1], None, ALU.add)
                    if s == 1:
                        # residual add interleaves with the remaining Wo
                        # matmuls so LN2 stats can start sooner
                        nc.vector.tensor_tensor(
                            x2T[m][:], xTq[m][:],
                            oT[m][:].rearrange("p s w -> p (s w)"), ALU.add)

        att_ctx.close()

        # =====================================================================
        # Phase 4: x2 = oT + xTq ; LN2 (not in place)
        # =====================================================================
        p_ln2T = top.enter_context(tc.tile_pool(name="p_ln2T", bufs=1))
        ln2T_tiles = [p_ln2T.tile([P, NQ], BF, tag=f"ln2T{kc}", name=f"ln2T{kc}")
                      for kc in range(KC)]
        ln2T = layernorm_T(x2T, NQ, "g2", "be2", "ln2T", out_tiles=ln2T_tiles)

        # =====================================================================
        # Phase 5: FFN in two hidden-dim halves
        # =====================================================================
        with ExitStack() as ph5:
            ff1_pool = ph5.enter_context(tc.tile_pool(name="ff1", bufs=1))
            facc_pool = ph5.enter_context(tc.tile_pool(name="facc", bufs=1))
            w1_pool = ph5.enter_context(tc.tile_pool(name="w1s", bufs=3))
            w2_pool = ph5.enter_context(tc.tile_pool(name="w2s", bufs=3))
            fst_pool = ph5.enter_context(tc.tile_pool(name="fst", bufs=3))
            pF = ph5.enter_context(tc.tile_pool(name="pF", bufs=4, space="PSUM"))
            ffacc = [facc_pool.tile([P, NQ], BF, tag=f"facc{m}", name=f"ffacc{m}")
                     for m in range(KC)]
            for half in range(2):
                ff1 = []
                for m in range(16):
                    mm_i = half * 16 + m
                    w1m = w1_pool.tile([P, KC, P], BF, tag="w1m", name="w1m")
                    nc.sync.dma_start(
                        out=w1m[:],
                        in_=w1_d[mm_i].rearrange("p (k m) -> p k m", k=KC))
                    f = ff1_pool.tile([P, NQ], BF, tag=f"f{m}", name=f"f{m}")
                    pss = [pF.tile([P, 512], F32, tag="proj", name="ps")
                           for _ in range(2)]
                    for kc in range(KC):
                        for tch in range(2):
                            MM(pss[tch][:], w1m[:, kc, :],
                               ln2T[kc][:, tch * 512:(tch + 1) * 512],
                               start=(kc == 0), stop=(kc == KC - 1), skip=True,
                               reuse=(tch > 0))
                    for tch in range(2):
                        s = slice(tch * 512, (tch + 1) * 512)
                        # relu(x + b1) eviction
                        nc.vector.tensor_scalar(f[:, s], pss[tch][:],
                                                b1t[:, mm_i:mm_i + 1],
                                                0.0, ALU.add, ALU.max)
                    ff1.append(f)
                for mc in range(KC):
                    w2m = w2_pool.tile([P, 16, P], BF, tag="w2m", name="w2m")
                    nc.sync.dma_start(
                        out=w2m[:],
                        in_=w2_d[mc][:, half * 2048:(half + 1) * 2048]
                        .rearrange("p (k m) -> p k m", k=16))
                    pss = [pF.tile([P, 512], F32, tag="proj", name="ps")
                           for _ in range(2)]
                    for kt in range(16):
                        for tch in range(2):
                            MM(pss[tch][:], w2m[:, kt, :],
                               ff1[kt][:, tch * 512:(tch + 1) * 512],
                               start=(kt == 0), stop=(kt == 15), skip=True,
                               reuse=(tch > 0))
                    for tch in range(2):
                        s = slice(tch * 512, (tch + 1) * 512)
                        if half == 0:
                            nc.scalar.activation(ffacc[mc][:, s], pss[tch][:],
                                                 AF.Copy)
                        else:
                            o = fst_pool.tile([P, 512], F32, tag="fo", name="fo")
                            nc.vector.tensor_scalar(o[:], pss[tch][:],
                                                    b2t[:, mc:mc + 1],
                                                    None, ALU.add)
                            nc.vector.tensor_tensor(o[:], o[:], ffacc[mc][:, s],
                                                    ALU.add)
                            nc.vector.tensor_tensor(o[:], o[:], x2T[mc][:, s],
                                                    ALU.add)
                            nc.sync.dma_start(out=out_d[mc * P:(mc + 1) * P, s],
                                              in_=o[:])

    nc.compile()
    return nc, names


_CACHE = {}


def _get_built():
    if "nc" not in _CACHE:
        _CACHE["nc"], _CACHE["names"] = _build()
    return _CACHE["nc"], _CACHE["names"]


def _host_inputs(x, Wq, Wk, Wv, Wo, bo, ln1_g, ln1_b, ln2_g, ln2_b, W1, b1, W2, b2):
    """Build the 8 per-core input maps (host work = sharding/layout only)."""
    from ml_dtypes import bfloat16
    f = np.float32

    def wtile(W, nmb, nkc):
        # [mb, p, kc*P_or_512] with [mb,p,kc*w+j] = W[kc*P+p, mb*wout+j]
        kin, cout = W.shape
        wout = cout // nmb
        return np.ascontiguousarray(
            W.reshape(nkc, P, nmb, wout).transpose(2, 1, 0, 3)
            .reshape(nmb, P, nkc * wout).astype(bfloat16))

    shared = {
        "wq": wtile(np.asarray(Wq, f), NPAIR, KC),
        "wk": wtile(np.asarray(Wk, f), NPAIR, KC),
        "wv": wtile(np.asarray(Wv, f), 2, KC),
        "wo": wtile(np.asarray(Wo, f), NPAIR, KC),
        "w1": wtile(np.asarray(W1, f), 32, KC),
        "w2": wtile(np.asarray(W2, f), NPAIR, 32),
        "bot": np.ascontiguousarray(np.asarray(bo, f).reshape(KC, P).T),
        "b1t": np.ascontiguousarray(np.asarray(b1, f).reshape(32, P).T),
        "b2t": np.ascontiguousarray(np.asarray(b2, f).reshape(KC, P).T),
        "g1c": np.ascontiguousarray(np.asarray(ln1_g, f).reshape(KC, P).T),
        "be1c": np.ascontiguousarray(np.asarray(ln1_b, f).reshape(KC, P).T),
        "g2c": np.ascontiguousarray(np.asarray(ln2_g, f).reshape(KC, P).T),
        "be2c": np.ascontiguousarray(np.asarray(ln2_b, f).reshape(KC, P).T),
        "onesC": np.ones((P, 1), bfloat16),
        "onesP": np.ones((P, P), bfloat16),
    }
    kl = np.arange(P)[:, None]
    ql = np.arange(CW)[None, :]
    in_maps = []
    for c in range(8):
        b, r = c // 2, c % 2
        gs = GSETS[r]
        xTb = np.ascontiguousarray(np.asarray(x[b], f).T.astype(bfloat16))
        qcols = np.concatenate([np.arange(CW * g, CW * (g + 1)) for g in gs])
        xTq = np.ascontiguousarray(xTb[:, qcols])
        m = np.empty((NSLOT, 8, P, CW), bfloat16)
        for s in range(NSLOT):
            q0 = CW * gs[s]
            base = 0 if s == 0 else 8
            for j in range(8):
                kt = base + j
                m[s, j] = ((P * kt + kl) <= (q0 + ql)).astype(bfloat16)
        im = dict(shared)
        im["xT"] = xTb
        im["xTq"] = xTq
        im["masks"] = m
        in_maps.append(im)
    return in_maps


def _unshard(outs):
    out = np.empty((4, T, C), np.float32)
    for c in range(8):
        b, r = c // 2, c % 2
        oT = outs[c]  # (C, NQ)
        for s, g in enumerate(GSETS[r]):
            out[b, CW * g:CW * (g + 1), :] = oT[:, CW * s:CW * (s + 1)].T
    return out


def kernel(**inputs):
    from concourse.bass_utils import run_bass_kernel_spmd
    from concourse.bass_interp import get_hw_module

    args = {k: np.asarray(v, np.float32) for k, v in inputs.items()}
    in_maps_named = _host_inputs(**args)

    nc, names = _get_built()
    in_maps = [{names[k]: v for k, v in im.items()} for im in in_maps_named]

    hw = get_hw_module(nc.m)
    old = nc.m
    nc.m = hw
    try:
        res = run_bass_kernel_spmd(nc, in_maps, core_ids=list(range(8)))
    finally:
        nc.m = old
    outs = [r[names["out"]] for r in res.results]
    return _unshard(outs)


if __name__ == "__main__":
    import reference
    inp = {k: np.asarray(v) for k, v in reference.setup_inputs().items()}
    got = kernel(**inp)
    exp = np.asarray(reference.reference(**inp))
    err = np.abs(got - exp).max() / np.abs(exp).max()
    print("Relative error:", err)
